# revision 1
# baseline (speedup 1.0000x reference)
"""Trainium2 Bass kernel for causal multi-head attention with RoPE + GQA.

Model: D_MODEL=1024, N_HEADS=16, NUM_KV_HEADS=4, D_K=64, B=4, T=2048.
Sharding (8 cores): core c -> batch b = c//2, head-group hg = c%2
(8 query heads / 2 kv heads per core). Each core computes a partial
output  y_partial = attn_out_local @ Wo[rows of its heads]  and the host
sums the two partials per batch (the tensor-parallel all-reduce happens
at gather time).

Device-side formulation (features-on-partitions "transposed" layout so
no on-chip transposes are needed; x arrives host-transposed):
  Q^T = Wq_s^T x^T   [512, 2048]     K^T = Wk_s^T x^T   [128, 2048]
  V'  = [x @ Wv_s | ones]            (lhsT = x^T column slices)
  RoPE via  q*cos + (R q)*sin  with R applied by one 128x128 matmul
  S^T[k,q] = K^T_h.T @ Q^T_h   row-packed head pairs (K=64 x2 groups)
  E^T = exp(S^T / 8)  on ScalarE, causal triangle masked on VectorE
  O'^T = V'_h.T @ E^T  PSUM-accumulated (M=65); output row 64 is the
         softmax denominator for free
  O^T = O'^T * recip(den)  (den broadcast via a DRAM-bounce DMA; head B
        assembled into oT partitions 64-127 by a partition-shifting DMA)
  y_partial = O^T.T @ Wo_s  (natural layout, contiguous DMA out)
Heads are paired (m, m+4) across the two kv groups so row-packed S^T
matmuls read distinct K^T partition halves; Wq columns / Wo rows are
permuted accordingly on the host. All matmul operands are float32r
(FP22 single-pass PE reads) - column tiling is unsupported for fp32r,
which is why PV uses M=65 instead of col-packed pairs.
"""

import numpy as np

D_MODEL = 1024
N_HEADS = 16
NUM_KV_HEADS = 4
D_K = 64
ROPE_BASE = 10000.0
B, T = 4, 2048
N_CORES = 8
KT = 16             # 128-row key tiles per sequence
QC = 4              # 512-col query chunks
DCH = 8             # 128-row feature (d_model) tiles

_PROGRAM = None     # cached compiled Bass program
LAST_RESULTS = None  # BassKernelResults of the most recent run


def _mm(nc, out, lhsT, rhs, **kw):
    nc.tensor.matmul(out, lhsT, rhs, **kw)


def _build_program(_DEBUG=False):
    import concourse.mybir as mybir
    import concourse.tile as tile
    from concourse import bacc

    f32 = mybir.dt.float32
    f32r = mybir.dt.float32r
    nc = bacc.Bacc("TRN2", target_bir_lowering=False, debug=False)

    xt_d = nc.dram_tensor("xt", [D_MODEL, T], f32, kind="ExternalInput")
    wq_d = nc.dram_tensor("wq", [D_MODEL, 512], f32, kind="ExternalInput")
    wk_d = nc.dram_tensor("wk", [D_MODEL, 128], f32, kind="ExternalInput")
    wv_d = nc.dram_tensor("wv", [D_MODEL, 128], f32, kind="ExternalInput")
    wo_d = nc.dram_tensor("wo", [512, D_MODEL], f32, kind="ExternalInput")
    rmat_d = nc.dram_tensor("rmat", [128, 128], f32, kind="ExternalInput")
    cos_d = nc.dram_tensor("costab", [128, T], f32, kind="ExternalInput")
    sin_d = nc.dram_tensor("sintab", [128, T], f32, kind="ExternalInput")
    tri_d = nc.dram_tensor("trimask", [128, 256], f32, kind="ExternalInput")
    ones_d = nc.dram_tensor("onesw", [128, 64], f32, kind="ExternalInput")
    y_d = nc.dram_tensor("y", [T, D_MODEL], f32, kind="ExternalOutput")
    dbg = {}
    if _DEBUG:
        dbg["qT0"] = nc.dram_tensor("dbg_qT0", [128, T], f32, kind="ExternalOutput")
        dbg["kT"] = nc.dram_tensor("dbg_kT", [128, T], f32, kind="ExternalOutput")
        dbg["v0"] = nc.dram_tensor("dbg_v0", [128, 130], f32, kind="ExternalOutput")
        dbg["e00"] = nc.dram_tensor("dbg_e00", [128, 1024], f32, kind="ExternalOutput")
        dbg["oT0"] = nc.dram_tensor("dbg_oT0", [128, T], f32, kind="ExternalOutput")
        dbg["oA"] = nc.dram_tensor("dbg_oA", [65, 512], f32, kind="ExternalOutput")


    mult = mybir.AluOpType.mult
    add = mybir.AluOpType.add

    with tile.TileContext(nc) as tc:
        with (
            tc.tile_pool(name="big", bufs=13) as big,
            tc.tile_pool(name="w", bufs=8) as wp,
            tc.tile_pool(name="const", bufs=1) as constp,
            tc.tile_pool(name="vt", bufs=16) as vtp,
            tc.tile_pool(name="cs", bufs=2) as csp,
            tc.tile_pool(name="tmp", bufs=2) as tmpp,
            tc.tile_pool(name="e", bufs=3) as ep,
            tc.tile_pool(name="rr", bufs=1) as rrp,
            tc.tile_pool(name="rb", bufs=1) as rbp,
            tc.tile_pool(name="ysb", bufs=2) as ysbp,
            tc.tile_pool(name="dr", bufs=2, space="DRAM") as drp,
            tc.tile_pool(name="ps_g", bufs=1, space="PSUM") as psg,
            tc.tile_pool(name="ps_s", bufs=2, space="PSUM") as pss,
            tc.tile_pool(name="ps_o", bufs=2, space="PSUM") as pso,
        ):
            # ---- load inputs: wk + consts, then xt (K^T proj starts as
            # soon as wk[k] + xt[k] land), then wq/wv/wo
            wk_sb = []
            for k in range(DCH):
                wkt = wp.tile([128, 128], f32r, tag="wk", bufs=8, name=f"wk{k}")
                nc.sync.dma_start(
                    wkt[:], wk_d[128 * k : 128 * (k + 1), :].bitcast(f32r)
                )
                wk_sb.append(wkt)
            rmat_sb = constp.tile([128, 128], f32r, tag="rmat", name="rmat_sb")
            nc.sync.dma_start(rmat_sb[:], rmat_d[:].bitcast(f32r))
            tri_sb = constp.tile([128, 256], f32r, tag="tri", name="tri_sb")
            nc.sync.dma_start(tri_sb[:], tri_d[:].bitcast(f32r))
            ones_sb = constp.tile([128, 64], f32r, tag="ones", name="ones_sb")
            nc.sync.dma_start(ones_sb[:], ones_d[:].bitcast(f32r))
            xt_sb = []
            for k in range(DCH):
                xtt = big.tile([128, T], f32r, tag="big", name=f"xt{k}")
                xt_sb.append(xtt)
            # qc-major quarters: the (m, qc) projection groups only need the
            # qc column slice of every k-tile, so this ordering lets the
            # first groups start ~6us into the 8 MB x^T load instead of 23us
            for qc in range(QC):
                cs_ = slice(512 * qc, 512 * (qc + 1))
                for k in range(DCH):
                    nc.sync.dma_start(
                        xt_sb[k][:, cs_],
                        xt_d[128 * k : 128 * (k + 1), cs_].bitcast(f32r),
                    )
            wq_sb, wv_sb = [], []
            for k in range(DCH):
                wqt = wp.tile([128, 512], f32r, tag="wq", bufs=8, name=f"wq{k}")
                nc.sync.dma_start(
                    wqt[:], wq_d[128 * k : 128 * (k + 1), :].bitcast(f32r)
                )
                wq_sb.append(wqt)
                wvt = wp.tile([128, 128], f32r, tag="wv", bufs=8, name=f"wv{k}")
                nc.sync.dma_start(
                    wvt[:], wv_d[128 * k : 128 * (k + 1), :].bitcast(f32r)
                )
                wv_sb.append(wvt)
            wo_sb = []
            for c in range(4):
                wot = wp.tile([128, 1024], f32r, tag="wo", bufs=4, name=f"wo{c}")
                nc.sync.dma_start(
                    wot[:], wo_d[128 * c : 128 * (c + 1), :].bitcast(f32r)
                )
                wo_sb.append(wot)

            # ---- fused projection + RoPE ------------------------------
            def project_rope(w_tiles, mslice, dst):
                """dst = RoPE(w^T x^T) for one 128-partition chunk."""
                for qc in range(QC):
                    cs_ = slice(512 * qc, 512 * (qc + 1))
                    ps = pso.tile([128, 512], f32, tag="o", bufs=3, name="ps_proj")
                    for k in range(DCH):
                        _mm(
                            nc,
                            ps[:],
                            w_tiles[k][:, mslice] if mslice else w_tiles[k][:],
                            xt_sb[k][:, cs_],
                            start=(k == 0),
                            stop=(k == DCH - 1),
                        )
                    nc.scalar.copy(dst[:, cs_], ps[:])
                for qc in range(QC):
                    cs_ = slice(512 * qc, 512 * (qc + 1))
                    rot = psg.tile([128, 512], f32, tag="psg", name="ps_rot")
                    _mm(nc, rot[:], rmat_sb[:], dst[:, cs_], start=True, stop=True)
                    cos_t = csp.tile([128, 512], f32, tag="cos", name="cos_t")
                    nc.sync.dma_start(cos_t[:], cos_d[:, cs_])
                    sin_t = csp.tile([128, 512], f32, tag="sin", name="sin_t")
                    nc.sync.dma_start(sin_t[:], sin_d[:, cs_])
                    t1 = tmpp.tile([128, 512], f32, tag="t1", bufs=1, name="t1")
                    nc.vector.tensor_tensor(t1[:], rot[:], sin_t[:], mult)
                    nc.gpsimd.tensor_tensor(dst[:, cs_], dst[:, cs_], cos_t[:], mult)
                    nc.vector.tensor_tensor(dst[:, cs_], dst[:, cs_], t1[:], add)

            kT = big.tile([128, T], f32r, tag="big", name="kT")
            project_rope(wk_sb, None, kT)
            if _DEBUG:
                nc.sync.dma_start(dbg["kT"][:].bitcast(f32r), kT[:])
            def v_proj(t):
                ps = pso.tile([128, 512], f32, tag="o", bufs=3, name="ps_v")
                for k in range(DCH):
                    _mm(
                        nc,
                        ps[:, 0:128],
                        xt_sb[k][:, 128 * t : 128 * (t + 1)],
                        wv_sb[k][:],
                        start=(k == 0),
                        stop=(k == DCH - 1),
                    )
                vt = vtp.tile([128, 130], f32r, tag="v", bufs=16, name=f"v{t}")
                nc.vector.tensor_copy(vt[:, 0:64], ps[:, 0:64])
                nc.vector.tensor_copy(vt[:, 64:65], ones_sb[:, 0:1])
                nc.vector.tensor_copy(vt[:, 65:129], ps[:, 64:128])
                nc.vector.tensor_copy(vt[:, 129:130], ones_sb[:, 0:1])
                if _DEBUG and t == 0:
                    nc.sync.dma_start(dbg["v0"][:].bitcast(f32r), vt[:])
                v_sb.append(vt)

            v_sb = []
            qT = []

            def q_proj(m):
                qTm = big.tile([128, T], f32r, tag="big", name=f"qT{m}")
                project_rope(wq_sb, slice(128 * m, 128 * (m + 1)), qTm)
                if _DEBUG and m == 0:
                    nc.sync.dma_start(dbg["qT0"][:].bitcast(f32r), qTm[:])
                qT.append(qTm)

            # ---- attention + output projection, per q-chunk -----------
            oT = [
                big.tile([128, T], f32r, tag="big", name=f"oT{m}") for m in range(4)
            ]
            tri3 = tri_sb[:].rearrange("p (two q) -> p two q", two=2)
            escale = float(1.0 / np.sqrt(D_K))

            def y_tile(t, last):
                """output projection for one 128-row token tile."""
                ty = ysbp.tile([128, 1024], f32, tag="y", name="ty")
                for nh in range(2):
                    if last:
                        ps = pso.tile([128, 512], f32, tag="o", bufs=3, name="ps_y")
                    else:
                        ps = psg.tile([128, 512], f32, tag="psg", name="ps_y")
                    for c in range(4):
                        _mm(
                            nc,
                            ps[:],
                            oT[c][:, 128 * t : 128 * (t + 1)],
                            wo_sb[c][:, 512 * nh : 512 * (nh + 1)],
                            start=(c == 0),
                            stop=(c == 3),
                        )
                    nc.vector.tensor_copy(ty[:, 512 * nh : 512 * (nh + 1)], ps[:])
                nc.sync.dma_start(y_d[128 * t : 128 * (t + 1), :], ty[:])

            for t in range(KT):
                v_proj(t)
            for m in range(4):
                q_proj(m)

            qp_order = [0, 1, 2, 3]
            for qi, qp in enumerate(qp_order):
                qsl = slice(512 * qp, 512 * (qp + 1))
                for hp in range(4):
                    oA = pso.tile([128, 512], f32, tag="o", bufs=3, name="oA")
                    oB = pso.tile([128, 512], f32, tag="o", bufs=3, name="oB")
                    nkt = 4 * qp + 4
                    for kt in range(nkt):
                        a = max(0, 128 * kt - 512 * qp)
                        ksl = slice(128 * kt, 128 * (kt + 1))
                        qsl_t = slice(512 * qp + a, 512 * (qp + 1))
                        s = pss.tile([128, 1024], f32, tag="s", name="s")
                        _mm(
                            nc,
                            s[:, a:512],
                            kT[0:64, ksl],
                            qT[hp][0:64, qsl_t],
                            start=True,
                            stop=True,
                            tile_position=(0, 0),
                        )
                        _mm(
                            nc,
                            s[:, 512 + a : 1024],
                            kT[64:128, ksl],
                            qT[hp][64:128, qsl_t],
                            start=True,
                            stop=True,
                            tile_position=(64, 0),
                        )
                        e = ep.tile([128, 1024], f32r, tag="e", name="e")
                        if a:
                            # one strided op over both heads' valid columns
                            sv = s[:].rearrange("p (two q) -> p two q", two=2)[
                                :, :, a:512
                            ]
                            ev = e[:].rearrange("p (two q) -> p two q", two=2)[
                                :, :, a:512
                            ]
                            nc.scalar.activation(
                                out=ev,
                                in_=sv,
                                func=mybir.ActivationFunctionType.Exp,
                                scale=escale,
                            )
                        else:
                            nc.scalar.activation(
                                out=e[:],
                                in_=s[:],
                                func=mybir.ActivationFunctionType.Exp,
                                scale=escale,
                            )
                        if kt >= 4 * qp:  # diagonal: causal triangle mask
                            o = 128 * kt - 512 * qp
                            e3 = e[:].rearrange("p (two q) -> p two q", two=2)[
                                :, :, o : o + 128
                            ]
                            nc.vector.tensor_tensor(e3, e3, tri3, mult)
                        if _DEBUG and qp == 0 and hp == 0 and kt == 0:
                            nc.sync.dma_start(dbg["e00"][:].bitcast(f32r), e[:])
                        st, sp = (kt == 0), (kt == nkt - 1)
                        # V' = [V | ones]: output row 64 accumulates the
                        # softmax denominator (M=65 -> no col tiling)
                        _mm(
                            nc,
                            oA[0:65, a:512],
                            v_sb[kt][:, 0:65],
                            e[:, a:512],
                            start=st,
                            stop=sp,
                            skip_group_check=True,
                        )
                        _mm(
                            nc,
                            oB[0:65, a:512],
                            v_sb[kt][:, 65:130],
                            e[:, 512 + a : 1024],
                            start=st,
                            stop=sp,
                            skip_group_check=True,
                        )
                    # evacuate O' to SBUF fast so the PSUM banks free up
                    # for the next head pair; normalize from the SBUF copy.
                    # recip AFTER broadcast: the custom DVE op only works at
                    # partition base 0 (broadcast via a DRAM bounce)
                    oraw = rrp.tile([128, 1024], f32, tag="rr", bufs=2, name="oraw")
                    nc.vector.tensor_copy(oraw[0:65, 0:512], oA[0:65, :])
                    nc.vector.tensor_copy(oraw[0:65, 512:1024], oB[0:65, :])
                    if _DEBUG and qp == 0 and hp == 0:
                        nc.sync.dma_start(dbg["oA"][:], oraw[0:65, 0:512])
                    rb = rbp.tile([128, 1024], f32, tag="rb", bufs=2, name="rb")
                    scr = drp.tile([1, 1024], f32, tag="scr", name="scr")
                    nc.sync.dma_start(scr[:], oraw[64:65, :])
                    nc.sync.dma_start(
                        rb[0:64, :], scr[:].to_broadcast((64, 1024))
                    )
                    nc.vector.reciprocal_approx_fast(rb[0:64, :], rb[0:64, :])
                    nc.vector.tensor_tensor(
                        oT[hp][0:64, qsl], oraw[0:64, 0:512], rb[0:64, 0:512], mult
                    )
                    nb = tmpp.tile([128, 512], f32r, tag="nb", bufs=2, name="nb")
                    nc.vector.tensor_tensor(
                        nb[0:64, :], oraw[0:64, 512:1024], rb[0:64, 512:1024], mult
                    )
                    # head B lives at oT partitions 64-127: partition-shift DMA
                    nc.sync.dma_start(oT[hp][64:128, qsl], nb[0:64, :])
                    if qi > 0:
                        # spread the previously processed q-chunk's output
                        # projection into this (ACT-paced) chunk's hp slots
                        y_tile(4 * qp_order[qi - 1] + hp, last=False)
                if _DEBUG and qp == QC - 1:
                    nc.sync.dma_start(dbg["oT0"][:].bitcast(f32r), oT[0][:])


            for t in range(4 * qp_order[-1], 4 * qp_order[-1] + 4):
                y_tile(t, last=True)

    nc.compile()
    return nc


def _get_program():
    global _PROGRAM
    if _PROGRAM is None:
        _PROGRAM = _build_program()
    return _PROGRAM


def _host_tables():
    """cos/sin [128, T] (two stacked 64-row copies), R^T (lhsT), tri mask."""
    d = D_K
    inv_freq = 1.0 / (ROPE_BASE ** (np.arange(0, d, 2, dtype=np.float32) / d))
    ang = np.arange(T, dtype=np.float32)[:, None] * inv_freq[None, :]  # [T, 32]
    cos64 = np.repeat(np.cos(ang).astype(np.float32), 2, axis=1).T.copy()
    sin64 = np.repeat(np.sin(ang).astype(np.float32), 2, axis=1).T.copy()
    cos128 = np.ascontiguousarray(np.concatenate([cos64, cos64], axis=0))
    sin128 = np.ascontiguousarray(np.concatenate([sin64, sin64], axis=0))
    # rot = R @ q with rot[2i] = -q[2i+1], rot[2i+1] = q[2i]; pass lhsT = R^T
    R = np.zeros((128, 128), dtype=np.float32)
    for i in range(64):
        R[2 * i, 2 * i + 1] = -1.0
        R[2 * i + 1, 2 * i] = 1.0
    rmat = np.ascontiguousarray(R.T)
    tri = np.triu(np.ones((128, 128), dtype=np.float32))  # keep kk <= qq
    tri2 = np.ascontiguousarray(np.concatenate([tri, tri], axis=1))
    return cos128, sin128, rmat, tri2


def _head_perm():
    """chunk m holds local heads (m, m+4) -> permute Wq cols / Wo rows."""
    perm = []
    for m in range(4):
        perm.extend(range(64 * m, 64 * m + 64))
        perm.extend(range(64 * (m + 4), 64 * (m + 4) + 64))
    return np.array(perm)


def make_in_maps(x, Wq, Wk, Wv, Wo):
    cos128, sin128, rmat, tri2 = _host_tables()
    perm = _head_perm()
    in_maps = []
    for c in range(N_CORES):
        b, hg = c // 2, c % 2
        in_maps.append(
            {
                "xt": np.ascontiguousarray(x[b].T),
                "wq": np.ascontiguousarray(Wq[:, hg * 512 : (hg + 1) * 512][:, perm]),
                "wk": np.ascontiguousarray(Wk[:, hg * 128 : (hg + 1) * 128]),
                "wv": np.ascontiguousarray(Wv[:, hg * 128 : (hg + 1) * 128]),
                "wo": np.ascontiguousarray(Wo[hg * 512 : (hg + 1) * 512, :][perm, :]),
                "rmat": rmat,
                "costab": cos128,
                "sintab": sin128,
                "trimask": tri2,
                "onesw": np.ones((128, 64), dtype=np.float32),
            }
        )
    return in_maps


def kernel(x, attention_mask, Wq, Wk, Wv, Wo, _trace=False, _trace_kwargs=None):
    global LAST_RESULTS
    from concourse import bass_utils

    x = np.asarray(x, dtype=np.float32)
    Wq = np.asarray(Wq, dtype=np.float32)
    Wk = np.asarray(Wk, dtype=np.float32)
    Wv = np.asarray(Wv, dtype=np.float32)
    Wo = np.asarray(Wo, dtype=np.float32)

    nc = _get_program()
    in_maps = make_in_maps(x, Wq, Wk, Wv, Wo)
    res = bass_utils.run_bass_kernel_spmd(
        nc,
        in_maps,
        core_ids=list(range(N_CORES)),
        trace=_trace,
        **(_trace_kwargs or {}),
    )
    LAST_RESULTS = res

    y = np.zeros((B, T, D_MODEL), dtype=np.float32)
    for b in range(B):
        y[b] = res.results[2 * b]["y"] + res.results[2 * b + 1]["y"]

    # faithful handling of padded (attention_mask == 0) query rows: the
    # reference's mask makes those rows uniform attention over ALL keys.
    am = np.asarray(attention_mask)
    if not np.all(am == 1):
        rep = N_HEADS // NUM_KV_HEADS
        for b in range(B):
            rows = np.where(am[b] == 0)[0]
            if rows.size:
                V = x[b] @ Wv
                Vfull = np.repeat(
                    V.reshape(T, NUM_KV_HEADS, D_K), rep, axis=1
                ).reshape(T, D_MODEL)
                y[b, rows] = (Vfull.mean(axis=0) @ Wo)[None, :]
    return y



# revision 28
# speedup vs baseline: 1.2170x; 1.2170x over previous
"""Trainium2 Bass kernel for causal multi-head attention with RoPE + GQA.

Model: D_MODEL=1024, N_HEADS=16, NUM_KV_HEADS=4, D_K=64, B=4, T=2048.
Sharding (8 cores): core c -> batch b = c//2, head-group hg = c%2
(8 query heads / 2 kv heads per core). Each core computes a partial
output  y_partial = attn_out_local @ Wo[rows of its heads]  and the host
sums the two partials per batch (the tensor-parallel all-reduce happens
at gather time).

Perf design (cost-model driven; ACT exp ~147us is the floor engine):
  - All GEMMs bf16 (1 PE cycle/row) except S = K^T.T Q^T, which runs in
    fp8e4m3 MatmulPerfMode.DoubleRow (0.5 cycles/row) with the d_k=64
    contraction laid out [32 partitions, 2 k-subtiles]. fp32 PSUM
    accumulation everywhere; end-to-end rel_err ~5e-3 (gate 2e-2).
  - Attention is emitted HEAD-PAIR-MAJOR with the q-chunk projections
    interleaved, so ScalarE starts exp'ing ~23us in and stays saturated
    while PE computes the remaining projections underneath it.
  - Each (qp, hp) unit's normalization tail (denominator DRAM-bounce
    broadcast, reciprocal, scale, head-B partition-shift DMA, and the
    previous q-chunk's output projection) is DEFERRED into the next
    unit's kt loop so it never sits between PV and the next S matmul.
  - DMA instruction COUNT is precious (~625ns serialized descriptor-gen
    each): all host inputs are pre-packed for single contiguous DMAs,
    x^T is split qc-major in 4 so the first projections start ~4us in,
    cos/sin load once in bf16.
  - Engine placement: ACT = exp only; Pool = PSUM->SBUF staging + RoPE
    cos-mult; DVE = RoPE sin-mult/add, masking, reciprocal, normalize.

Formulation (features-on-partitions; x arrives host-transposed bf16):
  Q^T = Wq_s^T x^T  [512,2048]    K^T = Wk_s^T x^T  [128,2048]
  V'  = [x @ Wv_s | ones]
  RoPE q*cos + (R q)*sin, R applied by one 128x128 matmul; result
  written fp8e4 and DMA-folded to the DoubleRow [32, (2h+half)*T + t]
  layout.
  S^T = K^T_h.T Q^T_h (fp8 DoubleRow), E^T = exp(S^T/8) -> bf16 (ACT),
  causal triangle on DVE, O'^T = V'_h.T E^T (M=65; row 64 = softmax
  denominator), O^T = O'^T * recip(den), y = O^T.T Wo_s -> bf16 DMA.
Heads are paired (m, m+4) across the two kv groups; Wq columns / Wo
rows are permuted accordingly on the host.
"""

import numpy as np

D_MODEL = 1024
N_HEADS = 16
NUM_KV_HEADS = 4
D_K = 64
ROPE_BASE = 10000.0
B, T = 4, 2048
N_CORES = 8
KT = 16             # 128-row key tiles per sequence
QC = 4              # 512-col query chunks
DCH = 8             # 128-row feature (d_model) tiles

_PROGRAM = None     # cached compiled Bass program
LAST_RESULTS = None  # BassKernelResults of the most recent run


def _mm(nc, out, lhsT, rhs, **kw):
    nc.tensor.matmul(out, lhsT, rhs, **kw)


def _build_program():
    import concourse.mybir as mybir
    import concourse.tile as tile
    from concourse import bacc

    f32 = mybir.dt.float32
    f32r = mybir.dt.float32r
    bf16 = mybir.dt.bfloat16
    f8 = mybir.dt.float8e4
    nc = bacc.Bacc("TRN2", target_bir_lowering=False, debug=False)

    # every input pre-packed on host; xt split qc-major in 4
    xt_d = nc.dram_tensor("xtp", [128, DCH * T], bf16, kind="ExternalInput")
    wq_d = nc.dram_tensor("wqp", [128, DCH * 512], bf16, kind="ExternalInput")
    wk_d = nc.dram_tensor("wkp", [128, DCH * 128], bf16, kind="ExternalInput")
    wv_d = nc.dram_tensor("wvp", [128, DCH * 128], bf16, kind="ExternalInput")
    wo_d = nc.dram_tensor("wop", [128, 4 * 1024], bf16, kind="ExternalInput")
    cb_d = nc.dram_tensor("constb", [128, 392], bf16, kind="ExternalInput")
    cos_d = nc.dram_tensor("costab", [128, T], bf16, kind="ExternalInput")
    sin_d = nc.dram_tensor("sintab", [128, T], bf16, kind="ExternalInput")
    y_d = nc.dram_tensor("y", [T, D_MODEL], bf16, kind="ExternalOutput")

    mult = mybir.AluOpType.mult
    add = mybir.AluOpType.add
    div = mybir.AluOpType.divide
    DR = mybir.MatmulPerfMode.DoubleRow

    with tile.TileContext(nc) as tc:
        with (
            tc.tile_pool(name="big", bufs=6) as big,
            tc.tile_pool(name="w", bufs=4) as wp,
            tc.tile_pool(name="const", bufs=1) as constp,
            tc.tile_pool(name="q8", bufs=5) as q8p,
            tc.tile_pool(name="vt", bufs=16) as vtp,
            tc.tile_pool(name="dst", bufs=5) as dstp,
            tc.tile_pool(name="tmp", bufs=2) as tmpp,
            tc.tile_pool(name="e", bufs=3) as ep,
            tc.tile_pool(name="rr", bufs=1) as rrp,
            tc.tile_pool(name="rb", bufs=1) as rbp,
            tc.tile_pool(name="ysb", bufs=2) as ysbp,
            tc.tile_pool(name="dr", bufs=2, space="DRAM") as drp,
            tc.tile_pool(name="ps_g", bufs=2, space="PSUM") as psg,
            tc.tile_pool(name="ps_s", bufs=2, space="PSUM") as pss,
            tc.tile_pool(name="ps_o", bufs=2, space="PSUM") as pso,
        ):
            # ---- input loads, ordered for earliest compute start ------
            cb = constp.tile([128, 392], bf16, tag="cb", name="cb")
            nc.sync.dma_start(cb[:], cb_d[:])
            xt_all = big.tile([128, DCH * T], bf16, tag="xt", bufs=1, name="xt_all")
            xt3 = xt_all[:].rearrange("p (k t) -> p k t", k=DCH)
            xtd3 = xt_d[:].rearrange("p (k t) -> p k t", k=DCH)
            cs0 = slice(0, 512)
            nc.sync.dma_start(xt3[:, :, cs0], xtd3[:, :, cs0])
            wk_all = wp.tile([128, DCH * 128], bf16, tag="wk", bufs=1, name="wk_all")
            nc.sync.dma_start(wk_all[:], wk_d[:])
            wq_all = wp.tile([128, DCH * 512], bf16, tag="wq", bufs=1, name="wq_all")
            nc.sync.dma_start(wq_all[:, 0:1024], wq_d[:, 0:1024])
            cos_sb = constp.tile([128, T], bf16, tag="cos", name="cos_sb")
            nc.sync.dma_start(cos_sb[:], cos_d[:])
            sin_sb = constp.tile([128, T], bf16, tag="sin", name="sin_sb")
            nc.sync.dma_start(sin_sb[:], sin_d[:])
            wv_all = wp.tile([128, DCH * 128], bf16, tag="wv", bufs=1, name="wv_all")
            nc.sync.dma_start(wv_all[:], wv_d[:])
            for qc in range(1, QC):
                cs_ = slice(512 * qc, 512 * (qc + 1))
                nc.sync.dma_start(xt3[:, :, cs_], xtd3[:, :, cs_])
            nc.sync.dma_start(wq_all[:, 1024:4096], wq_d[:, 1024:4096])
            wo_all = wp.tile([128, 4 * 1024], bf16, tag="wo", bufs=1, name="wo_all")
            nc.sync.dma_start(wo_all[:], wo_d[:])

            def xt(k):
                return xt_all[:, T * k : T * (k + 1)]

            def wqm(m, k):
                # m-major packing: [128, 1024*m + 128*k + j]
                return wq_all[:, 1024 * m + 128 * k : 1024 * m + 128 * (k + 1)]

            def wk(k):
                return wk_all[:, 128 * k : 128 * (k + 1)]

            def wv(k):
                return wv_all[:, 128 * k : 128 * (k + 1)]

            def wo(c):
                return wo_all[:, 1024 * c : 1024 * (c + 1)]

            rmat_sb = cb[:, 0:128]
            tri_sb = cb[:, 128:384]
            ones_bf = cb[:, 384:392]

            pending = []      # head-phase rope tails: flushed whole
            pending_work = []  # steady-state closures: flushed 1 per kt

            def flush_pending():
                for f in pending:
                    f()
                pending.clear()

            def flush_work(n=1):
                for _ in range(n):
                    if not pending_work:
                        return
                    pending_work.pop(0)()

            # PE pstate warm-up: the cost model charges LOW/MID clocks to
            # matmuls decoded within 3us of an idle->busy transition, so
            # keep PE trivially busy across head-phase DMA waits.
            def warm(n, ring="psg"):
                if ring == "o":
                    wt = pso.tile([128, 512], f32, tag="o", bufs=2, name="warm")
                else:
                    wt = psg.tile([128, 512], f32, tag="psg", bufs=2, name="warm")
                for _ in range(n):
                    _mm(
                        nc,
                        wt[0:64, 0:64],
                        cb[:, 0:64],
                        cb[:, 0:64],
                        start=True,
                        stop=True,
                        skip_group_check=True,
                    )

            # ---- fused projection + RoPE -> fp8 DoubleRow layout ------
            # dr layout: [32 partitions, (2*head + khalf)*T + t]
            # The rope tail (rot matmul + cos/sin combine) of chunk qc is
            # deferred until after chunk qc+1's projection matmuls: the
            # tile scheduler is run-ahead in-order-with-skip per engine,
            # so an op emitted before its input is ready gets parked
            # until the engine idles (which PE never does).
            def project_rope(wsel, dr_dst, head=False, fold_each=False):
                q8full = q8p.tile([128, T], f8, tag="q8f", bufs=2, name="q8full")

                def make_tail(qc, ps):
                    cs_ = slice(512 * qc, 512 * (qc + 1))
                    dst = dstp.tile([128, 512], bf16, tag="dst", bufs=5, name="dst")
                    if head:
                        nc.scalar.copy(dst[:], ps[:])
                    else:
                        nc.vector.tensor_copy(dst[:], ps[:])
                    # cos-multiply needs only dst: run it right away on Pool
                    c1 = tmpp.tile([128, 512], f32, tag="c1", bufs=4, name="c1")
                    nc.gpsimd.tensor_tensor(c1[:], dst[:], cos_sb[:, cs_], mult)

                    def tail():
                        rot = psg.tile([128, 512], f32, tag="psg", bufs=2, name="ps_rot")
                        _mm(nc, rot[:], rmat_sb, dst[:], start=True, stop=True)
                        t1 = tmpp.tile([128, 512], f32, tag="t1", bufs=4, name="t1")
                        nc.vector.tensor_tensor(t1[:], rot[:], sin_sb[:, cs_], mult)
                        nc.vector.tensor_tensor(q8full[:, cs_], c1[:], t1[:], add)
                        # partition fold into DoubleRow layout; per-qc for
                        # the head tiles so attention starts on partial K/Q
                        if fold_each:
                            for g in range(4):
                                nc.sync.dma_start(
                                    dr_dst[0:32, T * g + 512 * qc : T * g + 512 * (qc + 1)],
                                    q8full[32 * g : 32 * (g + 1), cs_],
                                )
                        elif qc == QC - 1:
                            for g in range(4):
                                nc.sync.dma_start(
                                    dr_dst[0:32, T * g : T * (g + 1)],
                                    q8full[32 * g : 32 * (g + 1), :],
                                )

                    return tail

                prev_tail = None
                for qc in range(QC):
                    cs_ = slice(512 * qc, 512 * (qc + 1))
                    if head and qc % 2 == 0:
                        ps = pso.tile([128, 512], f32, tag="o", bufs=2, name="ps_proj")
                    else:
                        ps = psg.tile([128, 512], f32, tag="psg", bufs=2, name="ps_proj")
                    for k in range(DCH):
                        _mm(
                            nc,
                            ps[:],
                            wsel(k),
                            xt(k)[:, cs_],
                            start=(k == 0),
                            stop=(k == DCH - 1),
                        )
                    if qc == 0:
                        # previous projection's last rope tail rides right
                        # behind this chunk's matmuls on the PE queue
                        flush_pending()
                    if head:
                        warm(18)
                    if prev_tail is not None:
                        prev_tail()
                    prev_tail = make_tail(qc, ps)
                pending.append(prev_tail)

            v_all = vtp.tile([128, KT * 130], bf16, tag="v", bufs=1, name="v_all")
            v3 = v_all[:].rearrange("p (t c) -> p t c", t=KT)
            nc.vector.memset(v3[:, :, 64:65], 1.0)
            nc.vector.memset(v3[:, :, 129:130], 1.0)

            def v_proj(t, ring=None):
                if ring == "o":
                    ps = pso.tile([128, 512], f32, tag="o", bufs=2, name="ps_v")
                else:
                    ps = psg.tile([128, 512], f32, tag="psg", bufs=2, name="ps_v")
                for k in range(DCH):
                    _mm(
                        nc,
                        ps[:, 0:128],
                        xt(k)[:, 128 * t : 128 * (t + 1)],
                        wv(k),
                        start=(k == 0),
                        stop=(k == DCH - 1),
                    )
                vt = v_all[:, 130 * t : 130 * (t + 1)]
                nc.vector.tensor_copy(vt[:, 0:64], ps[:, 0:64])
                nc.vector.tensor_copy(vt[:, 65:129], ps[:, 64:128])
                v_sb.append(vt)

            v_sb = []
            qT8 = []

            def q_proj(m, head=False):
                qt = q8p.tile([32, 4 * T], f8, tag="dr", bufs=5, name=f"qT8{m}")
                project_rope(
                    lambda k: wqm(m, k), qt, head=head, fold_each=head
                )
                qT8.append(qt)

            def q_proj_deferred(m):
                """emit q-chunk m's projection as per-qc closures so the
                attention kt loop interleaves them 1.7us at a time."""
                qt = q8p.tile([32, 4 * T], f8, tag="dr", bufs=5, name=f"qT8{m}")
                qT8.append(qt)
                wsel = lambda k: wqm(m, k)
                q8full = q8p.tile([128, T], f8, tag="q8f", bufs=2, name="q8full")
                state = {"tail": None}

                def make_qc(qc):
                    cs_ = slice(512 * qc, 512 * (qc + 1))

                    def go():
                        ps = psg.tile(
                            [128, 512], f32, tag="psg", bufs=2, name="ps_proj"
                        )
                        for k in range(DCH):
                            _mm(
                                nc,
                                ps[:],
                                wsel(k),
                                xt(k)[:, cs_],
                                start=(k == 0),
                                stop=(k == DCH - 1),
                            )
                        if state["tail"] is not None:
                            state["tail"]()
                        dst = dstp.tile(
                            [128, 512], bf16, tag="dst", bufs=5, name="dst"
                        )
                        nc.vector.tensor_copy(dst[:], ps[:])
                        c1 = tmpp.tile([128, 512], f32, tag="c1", bufs=4, name="c1")
                        nc.gpsimd.tensor_tensor(c1[:], dst[:], cos_sb[:, cs_], mult)

                        def tail():
                            rot = psg.tile(
                                [128, 512], f32, tag="psg", bufs=2, name="ps_rot"
                            )
                            _mm(nc, rot[:], rmat_sb, dst[:], start=True, stop=True)
                            t1 = tmpp.tile(
                                [128, 512], f32, tag="t1", bufs=4, name="t1"
                            )
                            nc.vector.tensor_tensor(
                                t1[:], rot[:], sin_sb[:, cs_], mult
                            )
                            nc.vector.tensor_tensor(q8full[:, cs_], c1[:], t1[:], add)
                            if qc == QC - 1:
                                for g in range(4):
                                    nc.sync.dma_start(
                                        qt[0:32, T * g : T * (g + 1)],
                                        q8full[32 * g : 32 * (g + 1), :],
                                    )

                        state["tail"] = tail

                    return go

                for qc in range(QC):
                    pending_work.append(make_qc(qc))
                pending_work.append(lambda: (state["tail"](), state.update(tail=None)))

            oT = [
                big.tile([128, T], bf16, tag="oT", bufs=4, name=f"oT{m}")
                for m in range(4)
            ]
            tri3 = tri_sb.rearrange("p (two q) -> p two q", two=2)
            escale = float(1.0 / np.sqrt(D_K))

            def y_tile(t):
                """output projection for one 128-row token tile. nh=0 uses
                the 'o' psum ring, nh=1 the 'psg' ring (parallel banks)."""
                ty = ysbp.tile([128, 1024], bf16, tag="y", name="ty")
                for nh in range(2):
                    ps = psg.tile([128, 512], f32, tag="psg", bufs=2, name="ps_y")
                    for c in range(4):
                        _mm(
                            nc,
                            ps[:],
                            oT[c][:, 128 * t : 128 * (t + 1)],
                            wo(c)[:, 512 * nh : 512 * (nh + 1)],
                            start=(c == 0),
                            stop=(c == 3),
                        )
                    nc.vector.tensor_copy(ty[:, 512 * nh : 512 * (nh + 1)], ps[:])
                nc.sync.dma_start(y_d[128 * t : 128 * (t + 1), :], ty[:])

            # ---- attention unit (one q-chunk x one head-pair) ---------
            kv4 = [None]
            escale_f = escale

            def attn_unit(qp, hp):
                qsl = slice(512 * qp, 512 * (qp + 1))
                qv4 = qT8[hp][:].rearrange("p (f t) -> p f t", f=4)
                oA = pso.tile([128, 512], f32, tag="o", bufs=2, name="oA")
                oB = pso.tile([128, 512], f32, tag="o", bufs=2, name="oB")
                nkt = 4 * qp + 4
                for kt in range(nkt):
                    a = max(0, 128 * kt - 512 * qp)
                    s = pss.tile([128, 1024], f32, tag="s", name="s")
                    for h in range(2):
                        out_sl = s[:, a:512] if h == 0 else s[:, 512 + a : 1024]
                        _mm(
                            nc,
                            out_sl,
                            kv4[0][:, 2 * h : 2 * h + 2, 128 * kt : 128 * (kt + 1)],
                            qv4[:, 2 * h : 2 * h + 2, 512 * qp + a : 512 * (qp + 1)],
                            start=True,
                            stop=True,
                            perf_mode=DR,
                            tile_position=(0, 0),
                        )
                    e = ep.tile([128, 1024], bf16, tag="e", name="e")
                    if a:
                        sv = s[:].rearrange("p (two q) -> p two q", two=2)[:, :, a:512]
                        ev = e[:].rearrange("p (two q) -> p two q", two=2)[:, :, a:512]
                        nc.scalar.activation(
                            out=ev,
                            in_=sv,
                            func=mybir.ActivationFunctionType.Exp,
                            scale=escale_f,
                        )
                    else:
                        nc.scalar.activation(
                            out=e[:],
                            in_=s[:],
                            func=mybir.ActivationFunctionType.Exp,
                            scale=escale_f,
                        )
                    if kt >= 4 * qp:  # diagonal: causal triangle mask
                        o = 128 * kt - 512 * qp
                        e3 = e[:].rearrange("p (two q) -> p two q", two=2)[
                            :, :, o : o + 128
                        ]
                        nc.vector.tensor_tensor(e3, e3, tri3, mult)
                    st, sp = (kt == 0), (kt == nkt - 1)
                    _mm(
                        nc,
                        oA[0:65, a:512],
                        v_sb[kt][:, 0:65],
                        e[:, a:512],
                        start=st,
                        stop=sp,
                        skip_group_check=True,
                    )
                    _mm(
                        nc,
                        oB[0:65, a:512],
                        v_sb[kt][:, 65:130],
                        e[:, 512 + a : 1024],
                        start=st,
                        stop=sp,
                        skip_group_check=True,
                    )
                    if kt == 1:
                        flush_pending()
                    if kt >= 1:
                        flush_work(1)
                # evacuate O' right away (frees the oA/oB psum ring for the
                # next unit); the rest of the tail is deferred
                oraw = rrp.tile([128, 1024], f32r, tag="rr", bufs=2, name="oraw")
                nc.vector.tensor_copy(oraw[0:65, 0:512], oA[0:65, :])
                nc.vector.tensor_copy(oraw[0:65, 512:1024], oB[0:65, :])

                def tail():
                    rb = rbp.tile([128, 1024], f32, tag="rb", bufs=2, name="rb")
                    scr = drp.tile([1, 1024], f32r, tag="scr", name="scr")
                    nc.sync.dma_start(scr[:], oraw[64:65, :])
                    nc.sync.dma_start(
                        rb[0:64, :].bitcast(f32r), scr[:].to_broadcast((64, 1024))
                    )
                    nc.vector.reciprocal_approx_fast(rb[0:64, :], rb[0:64, :])
                    nc.vector.tensor_tensor(
                        oT[hp][0:64, qsl], oraw[0:64, 0:512], rb[0:64, 0:512], mult
                    )
                    nb = tmpp.tile([128, 512], bf16, tag="nb", bufs=2, name="nb")
                    nc.vector.tensor_tensor(
                        nb[0:64, :], oraw[0:64, 512:1024], rb[0:64, 512:1024], mult
                    )
                    # head B -> oT partitions 64-127 (partition-shift DMA)
                    nc.sync.dma_start(oT[hp][64:128, qsl], nb[0:64, :])
                    if hp == 3:
                        for j in range(4):
                            pending_work.append(lambda j=j: y_tile(4 * qp + j))

                pending_work.append(tail)

            # ---- emission order: saturate ACT early, spread q-projs ---
            warm(130)
            kT8 = q8p.tile([32, 4 * T], f8, tag="dr", bufs=5, name="kT8")
            project_rope(wk, kT8, head=True, fold_each=True)
            q_proj(0, head=True)
            for t in range(4):
                v_proj(t, ring="o")
            kv4[0] = kT8[:].rearrange("p (f t) -> p f t", f=4)
            flush_pending()
            attn_unit(0, 0)
            for t in range(4, 8):
                v_proj(t)
            attn_unit(1, 0)
            for t in range(8, 12):
                v_proj(t)
            q_proj_deferred(1)
            attn_unit(2, 0)
            for t in range(12, 16):
                v_proj(t)
            attn_unit(3, 0)
            attn_unit(0, 1)
            q_proj_deferred(2)
            attn_unit(1, 1)
            attn_unit(2, 1)
            attn_unit(3, 1)
            attn_unit(0, 2)
            q_proj_deferred(3)
            attn_unit(1, 2)
            attn_unit(0, 3)
            attn_unit(2, 2)
            attn_unit(1, 3)
            attn_unit(3, 2)
            attn_unit(2, 3)
            attn_unit(3, 3)
            warm(60, ring="o")
            flush_pending()
            while pending_work:
                flush_work(1)

    nc.compile()
    return nc


def _get_program():
    global _PROGRAM
    if _PROGRAM is None:
        _PROGRAM = _build_program()
    return _PROGRAM


def _host_tables():
    """cos/sin [128, T] (two stacked 64-row copies), R^T (lhsT), tri mask."""
    d = D_K
    inv_freq = 1.0 / (ROPE_BASE ** (np.arange(0, d, 2, dtype=np.float32) / d))
    ang = np.arange(T, dtype=np.float32)[:, None] * inv_freq[None, :]  # [T, 32]
    cos64 = np.repeat(np.cos(ang).astype(np.float32), 2, axis=1).T.copy()
    sin64 = np.repeat(np.sin(ang).astype(np.float32), 2, axis=1).T.copy()
    cos128 = np.ascontiguousarray(np.concatenate([cos64, cos64], axis=0))
    sin128 = np.ascontiguousarray(np.concatenate([sin64, sin64], axis=0))
    # rot = R @ q with rot[2i] = -q[2i+1], rot[2i+1] = q[2i]; pass lhsT = R^T
    R = np.zeros((128, 128), dtype=np.float32)
    for i in range(64):
        R[2 * i, 2 * i + 1] = -1.0
        R[2 * i + 1, 2 * i] = 1.0
    rmat = np.ascontiguousarray(R.T)
    tri = np.triu(np.ones((128, 128), dtype=np.float32))  # keep kk <= qq
    tri2 = np.ascontiguousarray(np.concatenate([tri, tri], axis=1))
    return cos128, sin128, rmat, tri2


def _head_perm():
    """chunk m holds local heads (m, m+4) -> permute Wq cols / Wo rows."""
    perm = []
    for m in range(4):
        perm.extend(range(64 * m, 64 * m + 64))
        perm.extend(range(64 * (m + 4), 64 * (m + 4) + 64))
    return np.array(perm)


def _pack_rows(a, rows_per_tile=128):
    """[N*128, C] -> [128, N*C]: tile k's rows become column block k."""
    n = a.shape[0] // rows_per_tile
    return np.ascontiguousarray(
        np.concatenate(
            [a[rows_per_tile * k : rows_per_tile * (k + 1)] for k in range(n)], axis=1
        )
    )


def _pack_wq_mmajor(a):
    """[1024, 512] -> [128, (m, k, 128)]: chunk m's k-tiles contiguous."""
    out = np.empty((128, 4 * 8 * 128), dtype=a.dtype)
    for m in range(4):
        for k in range(8):
            out[:, 1024 * m + 128 * k : 1024 * m + 128 * (k + 1)] = a[
                128 * k : 128 * (k + 1), 128 * m : 128 * (m + 1)
            ]
    return np.ascontiguousarray(out)


def make_in_maps(x, Wq, Wk, Wv, Wo):
    import ml_dtypes

    bf = ml_dtypes.bfloat16
    cos128, sin128, rmat, tri2 = _host_tables()
    perm = _head_perm()
    constb = np.concatenate(
        [rmat, tri2, np.ones((128, 8), dtype=np.float32)], axis=1
    ).astype(bf)
    in_maps = []
    for c in range(N_CORES):
        b, hg = c // 2, c % 2
        in_maps.append(
            {
                "xtp": _pack_rows(np.ascontiguousarray(x[b].T).astype(bf)),
                "wqp": _pack_wq_mmajor(
                    Wq[:, hg * 512 : (hg + 1) * 512][:, perm].astype(bf)
                ),
                "wkp": _pack_rows(Wk[:, hg * 128 : (hg + 1) * 128].astype(bf)),
                "wvp": _pack_rows(Wv[:, hg * 128 : (hg + 1) * 128].astype(bf)),
                "wop": _pack_rows(
                    Wo[hg * 512 : (hg + 1) * 512, :][perm, :].astype(bf)
                ),
                "constb": constb,
                "costab": cos128.astype(bf),
                "sintab": sin128.astype(bf),
            }
        )
    return in_maps


def kernel(x, attention_mask, Wq, Wk, Wv, Wo, _trace=False, _trace_kwargs=None):
    global LAST_RESULTS
    from concourse import bass_utils

    x = np.asarray(x, dtype=np.float32)
    Wq = np.asarray(Wq, dtype=np.float32)
    Wk = np.asarray(Wk, dtype=np.float32)
    Wv = np.asarray(Wv, dtype=np.float32)
    Wo = np.asarray(Wo, dtype=np.float32)

    nc = _get_program()
    in_maps = make_in_maps(x, Wq, Wk, Wv, Wo)
    res = bass_utils.run_bass_kernel_spmd(
        nc,
        in_maps,
        core_ids=list(range(N_CORES)),
        trace=_trace,
        **(_trace_kwargs or {}),
    )
    LAST_RESULTS = res

    y = np.zeros((B, T, D_MODEL), dtype=np.float32)
    for b in range(B):
        y[b] = np.asarray(res.results[2 * b]["y"], dtype=np.float32) + np.asarray(
            res.results[2 * b + 1]["y"], dtype=np.float32
        )

    # faithful handling of padded (attention_mask == 0) query rows: the
    # reference's mask makes those rows uniform attention over ALL keys.
    am = np.asarray(attention_mask)
    if not np.all(am == 1):
        rep = N_HEADS // NUM_KV_HEADS
        for b in range(B):
            rows = np.where(am[b] == 0)[0]
            if rows.size:
                V = x[b] @ Wv
                Vfull = np.repeat(
                    V.reshape(T, NUM_KV_HEADS, D_K), rep, axis=1
                ).reshape(T, D_MODEL)
                y[b, rows] = (Vfull.mean(axis=0) @ Wo)[None, :]
    return y


# revision 31
# speedup vs baseline: 1.2314x; 1.0118x over previous
"""Trainium2 Bass kernel for causal multi-head attention with RoPE + GQA.

Model: D_MODEL=1024, N_HEADS=16, NUM_KV_HEADS=4, D_K=64, B=4, T=2048.
Sharding (8 cores): core c -> batch b = c//2, head-group hg = c%2
(8 query heads / 2 kv heads per core). Each core computes a partial
output  y_partial = attn_out_local @ Wo[rows of its heads]  and the host
sums the two partials per batch (the tensor-parallel all-reduce happens
at gather time).

Perf design (cost-model driven; ACT exp ~147us is the floor engine):
  - All GEMMs bf16 (1 PE cycle/row) except S = K^T.T Q^T, which runs in
    fp8e4m3 MatmulPerfMode.DoubleRow (0.5 cycles/row) with the d_k=64
    contraction laid out [32 partitions, 2 k-subtiles]. fp32 PSUM
    accumulation everywhere; end-to-end rel_err ~5e-3 (gate 2e-2).
  - Attention is emitted HEAD-PAIR-MAJOR with the q-chunk projections
    interleaved, so ScalarE starts exp'ing ~23us in and stays saturated
    while PE computes the remaining projections underneath it.
  - Each (qp, hp) unit's normalization tail (denominator DRAM-bounce
    broadcast, reciprocal, scale, head-B partition-shift DMA, and the
    previous q-chunk's output projection) is DEFERRED into the next
    unit's kt loop so it never sits between PV and the next S matmul.
  - DMA instruction COUNT is precious (~625ns serialized descriptor-gen
    each): all host inputs are pre-packed for single contiguous DMAs,
    x^T is split qc-major in 4 so the first projections start ~4us in,
    cos/sin load once in bf16.
  - Engine placement: ACT = exp only; Pool = PSUM->SBUF staging + RoPE
    cos-mult; DVE = RoPE sin-mult/add, masking, reciprocal, normalize.

Formulation (features-on-partitions; x arrives host-transposed bf16):
  Q^T = Wq_s^T x^T  [512,2048]    K^T = Wk_s^T x^T  [128,2048]
  V'  = [x @ Wv_s | ones]
  RoPE q*cos + (R q)*sin, R applied by one 128x128 matmul; result
  written fp8e4 and DMA-folded to the DoubleRow [32, (2h+half)*T + t]
  layout.
  S^T = K^T_h.T Q^T_h (fp8 DoubleRow), E^T = exp(S^T/8) -> bf16 (ACT),
  causal triangle on DVE, O'^T = V'_h.T E^T (M=65; row 64 = softmax
  denominator), O^T = O'^T * recip(den), y = O^T.T Wo_s -> bf16 DMA.
Heads are paired (m, m+4) across the two kv groups; Wq columns / Wo
rows are permuted accordingly on the host.
"""

import numpy as np

D_MODEL = 1024
N_HEADS = 16
NUM_KV_HEADS = 4
D_K = 64
ROPE_BASE = 10000.0
B, T = 4, 2048
N_CORES = 8
KT = 16             # 128-row key tiles per sequence
QC = 4              # 512-col query chunks
DCH = 8             # 128-row feature (d_model) tiles

_PROGRAM = None     # cached compiled Bass program
LAST_RESULTS = None  # BassKernelResults of the most recent run


def _mm(nc, out, lhsT, rhs, **kw):
    nc.tensor.matmul(out, lhsT, rhs, **kw)


def _build_program():
    import concourse.mybir as mybir
    import concourse.tile as tile
    from concourse import bacc

    f32 = mybir.dt.float32
    f32r = mybir.dt.float32r
    bf16 = mybir.dt.bfloat16
    f8 = mybir.dt.float8e4
    nc = bacc.Bacc("TRN2", target_bir_lowering=False, debug=False)

    # every input pre-packed on host; x/w as fp8 hi+lo splits (w scaled
    # x64 on host to clear fp8's subnormal floor; compensated via the exp
    # scale and the V' ones value)
    xth_d = nc.dram_tensor("xtp8h", [128, DCH * T], f8, kind="ExternalInput")
    xtl_d = nc.dram_tensor("xtp8l", [128, DCH * T], f8, kind="ExternalInput")
    wqh_d = nc.dram_tensor("wqp8h", [128, DCH * 512], f8, kind="ExternalInput")
    wql_d = nc.dram_tensor("wqp8l", [128, DCH * 512], f8, kind="ExternalInput")
    wkh_d = nc.dram_tensor("wkp8h", [128, DCH * 128], f8, kind="ExternalInput")
    wkl_d = nc.dram_tensor("wkp8l", [128, DCH * 128], f8, kind="ExternalInput")
    wvh_d = nc.dram_tensor("wvp8h", [128, DCH * 128], f8, kind="ExternalInput")
    wvl_d = nc.dram_tensor("wvp8l", [128, DCH * 128], f8, kind="ExternalInput")
    wo_d = nc.dram_tensor("wop", [128, 4 * 1024], bf16, kind="ExternalInput")
    cb_d = nc.dram_tensor("constb", [128, 392], bf16, kind="ExternalInput")
    cos_d = nc.dram_tensor("costab", [128, T], bf16, kind="ExternalInput")
    sin_d = nc.dram_tensor("sintab", [128, T], bf16, kind="ExternalInput")
    y_d = nc.dram_tensor("y", [T, D_MODEL], bf16, kind="ExternalOutput")

    mult = mybir.AluOpType.mult
    add = mybir.AluOpType.add
    div = mybir.AluOpType.divide
    DR = mybir.MatmulPerfMode.DoubleRow

    with tile.TileContext(nc) as tc:
        with (
            tc.tile_pool(name="big", bufs=6) as big,
            tc.tile_pool(name="w", bufs=4) as wp,
            tc.tile_pool(name="const", bufs=1) as constp,
            tc.tile_pool(name="q8", bufs=5) as q8p,
            tc.tile_pool(name="vt", bufs=16) as vtp,
            tc.tile_pool(name="dst", bufs=5) as dstp,
            tc.tile_pool(name="tmp", bufs=2) as tmpp,
            tc.tile_pool(name="e", bufs=3) as ep,
            tc.tile_pool(name="rr", bufs=1) as rrp,
            tc.tile_pool(name="rb", bufs=1) as rbp,
            tc.tile_pool(name="ysb", bufs=2) as ysbp,
            tc.tile_pool(name="dr", bufs=2, space="DRAM") as drp,
            tc.tile_pool(name="ps_g", bufs=2, space="PSUM") as psg,
            tc.tile_pool(name="ps_s", bufs=2, space="PSUM") as pss,
            tc.tile_pool(name="ps_o", bufs=2, space="PSUM") as pso,
        ):
            # ---- input loads, ordered for earliest compute start ------
            cb = constp.tile([128, 392], bf16, tag="cb", name="cb")
            nc.sync.dma_start(cb[:], cb_d[:])
            xt_sb = [
                big.tile([128, DCH * T], f8, tag=f"xt{i}", bufs=1, name=f"xt8{i}")
                for i in range(2)
            ]
            xt3 = [t[:].rearrange("p (k t) -> p k t", k=DCH) for t in xt_sb]
            xtd3 = [
                d[:].rearrange("p (k t) -> p k t", k=DCH) for d in (xth_d, xtl_d)
            ]
            cs0 = slice(0, 512)
            nc.sync.dma_start(xt3[0][:, :, cs0], xtd3[0][:, :, cs0])
            nc.sync.dma_start(xt3[1][:, :, cs0], xtd3[1][:, :, cs0])
            wk_sb = [
                wp.tile([128, DCH * 128], f8, tag=f"wk{i}", bufs=1, name=f"wk8{i}")
                for i in range(2)
            ]
            nc.sync.dma_start(wk_sb[0][:], wkh_d[:])
            nc.sync.dma_start(wk_sb[1][:], wkl_d[:])
            wq_sb = [
                wp.tile([128, DCH * 512], f8, tag=f"wq{i}", bufs=1, name=f"wq8{i}")
                for i in range(2)
            ]
            nc.sync.dma_start(wq_sb[0][:, 0:1024], wqh_d[:, 0:1024])
            nc.sync.dma_start(wq_sb[1][:, 0:1024], wql_d[:, 0:1024])
            cos_sb = constp.tile([128, T], bf16, tag="cos", name="cos_sb")
            nc.sync.dma_start(cos_sb[:], cos_d[:])
            sin_sb = constp.tile([128, T], bf16, tag="sin", name="sin_sb")
            nc.sync.dma_start(sin_sb[:], sin_d[:])
            wv_sb = [
                wp.tile([128, DCH * 128], f8, tag=f"wv{i}", bufs=1, name=f"wv8{i}")
                for i in range(2)
            ]
            nc.sync.dma_start(wv_sb[0][:], wvh_d[:])
            nc.sync.dma_start(wv_sb[1][:], wvl_d[:])
            for qc in range(1, QC):
                cs_ = slice(512 * qc, 512 * (qc + 1))
                nc.sync.dma_start(xt3[0][:, :, cs_], xtd3[0][:, :, cs_])
                nc.sync.dma_start(xt3[1][:, :, cs_], xtd3[1][:, :, cs_])
            nc.sync.dma_start(wq_sb[0][:, 1024:4096], wqh_d[:, 1024:4096])
            nc.sync.dma_start(wq_sb[1][:, 1024:4096], wql_d[:, 1024:4096])
            wo_all = wp.tile([128, 4 * 1024], bf16, tag="wo", bufs=1, name="wo_all")
            nc.sync.dma_start(wo_all[:], wo_d[:])

            # pair views for DoubleRow: [128, 2 k-subtiles, cols]
            wk3 = [t[:].rearrange("p (k j) -> p k j", k=DCH) for t in wk_sb]
            wv3 = [t[:].rearrange("p (k j) -> p k j", k=DCH) for t in wv_sb]
            wq3 = [t[:].rearrange("p (g j) -> p g j", g=4 * DCH) for t in wq_sb]

            def xpair(b, j, cs_):
                return xt3[b][:, 2 * j : 2 * j + 2, cs_]

            def wkpair(a, j):
                return wk3[a][:, 2 * j : 2 * j + 2, :]

            def wqpair(m, a, j):
                # m-major packing: group g = 8*m + k
                return wq3[a][:, 8 * m + 2 * j : 8 * m + 2 * j + 2, :]

            def wo(c):
                return wo_all[:, 1024 * c : 1024 * (c + 1)]

            PRODS = ((0, 0), (1, 0), (0, 1))  # (w hi/lo, x hi/lo)

            rmat_sb = cb[:, 0:128]
            tri_sb = cb[:, 128:384]
            ones_bf = cb[:, 384:392]

            pending = []      # head-phase rope tails: flushed whole
            pending_work = []  # steady-state closures: flushed 1 per kt

            def flush_pending():
                for f in pending:
                    f()
                pending.clear()

            def flush_work(n=1):
                for _ in range(n):
                    if not pending_work:
                        return
                    pending_work.pop(0)()

            # PE pstate warm-up: the cost model charges LOW/MID clocks to
            # matmuls decoded within 3us of an idle->busy transition, so
            # keep PE trivially busy across head-phase DMA waits.
            def warm(n, ring="psg"):
                if ring == "o":
                    wt = pso.tile([128, 512], f32, tag="o", bufs=2, name="warm")
                else:
                    wt = psg.tile([128, 512], f32, tag="psg", bufs=2, name="warm")
                for _ in range(n):
                    _mm(
                        nc,
                        wt[0:64, 0:64],
                        cb[:, 0:64],
                        cb[:, 0:64],
                        start=True,
                        stop=True,
                        skip_group_check=True,
                    )

            # ---- fused projection + RoPE -> fp8 DoubleRow layout ------
            # dr layout: [32 partitions, (2*head + khalf)*T + t]
            # The rope tail (rot matmul + cos/sin combine) of chunk qc is
            # deferred until after chunk qc+1's projection matmuls: the
            # tile scheduler is run-ahead in-order-with-skip per engine,
            # so an op emitted before its input is ready gets parked
            # until the engine idles (which PE never does).
            def project_rope(wpair, dr_dst, head=False, fold_each=False):
                q8full = q8p.tile([128, T], f8, tag="q8f", bufs=2, name="q8full")

                def make_tail(qc, ps):
                    cs_ = slice(512 * qc, 512 * (qc + 1))
                    dst = dstp.tile([128, 512], bf16, tag="dst", bufs=5, name="dst")
                    if head:
                        nc.scalar.copy(dst[:], ps[:])
                    else:
                        nc.vector.tensor_copy(dst[:], ps[:])
                    # cos-multiply needs only dst: run it right away on Pool
                    c1 = tmpp.tile([128, 512], f32, tag="c1", bufs=4, name="c1")
                    nc.gpsimd.tensor_tensor(c1[:], dst[:], cos_sb[:, cs_], mult)

                    def tail():
                        rot = psg.tile([128, 512], f32, tag="psg", bufs=2, name="ps_rot")
                        _mm(nc, rot[:], rmat_sb, dst[:], start=True, stop=True)
                        t1 = tmpp.tile([128, 512], f32, tag="t1", bufs=4, name="t1")
                        nc.vector.tensor_tensor(t1[:], rot[:], sin_sb[:, cs_], mult)
                        nc.vector.tensor_tensor(q8full[:, cs_], c1[:], t1[:], add)
                        # partition fold into DoubleRow layout; per-qc for
                        # the head tiles so attention starts on partial K/Q
                        if fold_each:
                            for g in range(4):
                                nc.sync.dma_start(
                                    dr_dst[0:32, T * g + 512 * qc : T * g + 512 * (qc + 1)],
                                    q8full[32 * g : 32 * (g + 1), cs_],
                                )
                        elif qc == QC - 1:
                            for g in range(4):
                                nc.sync.dma_start(
                                    dr_dst[0:32, T * g : T * (g + 1)],
                                    q8full[32 * g : 32 * (g + 1), :],
                                )

                    return tail

                prev_tail = None
                for qc in range(QC):
                    cs_ = slice(512 * qc, 512 * (qc + 1))
                    if head and qc % 2 == 0:
                        ps = pso.tile([128, 512], f32, tag="o", bufs=2, name="ps_proj")
                    else:
                        ps = psg.tile([128, 512], f32, tag="psg", bufs=2, name="ps_proj")
                    for j in range(DCH // 2):
                        for pi, (a, b) in enumerate(PRODS):
                            _mm(
                                nc,
                                ps[:],
                                wpair(a, j),
                                xpair(b, j, cs_),
                                start=(j == 0 and pi == 0),
                                stop=(j == DCH // 2 - 1 and pi == 2),
                                perf_mode=DR,
                                tile_position=(0, 0),
                            )
                    if qc == 0:
                        # previous projection's last rope tail rides right
                        # behind this chunk's matmuls on the PE queue
                        flush_pending()
                    if head:
                        warm(18)
                    if prev_tail is not None:
                        prev_tail()
                    prev_tail = make_tail(qc, ps)
                pending.append(prev_tail)

            v_all = vtp.tile([128, KT * 130], bf16, tag="v", bufs=1, name="v_all")
            v3 = v_all[:].rearrange("p (t c) -> p t c", t=KT)
            # V rows carry 64x-scaled V; ones row = 64 keeps num/den exact
            nc.vector.memset(v3[:, :, 64:65], 64.0)
            nc.vector.memset(v3[:, :, 129:130], 64.0)

            def v_proj(t, ring=None):
                if ring == "o":
                    ps = pso.tile([128, 512], f32, tag="o", bufs=2, name="ps_v")
                else:
                    ps = psg.tile([128, 512], f32, tag="psg", bufs=2, name="ps_v")
                for j in range(DCH // 2):
                    for pi, (a, b) in enumerate(PRODS):
                        _mm(
                            nc,
                            ps[:, 0:128],
                            xt3[b][:, 2 * j : 2 * j + 2, 128 * t : 128 * (t + 1)],
                            wv3[a][:, 2 * j : 2 * j + 2, :],
                            start=(j == 0 and pi == 0),
                            stop=(j == DCH // 2 - 1 and pi == 2),
                            perf_mode=DR,
                            tile_position=(0, 0),
                        )
                vt = v_all[:, 130 * t : 130 * (t + 1)]
                nc.vector.tensor_copy(vt[:, 0:64], ps[:, 0:64])
                nc.vector.tensor_copy(vt[:, 65:129], ps[:, 64:128])
                v_sb.append(vt)

            v_sb = []
            qT8 = []

            def q_proj(m, head=False):
                qt = q8p.tile([32, 4 * T], f8, tag="dr", bufs=5, name=f"qT8{m}")
                project_rope(
                    lambda a, j: wqpair(m, a, j), qt, head=head, fold_each=head
                )
                qT8.append(qt)

            def q_proj_deferred(m):
                """emit q-chunk m's projection as per-qc closures so the
                attention kt loop interleaves them 1.7us at a time."""
                qt = q8p.tile([32, 4 * T], f8, tag="dr", bufs=5, name=f"qT8{m}")
                qT8.append(qt)
                wsel = lambda a, j: wqpair(m, a, j)
                q8full = q8p.tile([128, T], f8, tag="q8f", bufs=2, name="q8full")
                state = {"tail": None}

                def make_qc(qc):
                    cs_ = slice(512 * qc, 512 * (qc + 1))

                    def go():
                        ps = psg.tile(
                            [128, 512], f32, tag="psg", bufs=2, name="ps_proj"
                        )
                        for j in range(DCH // 2):
                            for pi, (a, b) in enumerate(PRODS):
                                _mm(
                                    nc,
                                    ps[:],
                                    wsel(a, j),
                                    xpair(b, j, cs_),
                                    start=(j == 0 and pi == 0),
                                    stop=(j == DCH // 2 - 1 and pi == 2),
                                    perf_mode=DR,
                                    tile_position=(0, 0),
                                )
                        if state["tail"] is not None:
                            state["tail"]()
                        dst = dstp.tile(
                            [128, 512], bf16, tag="dst", bufs=5, name="dst"
                        )
                        nc.vector.tensor_copy(dst[:], ps[:])
                        c1 = tmpp.tile([128, 512], f32, tag="c1", bufs=4, name="c1")
                        nc.gpsimd.tensor_tensor(c1[:], dst[:], cos_sb[:, cs_], mult)

                        def tail():
                            rot = psg.tile(
                                [128, 512], f32, tag="psg", bufs=2, name="ps_rot"
                            )
                            _mm(nc, rot[:], rmat_sb, dst[:], start=True, stop=True)
                            t1 = tmpp.tile(
                                [128, 512], f32, tag="t1", bufs=4, name="t1"
                            )
                            nc.vector.tensor_tensor(
                                t1[:], rot[:], sin_sb[:, cs_], mult
                            )
                            nc.vector.tensor_tensor(q8full[:, cs_], c1[:], t1[:], add)
                            if qc == QC - 1:
                                for g in range(4):
                                    nc.sync.dma_start(
                                        qt[0:32, T * g : T * (g + 1)],
                                        q8full[32 * g : 32 * (g + 1), :],
                                    )

                        state["tail"] = tail

                    return go

                for qc in range(QC):
                    pending_work.append(make_qc(qc))
                pending_work.append(lambda: (state["tail"](), state.update(tail=None)))

            oT = [
                big.tile([128, T], bf16, tag="oT", bufs=4, name=f"oT{m}")
                for m in range(4)
            ]
            tri3 = tri_sb.rearrange("p (two q) -> p two q", two=2)
            # PSUM carries 64x-scaled Q/K (w*64 on host): S is 4096x
            escale = float(1.0 / np.sqrt(D_K)) / 4096.0

            def y_tile(t):
                """output projection for one 128-row token tile. nh=0 uses
                the 'o' psum ring, nh=1 the 'psg' ring (parallel banks)."""
                ty = ysbp.tile([128, 1024], bf16, tag="y", name="ty")
                for nh in range(2):
                    ps = psg.tile([128, 512], f32, tag="psg", bufs=2, name="ps_y")
                    for c in range(4):
                        _mm(
                            nc,
                            ps[:],
                            oT[c][:, 128 * t : 128 * (t + 1)],
                            wo(c)[:, 512 * nh : 512 * (nh + 1)],
                            start=(c == 0),
                            stop=(c == 3),
                        )
                    nc.vector.tensor_copy(ty[:, 512 * nh : 512 * (nh + 1)], ps[:])
                nc.sync.dma_start(y_d[128 * t : 128 * (t + 1), :], ty[:])

            # ---- attention unit (one q-chunk x one head-pair) ---------
            kv4 = [None]
            escale_f = escale

            def attn_unit(qp, hp):
                qsl = slice(512 * qp, 512 * (qp + 1))
                qv4 = qT8[hp][:].rearrange("p (f t) -> p f t", f=4)
                oA = pso.tile([128, 512], f32, tag="o", bufs=2, name="oA")
                oB = pso.tile([128, 512], f32, tag="o", bufs=2, name="oB")
                nkt = 4 * qp + 4
                for kt in range(nkt):
                    a = max(0, 128 * kt - 512 * qp)
                    s = pss.tile([128, 1024], f32, tag="s", name="s")
                    for h in range(2):
                        out_sl = s[:, a:512] if h == 0 else s[:, 512 + a : 1024]
                        _mm(
                            nc,
                            out_sl,
                            kv4[0][:, 2 * h : 2 * h + 2, 128 * kt : 128 * (kt + 1)],
                            qv4[:, 2 * h : 2 * h + 2, 512 * qp + a : 512 * (qp + 1)],
                            start=True,
                            stop=True,
                            perf_mode=DR,
                            tile_position=(0, 0),
                        )
                    e = ep.tile([128, 1024], bf16, tag="e", name="e")
                    if a:
                        sv = s[:].rearrange("p (two q) -> p two q", two=2)[:, :, a:512]
                        ev = e[:].rearrange("p (two q) -> p two q", two=2)[:, :, a:512]
                        nc.scalar.activation(
                            out=ev,
                            in_=sv,
                            func=mybir.ActivationFunctionType.Exp,
                            scale=escale_f,
                        )
                    else:
                        nc.scalar.activation(
                            out=e[:],
                            in_=s[:],
                            func=mybir.ActivationFunctionType.Exp,
                            scale=escale_f,
                        )
                    if kt >= 4 * qp:  # diagonal: causal triangle mask
                        o = 128 * kt - 512 * qp
                        e3 = e[:].rearrange("p (two q) -> p two q", two=2)[
                            :, :, o : o + 128
                        ]
                        nc.vector.tensor_tensor(e3, e3, tri3, mult)
                    st, sp = (kt == 0), (kt == nkt - 1)
                    _mm(
                        nc,
                        oA[0:65, a:512],
                        v_sb[kt][:, 0:65],
                        e[:, a:512],
                        start=st,
                        stop=sp,
                        skip_group_check=True,
                    )
                    _mm(
                        nc,
                        oB[0:65, a:512],
                        v_sb[kt][:, 65:130],
                        e[:, 512 + a : 1024],
                        start=st,
                        stop=sp,
                        skip_group_check=True,
                    )
                    if kt == 1:
                        flush_pending()
                    if kt >= 1:
                        flush_work(1)
                # evacuate O' right away (frees the oA/oB psum ring for the
                # next unit); the rest of the tail is deferred
                oraw = rrp.tile([128, 1024], f32r, tag="rr", bufs=2, name="oraw")
                nc.vector.tensor_copy(oraw[0:65, 0:512], oA[0:65, :])
                nc.vector.tensor_copy(oraw[0:65, 512:1024], oB[0:65, :])

                def tail():
                    rb = rbp.tile([128, 1024], f32, tag="rb", bufs=2, name="rb")
                    scr = drp.tile([1, 1024], f32r, tag="scr", name="scr")
                    nc.sync.dma_start(scr[:], oraw[64:65, :])
                    nc.sync.dma_start(
                        rb[0:64, :].bitcast(f32r), scr[:].to_broadcast((64, 1024))
                    )
                    nc.vector.reciprocal_approx_fast(rb[0:64, :], rb[0:64, :])
                    nc.vector.tensor_tensor(
                        oT[hp][0:64, qsl], oraw[0:64, 0:512], rb[0:64, 0:512], mult
                    )
                    nb = tmpp.tile([128, 512], bf16, tag="nb", bufs=2, name="nb")
                    nc.vector.tensor_tensor(
                        nb[0:64, :], oraw[0:64, 512:1024], rb[0:64, 512:1024], mult
                    )
                    # head B -> oT partitions 64-127 (partition-shift DMA)
                    nc.sync.dma_start(oT[hp][64:128, qsl], nb[0:64, :])
                    if hp == 3:
                        for j in range(4):
                            pending_work.append(lambda j=j: y_tile(4 * qp + j))

                pending_work.append(tail)

            # ---- emission order: saturate ACT early, spread q-projs ---
            warm(130)
            kT8 = q8p.tile([32, 4 * T], f8, tag="dr", bufs=5, name="kT8")
            project_rope(wkpair, kT8, head=True, fold_each=True)
            q_proj(0, head=True)
            for t in range(4):
                v_proj(t, ring="o")
            kv4[0] = kT8[:].rearrange("p (f t) -> p f t", f=4)
            flush_pending()
            attn_unit(0, 0)
            for t in range(4, 8):
                v_proj(t)
            attn_unit(1, 0)
            for t in range(8, 12):
                v_proj(t)
            q_proj_deferred(1)
            attn_unit(2, 0)
            for t in range(12, 16):
                v_proj(t)
            attn_unit(3, 0)
            attn_unit(0, 1)
            q_proj_deferred(2)
            attn_unit(1, 1)
            attn_unit(2, 1)
            attn_unit(3, 1)
            attn_unit(0, 2)
            q_proj_deferred(3)
            attn_unit(1, 2)
            attn_unit(0, 3)
            attn_unit(2, 2)
            attn_unit(1, 3)
            attn_unit(3, 2)
            attn_unit(2, 3)
            attn_unit(3, 3)
            warm(60, ring="o")
            flush_pending()
            while pending_work:
                flush_work(1)

    nc.compile()
    return nc


def _get_program():
    global _PROGRAM
    if _PROGRAM is None:
        _PROGRAM = _build_program()
    return _PROGRAM


def _host_tables():
    """cos/sin [128, T] (two stacked 64-row copies), R^T (lhsT), tri mask."""
    d = D_K
    inv_freq = 1.0 / (ROPE_BASE ** (np.arange(0, d, 2, dtype=np.float32) / d))
    ang = np.arange(T, dtype=np.float32)[:, None] * inv_freq[None, :]  # [T, 32]
    cos64 = np.repeat(np.cos(ang).astype(np.float32), 2, axis=1).T.copy()
    sin64 = np.repeat(np.sin(ang).astype(np.float32), 2, axis=1).T.copy()
    cos128 = np.ascontiguousarray(np.concatenate([cos64, cos64], axis=0))
    sin128 = np.ascontiguousarray(np.concatenate([sin64, sin64], axis=0))
    # rot = R @ q with rot[2i] = -q[2i+1], rot[2i+1] = q[2i]; pass lhsT = R^T
    R = np.zeros((128, 128), dtype=np.float32)
    for i in range(64):
        R[2 * i, 2 * i + 1] = -1.0
        R[2 * i + 1, 2 * i] = 1.0
    rmat = np.ascontiguousarray(R.T)
    tri = np.triu(np.ones((128, 128), dtype=np.float32))  # keep kk <= qq
    tri2 = np.ascontiguousarray(np.concatenate([tri, tri], axis=1))
    return cos128, sin128, rmat, tri2


def _head_perm():
    """chunk m holds local heads (m, m+4) -> permute Wq cols / Wo rows."""
    perm = []
    for m in range(4):
        perm.extend(range(64 * m, 64 * m + 64))
        perm.extend(range(64 * (m + 4), 64 * (m + 4) + 64))
    return np.array(perm)


def _pack_rows(a, rows_per_tile=128):
    """[N*128, C] -> [128, N*C]: tile k's rows become column block k."""
    n = a.shape[0] // rows_per_tile
    return np.ascontiguousarray(
        np.concatenate(
            [a[rows_per_tile * k : rows_per_tile * (k + 1)] for k in range(n)], axis=1
        )
    )


def _pack_wq_mmajor(a):
    """[1024, 512] -> [128, (m, k, 128)]: chunk m's k-tiles contiguous."""
    out = np.empty((128, 4 * 8 * 128), dtype=a.dtype)
    for m in range(4):
        for k in range(8):
            out[:, 1024 * m + 128 * k : 1024 * m + 128 * (k + 1)] = a[
                128 * k : 128 * (k + 1), 128 * m : 128 * (m + 1)
            ]
    return np.ascontiguousarray(out)


def _split8(a):
    """fp8e4m3 hi+lo split of an f32 array."""
    import ml_dtypes

    F8 = ml_dtypes.float8_e4m3
    hi = a.astype(F8)
    lo = (a - hi.astype(np.float32)).astype(F8)
    return hi, lo


def make_in_maps(x, Wq, Wk, Wv, Wo):
    import ml_dtypes

    bf = ml_dtypes.bfloat16
    cos128, sin128, rmat, tri2 = _host_tables()
    perm = _head_perm()
    constb = np.concatenate(
        [rmat, tri2, np.ones((128, 8), dtype=np.float32)], axis=1
    ).astype(bf)
    in_maps = []
    for c in range(N_CORES):
        b, hg = c // 2, c % 2
        xth, xtl = _split8(np.ascontiguousarray(x[b].T))
        wqh, wql = _split8(Wq[:, hg * 512 : (hg + 1) * 512][:, perm] * 64.0)
        wkh, wkl = _split8(Wk[:, hg * 128 : (hg + 1) * 128] * 64.0)
        wvh, wvl = _split8(Wv[:, hg * 128 : (hg + 1) * 128] * 64.0)
        in_maps.append(
            {
                "xtp8h": _pack_rows(xth),
                "xtp8l": _pack_rows(xtl),
                "wqp8h": _pack_wq_mmajor(wqh),
                "wqp8l": _pack_wq_mmajor(wql),
                "wkp8h": _pack_rows(wkh),
                "wkp8l": _pack_rows(wkl),
                "wvp8h": _pack_rows(wvh),
                "wvp8l": _pack_rows(wvl),
                "wop": _pack_rows(
                    Wo[hg * 512 : (hg + 1) * 512, :][perm, :].astype(bf)
                ),
                "constb": constb,
                "costab": cos128.astype(bf),
                "sintab": sin128.astype(bf),
            }
        )
    return in_maps


def kernel(x, attention_mask, Wq, Wk, Wv, Wo, _trace=False, _trace_kwargs=None):
    global LAST_RESULTS
    from concourse import bass_utils

    x = np.asarray(x, dtype=np.float32)
    Wq = np.asarray(Wq, dtype=np.float32)
    Wk = np.asarray(Wk, dtype=np.float32)
    Wv = np.asarray(Wv, dtype=np.float32)
    Wo = np.asarray(Wo, dtype=np.float32)

    nc = _get_program()
    in_maps = make_in_maps(x, Wq, Wk, Wv, Wo)
    res = bass_utils.run_bass_kernel_spmd(
        nc,
        in_maps,
        core_ids=list(range(N_CORES)),
        trace=_trace,
        **(_trace_kwargs or {}),
    )
    LAST_RESULTS = res

    y = np.zeros((B, T, D_MODEL), dtype=np.float32)
    for b in range(B):
        y[b] = np.asarray(res.results[2 * b]["y"], dtype=np.float32) + np.asarray(
            res.results[2 * b + 1]["y"], dtype=np.float32
        )

    # faithful handling of padded (attention_mask == 0) query rows: the
    # reference's mask makes those rows uniform attention over ALL keys.
    am = np.asarray(attention_mask)
    if not np.all(am == 1):
        rep = N_HEADS // NUM_KV_HEADS
        for b in range(B):
            rows = np.where(am[b] == 0)[0]
            if rows.size:
                V = x[b] @ Wv
                Vfull = np.repeat(
                    V.reshape(T, NUM_KV_HEADS, D_K), rep, axis=1
                ).reshape(T, D_MODEL)
                y[b, rows] = (Vfull.mean(axis=0) @ Wo)[None, :]
    return y


# revision 34
# speedup vs baseline: 1.2566x; 1.0205x over previous
"""Trainium2 Bass kernel for causal multi-head attention with RoPE + GQA.

Model: D_MODEL=1024, N_HEADS=16, NUM_KV_HEADS=4, D_K=64, B=4, T=2048.
Sharding (8 cores): core c -> batch b = c//2, head-group hg = c%2
(8 query heads / 2 kv heads per core). Each core computes a partial
output  y_partial = attn_out_local @ Wo[rows of its heads]  and the host
sums the two partials per batch (the tensor-parallel all-reduce happens
at gather time).

Perf design (cost-model driven; ACT exp ~147us is the floor engine):
  - All GEMMs bf16 (1 PE cycle/row) except S = K^T.T Q^T, which runs in
    fp8e4m3 MatmulPerfMode.DoubleRow (0.5 cycles/row) with the d_k=64
    contraction laid out [32 partitions, 2 k-subtiles]. fp32 PSUM
    accumulation everywhere; end-to-end rel_err ~5e-3 (gate 2e-2).
  - Attention is emitted HEAD-PAIR-MAJOR with the q-chunk projections
    interleaved, so ScalarE starts exp'ing ~23us in and stays saturated
    while PE computes the remaining projections underneath it.
  - Each (qp, hp) unit's normalization tail (denominator DRAM-bounce
    broadcast, reciprocal, scale, head-B partition-shift DMA, and the
    previous q-chunk's output projection) is DEFERRED into the next
    unit's kt loop so it never sits between PV and the next S matmul.
  - DMA instruction COUNT is precious (~625ns serialized descriptor-gen
    each): all host inputs are pre-packed for single contiguous DMAs,
    x^T is split qc-major in 4 so the first projections start ~4us in,
    cos/sin load once in bf16.
  - Engine placement: ACT = exp only; Pool = PSUM->SBUF staging + RoPE
    cos-mult; DVE = RoPE sin-mult/add, masking, reciprocal, normalize.

Formulation (features-on-partitions; x arrives host-transposed bf16):
  Q^T = Wq_s^T x^T  [512,2048]    K^T = Wk_s^T x^T  [128,2048]
  V'  = [x @ Wv_s | ones]
  RoPE q*cos + (R q)*sin, R applied by one 128x128 matmul; result
  written fp8e4 and DMA-folded to the DoubleRow [32, (2h+half)*T + t]
  layout.
  S^T = K^T_h.T Q^T_h (fp8 DoubleRow), E^T = exp(S^T/8) -> bf16 (ACT),
  causal triangle on DVE, O'^T = V'_h.T E^T (M=65; row 64 = softmax
  denominator), O^T = O'^T * recip(den), y = O^T.T Wo_s -> bf16 DMA.
Heads are paired (m, m+4) across the two kv groups; Wq columns / Wo
rows are permuted accordingly on the host.
"""

import numpy as np

D_MODEL = 1024
N_HEADS = 16
NUM_KV_HEADS = 4
D_K = 64
ROPE_BASE = 10000.0
B, T = 4, 2048
N_CORES = 8
KT = 16             # 128-row key tiles per sequence
QC = 4              # 512-col query chunks
DCH = 8             # 128-row feature (d_model) tiles

_PROGRAM = None     # cached compiled Bass program
LAST_RESULTS = None  # BassKernelResults of the most recent run


def _mm(nc, out, lhsT, rhs, **kw):
    nc.tensor.matmul(out, lhsT, rhs, **kw)


def _build_program():
    import concourse.mybir as mybir
    import concourse.tile as tile
    from concourse import bacc

    f32 = mybir.dt.float32
    f32r = mybir.dt.float32r
    bf16 = mybir.dt.bfloat16
    f8 = mybir.dt.float8e4
    nc = bacc.Bacc("TRN2", target_bir_lowering=False, debug=False)

    # every input pre-packed on host; x/w as fp8 hi+lo splits (w scaled
    # x64 on host to clear fp8's subnormal floor; compensated via the exp
    # scale and the V' ones value)
    xth_d = nc.dram_tensor("xtp8h", [128, DCH * T], f8, kind="ExternalInput")
    xtl_d = nc.dram_tensor("xtp8l", [128, DCH * T], f8, kind="ExternalInput")
    wqh_d = nc.dram_tensor("wqp8h", [128, DCH * 512], f8, kind="ExternalInput")
    wql_d = nc.dram_tensor("wqp8l", [128, DCH * 512], f8, kind="ExternalInput")
    wkh_d = nc.dram_tensor("wkp8h", [128, DCH * 128], f8, kind="ExternalInput")
    wkl_d = nc.dram_tensor("wkp8l", [128, DCH * 128], f8, kind="ExternalInput")
    wvh_d = nc.dram_tensor("wvp8h", [128, DCH * 128], f8, kind="ExternalInput")
    wvl_d = nc.dram_tensor("wvp8l", [128, DCH * 128], f8, kind="ExternalInput")
    wo_d = nc.dram_tensor("wop", [128, 4 * 1024], bf16, kind="ExternalInput")
    cb_d = nc.dram_tensor("constb", [128, 392], bf16, kind="ExternalInput")
    cos_d = nc.dram_tensor("costab", [128, T], bf16, kind="ExternalInput")
    sin_d = nc.dram_tensor("sintab", [128, T], bf16, kind="ExternalInput")
    y_d = nc.dram_tensor("y", [T, D_MODEL], bf16, kind="ExternalOutput")

    mult = mybir.AluOpType.mult
    add = mybir.AluOpType.add
    div = mybir.AluOpType.divide
    DR = mybir.MatmulPerfMode.DoubleRow

    with tile.TileContext(nc) as tc:
        with (
            tc.tile_pool(name="big", bufs=6) as big,
            tc.tile_pool(name="w", bufs=4) as wp,
            tc.tile_pool(name="const", bufs=1) as constp,
            tc.tile_pool(name="q8", bufs=5) as q8p,
            tc.tile_pool(name="vt", bufs=16) as vtp,
            tc.tile_pool(name="dst", bufs=5) as dstp,
            tc.tile_pool(name="tmp", bufs=4) as tmpp,
            tc.tile_pool(name="e", bufs=4) as ep,
            tc.tile_pool(name="rr", bufs=3) as rrp,
            tc.tile_pool(name="rb", bufs=1) as rbp,
            tc.tile_pool(name="ysb", bufs=2) as ysbp,
            tc.tile_pool(name="dr", bufs=2, space="DRAM") as drp,
            tc.tile_pool(name="ps_g", bufs=2, space="PSUM") as psg,
            tc.tile_pool(name="ps_s", bufs=2, space="PSUM") as pss,
            tc.tile_pool(name="ps_o", bufs=2, space="PSUM") as pso,
        ):
            # ---- input loads, ordered for earliest compute start ------
            cb = constp.tile([128, 392], bf16, tag="cb", name="cb")
            nc.sync.dma_start(cb[:], cb_d[:])
            xt_sb = [
                big.tile([128, DCH * T], f8, tag=f"xt{i}", bufs=1, name=f"xt8{i}")
                for i in range(2)
            ]
            xt3 = [t[:].rearrange("p (k t) -> p k t", k=DCH) for t in xt_sb]
            xtd3 = [
                d[:].rearrange("p (k t) -> p k t", k=DCH) for d in (xth_d, xtl_d)
            ]
            cs0 = slice(0, 512)
            nc.sync.dma_start(xt3[0][:, :, cs0], xtd3[0][:, :, cs0])
            nc.sync.dma_start(xt3[1][:, :, cs0], xtd3[1][:, :, cs0])
            wk_sb = [
                wp.tile([128, DCH * 128], f8, tag=f"wk{i}", bufs=1, name=f"wk8{i}")
                for i in range(2)
            ]
            nc.sync.dma_start(wk_sb[0][:], wkh_d[:])
            nc.sync.dma_start(wk_sb[1][:], wkl_d[:])
            wq_sb = [
                wp.tile([128, DCH * 512], f8, tag=f"wq{i}", bufs=1, name=f"wq8{i}")
                for i in range(2)
            ]
            nc.sync.dma_start(wq_sb[0][:, 0:1024], wqh_d[:, 0:1024])
            nc.sync.dma_start(wq_sb[1][:, 0:1024], wql_d[:, 0:1024])
            cos_sb = constp.tile([128, T], bf16, tag="cos", name="cos_sb")
            nc.sync.dma_start(cos_sb[:], cos_d[:])
            sin_sb = constp.tile([128, T], bf16, tag="sin", name="sin_sb")
            nc.sync.dma_start(sin_sb[:], sin_d[:])
            wv_sb = [
                wp.tile([128, DCH * 128], f8, tag=f"wv{i}", bufs=1, name=f"wv8{i}")
                for i in range(2)
            ]
            nc.sync.dma_start(wv_sb[0][:], wvh_d[:])
            nc.sync.dma_start(wv_sb[1][:], wvl_d[:])
            for qc in range(1, QC):
                cs_ = slice(512 * qc, 512 * (qc + 1))
                nc.sync.dma_start(xt3[0][:, :, cs_], xtd3[0][:, :, cs_])
                nc.sync.dma_start(xt3[1][:, :, cs_], xtd3[1][:, :, cs_])
            nc.sync.dma_start(wq_sb[0][:, 1024:4096], wqh_d[:, 1024:4096])
            nc.sync.dma_start(wq_sb[1][:, 1024:4096], wql_d[:, 1024:4096])
            wo_all = wp.tile([128, 4 * 1024], bf16, tag="wo", bufs=1, name="wo_all")
            nc.sync.dma_start(wo_all[:], wo_d[:])

            # pair views for DoubleRow: [128, 2 k-subtiles, cols]
            wk3 = [t[:].rearrange("p (k j) -> p k j", k=DCH) for t in wk_sb]
            wv3 = [t[:].rearrange("p (k j) -> p k j", k=DCH) for t in wv_sb]
            wq3 = [t[:].rearrange("p (g j) -> p g j", g=4 * DCH) for t in wq_sb]

            def xpair(b, j, cs_):
                return xt3[b][:, 2 * j : 2 * j + 2, cs_]

            def wkpair(a, j):
                return wk3[a][:, 2 * j : 2 * j + 2, :]

            def wqpair(m, a, j):
                # m-major packing: group g = 8*m + k
                return wq3[a][:, 8 * m + 2 * j : 8 * m + 2 * j + 2, :]

            def wo(c):
                return wo_all[:, 1024 * c : 1024 * (c + 1)]

            PRODS = ((0, 0), (1, 0), (0, 1))  # (w hi/lo, x hi/lo)

            rmat_sb = cb[:, 0:128]
            tri_sb = cb[:, 128:384]
            ones_bf = cb[:, 384:392]

            pending = []      # head-phase rope tails: flushed whole
            pending_work = []  # steady-state closures: flushed 1 per kt

            def flush_pending():
                for f in pending:
                    f()
                pending.clear()

            def flush_work(n=1):
                for _ in range(n):
                    if not pending_work:
                        return
                    pending_work.pop(0)()

            # PE pstate warm-up: the cost model charges LOW/MID clocks to
            # matmuls decoded within 3us of an idle->busy transition, so
            # keep PE trivially busy across head-phase DMA waits.
            def warm(n, ring="psg"):
                if ring == "o":
                    wt = pso.tile([128, 512], f32, tag="o", bufs=2, name="warm")
                else:
                    wt = psg.tile([128, 512], f32, tag="psg", bufs=2, name="warm")
                for _ in range(n):
                    _mm(
                        nc,
                        wt[0:64, 0:64],
                        cb[:, 0:64],
                        cb[:, 0:64],
                        start=True,
                        stop=True,
                        skip_group_check=True,
                    )

            # ---- fused projection + RoPE -> fp8 DoubleRow layout ------
            # dr layout: [32 partitions, (2*head + khalf)*T + t]
            # The rope tail (rot matmul + cos/sin combine) of chunk qc is
            # deferred until after chunk qc+1's projection matmuls: the
            # tile scheduler is run-ahead in-order-with-skip per engine,
            # so an op emitted before its input is ready gets parked
            # until the engine idles (which PE never does).
            def project_rope(wpair, dr_dst, head=False, fold_each=False):
                q8full = q8p.tile([128, T], f8, tag="q8f", bufs=2, name="q8full")

                def make_tail(qc, ps):
                    cs_ = slice(512 * qc, 512 * (qc + 1))
                    dst = dstp.tile([128, 512], bf16, tag="dst", bufs=5, name="dst")
                    if head:
                        nc.scalar.copy(dst[:], ps[:])
                    else:
                        nc.vector.tensor_copy(dst[:], ps[:])
                    # cos-multiply needs only dst: run it right away on Pool
                    c1 = tmpp.tile([128, 512], f32, tag="c1", bufs=4, name="c1")
                    nc.gpsimd.tensor_tensor(c1[:], dst[:], cos_sb[:, cs_], mult)

                    def tail():
                        rot = psg.tile([128, 512], f32, tag="psg", bufs=2, name="ps_rot")
                        _mm(nc, rot[:], rmat_sb, dst[:], start=True, stop=True)
                        t1 = tmpp.tile([128, 512], f32, tag="t1", bufs=4, name="t1")
                        nc.vector.tensor_tensor(t1[:], rot[:], sin_sb[:, cs_], mult)
                        nc.vector.tensor_tensor(q8full[:, cs_], c1[:], t1[:], add)
                        # partition fold into DoubleRow layout; per-qc for
                        # the head tiles so attention starts on partial K/Q
                        if fold_each:
                            for g in range(4):
                                nc.sync.dma_start(
                                    dr_dst[0:32, T * g + 512 * qc : T * g + 512 * (qc + 1)],
                                    q8full[32 * g : 32 * (g + 1), cs_],
                                )
                        elif qc == QC - 1:
                            for g in range(4):
                                nc.sync.dma_start(
                                    dr_dst[0:32, T * g : T * (g + 1)],
                                    q8full[32 * g : 32 * (g + 1), :],
                                )

                    return tail

                prev_tail = None
                for qc in range(QC):
                    cs_ = slice(512 * qc, 512 * (qc + 1))
                    if head and qc % 2 == 0:
                        ps = pso.tile([128, 512], f32, tag="o", bufs=2, name="ps_proj")
                    else:
                        ps = psg.tile([128, 512], f32, tag="psg", bufs=2, name="ps_proj")
                    for j in range(DCH // 2):
                        for pi, (a, b) in enumerate(PRODS):
                            _mm(
                                nc,
                                ps[:],
                                wpair(a, j),
                                xpair(b, j, cs_),
                                start=(j == 0 and pi == 0),
                                stop=(j == DCH // 2 - 1 and pi == 2),
                                perf_mode=DR,
                                tile_position=(0, 0),
                            )
                    if qc == 0:
                        # previous projection's last rope tail rides right
                        # behind this chunk's matmuls on the PE queue
                        flush_pending()
                    if head:
                        warm(18)
                    if prev_tail is not None:
                        prev_tail()
                    prev_tail = make_tail(qc, ps)
                pending.append(prev_tail)

            v_all = vtp.tile([128, KT * 130], bf16, tag="v", bufs=1, name="v_all")
            v3 = v_all[:].rearrange("p (t c) -> p t c", t=KT)
            # V rows carry 64x-scaled V; ones row = 64 keeps num/den exact
            nc.vector.memset(v3[:, :, 64:65], 64.0)
            nc.vector.memset(v3[:, :, 129:130], 64.0)

            def v_proj(t, ring=None):
                if ring == "o":
                    ps = pso.tile([128, 512], f32, tag="o", bufs=2, name="ps_v")
                else:
                    ps = psg.tile([128, 512], f32, tag="psg", bufs=2, name="ps_v")
                for j in range(DCH // 2):
                    for pi, (a, b) in enumerate(PRODS):
                        _mm(
                            nc,
                            ps[:, 0:128],
                            xt3[b][:, 2 * j : 2 * j + 2, 128 * t : 128 * (t + 1)],
                            wv3[a][:, 2 * j : 2 * j + 2, :],
                            start=(j == 0 and pi == 0),
                            stop=(j == DCH // 2 - 1 and pi == 2),
                            perf_mode=DR,
                            tile_position=(0, 0),
                        )
                vt = v_all[:, 130 * t : 130 * (t + 1)]
                nc.vector.tensor_copy(vt[:, 0:64], ps[:, 0:64])
                nc.vector.tensor_copy(vt[:, 65:129], ps[:, 64:128])
                v_sb.append(vt)

            v_sb = []
            qT8 = []

            def q_proj(m, head=False):
                qt = q8p.tile([32, 4 * T], f8, tag="dr", bufs=5, name=f"qT8{m}")
                project_rope(
                    lambda a, j: wqpair(m, a, j), qt, head=head, fold_each=head
                )
                qT8.append(qt)

            def q_proj_deferred(m):
                """emit q-chunk m's projection as per-qc closures so the
                attention kt loop interleaves them 1.7us at a time."""
                qt = q8p.tile([32, 4 * T], f8, tag="dr", bufs=5, name=f"qT8{m}")
                qT8.append(qt)
                wsel = lambda a, j: wqpair(m, a, j)
                q8full = q8p.tile([128, T], f8, tag="q8f", bufs=2, name="q8full")
                state = {"tail": None}

                def make_qc(qc):
                    cs_ = slice(512 * qc, 512 * (qc + 1))

                    def go():
                        ps = psg.tile(
                            [128, 512], f32, tag="psg", bufs=2, name="ps_proj"
                        )
                        for j in range(DCH // 2):
                            for pi, (a, b) in enumerate(PRODS):
                                _mm(
                                    nc,
                                    ps[:],
                                    wsel(a, j),
                                    xpair(b, j, cs_),
                                    start=(j == 0 and pi == 0),
                                    stop=(j == DCH // 2 - 1 and pi == 2),
                                    perf_mode=DR,
                                    tile_position=(0, 0),
                                )
                        if state["tail"] is not None:
                            state["tail"]()
                        dst = dstp.tile(
                            [128, 512], bf16, tag="dst", bufs=5, name="dst"
                        )
                        nc.vector.tensor_copy(dst[:], ps[:])
                        c1 = tmpp.tile([128, 512], f32, tag="c1", bufs=4, name="c1")
                        nc.gpsimd.tensor_tensor(c1[:], dst[:], cos_sb[:, cs_], mult)

                        def tail():
                            rot = psg.tile(
                                [128, 512], f32, tag="psg", bufs=2, name="ps_rot"
                            )
                            _mm(nc, rot[:], rmat_sb, dst[:], start=True, stop=True)
                            t1 = tmpp.tile(
                                [128, 512], f32, tag="t1", bufs=4, name="t1"
                            )
                            nc.vector.tensor_tensor(
                                t1[:], rot[:], sin_sb[:, cs_], mult
                            )
                            nc.vector.tensor_tensor(q8full[:, cs_], c1[:], t1[:], add)
                            if qc == QC - 1:
                                for g in range(4):
                                    nc.sync.dma_start(
                                        qt[0:32, T * g : T * (g + 1)],
                                        q8full[32 * g : 32 * (g + 1), :],
                                    )

                        state["tail"] = tail

                    return go

                for qc in range(QC):
                    pending_work.append(make_qc(qc))
                pending_work.append(lambda: (state["tail"](), state.update(tail=None)))

            oT = [
                big.tile([128, T], bf16, tag="oT", bufs=4, name=f"oT{m}")
                for m in range(4)
            ]
            tri3 = tri_sb.rearrange("p (two q) -> p two q", two=2)
            # PSUM carries 64x-scaled Q/K (w*64 on host): S is 4096x
            escale = float(1.0 / np.sqrt(D_K)) / 4096.0

            def y_tile(t):
                """output projection for one 128-row token tile. nh=0 uses
                the 'o' psum ring, nh=1 the 'psg' ring (parallel banks)."""
                ty = ysbp.tile([128, 1024], bf16, tag="y", name="ty")
                for nh in range(2):
                    ps = psg.tile([128, 512], f32, tag="psg", bufs=2, name="ps_y")
                    for c in range(4):
                        _mm(
                            nc,
                            ps[:],
                            oT[c][:, 128 * t : 128 * (t + 1)],
                            wo(c)[:, 512 * nh : 512 * (nh + 1)],
                            start=(c == 0),
                            stop=(c == 3),
                        )
                    nc.vector.tensor_copy(ty[:, 512 * nh : 512 * (nh + 1)], ps[:])
                nc.sync.dma_start(y_d[128 * t : 128 * (t + 1), :], ty[:])

            # ---- attention unit (one q-chunk x one head-pair) ---------
            kv4 = [None]
            escale_f = escale

            def attn_unit(qp, hp):
                qsl = slice(512 * qp, 512 * (qp + 1))
                qv4 = qT8[hp][:].rearrange("p (f t) -> p f t", f=4)
                oA = pso.tile([128, 512], f32, tag="o", bufs=2, name="oA")
                oB = pso.tile([128, 512], f32, tag="o", bufs=2, name="oB")
                nkt = 4 * qp + 4
                for kt in range(nkt):
                    a = max(0, 128 * kt - 512 * qp)
                    s = pss.tile([128, 1024], f32, tag="s", name="s")
                    for h in range(2):
                        out_sl = s[:, a:512] if h == 0 else s[:, 512 + a : 1024]
                        _mm(
                            nc,
                            out_sl,
                            kv4[0][:, 2 * h : 2 * h + 2, 128 * kt : 128 * (kt + 1)],
                            qv4[:, 2 * h : 2 * h + 2, 512 * qp + a : 512 * (qp + 1)],
                            start=True,
                            stop=True,
                            perf_mode=DR,
                            tile_position=(0, 0),
                        )
                    e = ep.tile([128, 1024], bf16, tag="e", bufs=4, name="e")
                    if a:
                        sv = s[:].rearrange("p (two q) -> p two q", two=2)[:, :, a:512]
                        ev = e[:].rearrange("p (two q) -> p two q", two=2)[:, :, a:512]
                        nc.scalar.activation(
                            out=ev,
                            in_=sv,
                            func=mybir.ActivationFunctionType.Exp,
                            scale=escale_f,
                        )
                    else:
                        nc.scalar.activation(
                            out=e[:],
                            in_=s[:],
                            func=mybir.ActivationFunctionType.Exp,
                            scale=escale_f,
                        )
                    if kt >= 4 * qp:  # diagonal: causal triangle mask
                        o = 128 * kt - 512 * qp
                        e3 = e[:].rearrange("p (two q) -> p two q", two=2)[
                            :, :, o : o + 128
                        ]
                        nc.vector.tensor_tensor(e3, e3, tri3, mult)
                    st, sp = (kt == 0), (kt == nkt - 1)
                    _mm(
                        nc,
                        oA[0:65, a:512],
                        v_sb[kt][:, 0:65],
                        e[:, a:512],
                        start=st,
                        stop=sp,
                        skip_group_check=True,
                    )
                    _mm(
                        nc,
                        oB[0:65, a:512],
                        v_sb[kt][:, 65:130],
                        e[:, 512 + a : 1024],
                        start=st,
                        stop=sp,
                        skip_group_check=True,
                    )
                    if kt == 1:
                        flush_pending()
                    if kt >= 1:
                        flush_work(1)
                # evacuate O' right away (frees the oA/oB psum ring for the
                # next unit); the rest of the tail is deferred
                oraw = rrp.tile([128, 1024], f32r, tag="rr", bufs=3, name="oraw")
                nc.vector.tensor_copy(oraw[0:65, 0:512], oA[0:65, :])
                nc.vector.tensor_copy(oraw[0:65, 512:1024], oB[0:65, :])

                def tail():
                    rb = rbp.tile([128, 1024], f32, tag="rb", bufs=3, name="rb")
                    scr = drp.tile([1, 1024], f32r, tag="scr", name="scr")
                    nc.sync.dma_start(scr[:], oraw[64:65, :])
                    nc.sync.dma_start(
                        rb[0:64, :].bitcast(f32r), scr[:].to_broadcast((64, 1024))
                    )
                    nc.vector.reciprocal_approx_fast(rb[0:64, :], rb[0:64, :])
                    nc.vector.tensor_tensor(
                        oT[hp][0:64, qsl], oraw[0:64, 0:512], rb[0:64, 0:512], mult
                    )
                    nb = tmpp.tile([128, 512], bf16, tag="nb", bufs=3, name="nb")
                    nc.vector.tensor_tensor(
                        nb[0:64, :], oraw[0:64, 512:1024], rb[0:64, 512:1024], mult
                    )
                    # head B -> oT partitions 64-127 (partition-shift DMA)
                    nc.sync.dma_start(oT[hp][64:128, qsl], nb[0:64, :])
                    if hp == 3:
                        for j in range(4):
                            pending_work.append(lambda j=j: y_tile(4 * qp + j))

                pending_work.append(tail)

            # ---- emission order: saturate ACT early, spread q-projs ---
            warm(130)
            kT8 = q8p.tile([32, 4 * T], f8, tag="dr", bufs=5, name="kT8")
            project_rope(wkpair, kT8, head=True, fold_each=True)
            q_proj(0, head=True)
            for t in range(4):
                v_proj(t, ring="o")
            kv4[0] = kT8[:].rearrange("p (f t) -> p f t", f=4)
            flush_pending()
            attn_unit(0, 0)
            for t in range(4, 8):
                v_proj(t)
            attn_unit(1, 0)
            for t in range(8, 12):
                v_proj(t)
            q_proj_deferred(1)
            attn_unit(2, 0)
            for t in range(12, 16):
                v_proj(t)
            attn_unit(3, 0)
            attn_unit(0, 1)
            q_proj_deferred(2)
            attn_unit(1, 1)
            attn_unit(2, 1)
            attn_unit(3, 1)
            attn_unit(0, 2)
            q_proj_deferred(3)
            attn_unit(1, 2)
            attn_unit(0, 3)
            attn_unit(2, 2)
            attn_unit(1, 3)
            attn_unit(3, 2)
            attn_unit(2, 3)
            attn_unit(3, 3)
            warm(60, ring="o")
            flush_pending()
            while pending_work:
                flush_work(1)

    nc.compile()
    return nc


def _get_program():
    global _PROGRAM
    if _PROGRAM is None:
        _PROGRAM = _build_program()
    return _PROGRAM


def _host_tables():
    """cos/sin [128, T] (two stacked 64-row copies), R^T (lhsT), tri mask."""
    d = D_K
    inv_freq = 1.0 / (ROPE_BASE ** (np.arange(0, d, 2, dtype=np.float32) / d))
    ang = np.arange(T, dtype=np.float32)[:, None] * inv_freq[None, :]  # [T, 32]
    cos64 = np.repeat(np.cos(ang).astype(np.float32), 2, axis=1).T.copy()
    sin64 = np.repeat(np.sin(ang).astype(np.float32), 2, axis=1).T.copy()
    cos128 = np.ascontiguousarray(np.concatenate([cos64, cos64], axis=0))
    sin128 = np.ascontiguousarray(np.concatenate([sin64, sin64], axis=0))
    # rot = R @ q with rot[2i] = -q[2i+1], rot[2i+1] = q[2i]; pass lhsT = R^T
    R = np.zeros((128, 128), dtype=np.float32)
    for i in range(64):
        R[2 * i, 2 * i + 1] = -1.0
        R[2 * i + 1, 2 * i] = 1.0
    rmat = np.ascontiguousarray(R.T)
    tri = np.triu(np.ones((128, 128), dtype=np.float32))  # keep kk <= qq
    tri2 = np.ascontiguousarray(np.concatenate([tri, tri], axis=1))
    return cos128, sin128, rmat, tri2


def _head_perm():
    """chunk m holds local heads (m, m+4) -> permute Wq cols / Wo rows."""
    perm = []
    for m in range(4):
        perm.extend(range(64 * m, 64 * m + 64))
        perm.extend(range(64 * (m + 4), 64 * (m + 4) + 64))
    return np.array(perm)


def _pack_rows(a, rows_per_tile=128):
    """[N*128, C] -> [128, N*C]: tile k's rows become column block k."""
    n = a.shape[0] // rows_per_tile
    return np.ascontiguousarray(
        np.concatenate(
            [a[rows_per_tile * k : rows_per_tile * (k + 1)] for k in range(n)], axis=1
        )
    )


def _pack_wq_mmajor(a):
    """[1024, 512] -> [128, (m, k, 128)]: chunk m's k-tiles contiguous."""
    out = np.empty((128, 4 * 8 * 128), dtype=a.dtype)
    for m in range(4):
        for k in range(8):
            out[:, 1024 * m + 128 * k : 1024 * m + 128 * (k + 1)] = a[
                128 * k : 128 * (k + 1), 128 * m : 128 * (m + 1)
            ]
    return np.ascontiguousarray(out)


def _split8(a):
    """fp8e4m3 hi+lo split of an f32 array."""
    import ml_dtypes

    F8 = ml_dtypes.float8_e4m3
    hi = a.astype(F8)
    lo = (a - hi.astype(np.float32)).astype(F8)
    return hi, lo


def make_in_maps(x, Wq, Wk, Wv, Wo):
    import ml_dtypes

    bf = ml_dtypes.bfloat16
    cos128, sin128, rmat, tri2 = _host_tables()
    perm = _head_perm()
    constb = np.concatenate(
        [rmat, tri2, np.ones((128, 8), dtype=np.float32)], axis=1
    ).astype(bf)
    in_maps = []
    for c in range(N_CORES):
        b, hg = c // 2, c % 2
        xth, xtl = _split8(np.ascontiguousarray(x[b].T))
        wqh, wql = _split8(Wq[:, hg * 512 : (hg + 1) * 512][:, perm] * 64.0)
        wkh, wkl = _split8(Wk[:, hg * 128 : (hg + 1) * 128] * 64.0)
        wvh, wvl = _split8(Wv[:, hg * 128 : (hg + 1) * 128] * 64.0)
        in_maps.append(
            {
                "xtp8h": _pack_rows(xth),
                "xtp8l": _pack_rows(xtl),
                "wqp8h": _pack_wq_mmajor(wqh),
                "wqp8l": _pack_wq_mmajor(wql),
                "wkp8h": _pack_rows(wkh),
                "wkp8l": _pack_rows(wkl),
                "wvp8h": _pack_rows(wvh),
                "wvp8l": _pack_rows(wvl),
                "wop": _pack_rows(
                    Wo[hg * 512 : (hg + 1) * 512, :][perm, :].astype(bf)
                ),
                "constb": constb,
                "costab": cos128.astype(bf),
                "sintab": sin128.astype(bf),
            }
        )
    return in_maps


def kernel(x, attention_mask, Wq, Wk, Wv, Wo, _trace=False, _trace_kwargs=None):
    global LAST_RESULTS
    from concourse import bass_utils

    x = np.asarray(x, dtype=np.float32)
    Wq = np.asarray(Wq, dtype=np.float32)
    Wk = np.asarray(Wk, dtype=np.float32)
    Wv = np.asarray(Wv, dtype=np.float32)
    Wo = np.asarray(Wo, dtype=np.float32)

    nc = _get_program()
    in_maps = make_in_maps(x, Wq, Wk, Wv, Wo)
    res = bass_utils.run_bass_kernel_spmd(
        nc,
        in_maps,
        core_ids=list(range(N_CORES)),
        trace=_trace,
        **(_trace_kwargs or {}),
    )
    LAST_RESULTS = res

    y = np.zeros((B, T, D_MODEL), dtype=np.float32)
    for b in range(B):
        y[b] = np.asarray(res.results[2 * b]["y"], dtype=np.float32) + np.asarray(
            res.results[2 * b + 1]["y"], dtype=np.float32
        )

    # faithful handling of padded (attention_mask == 0) query rows: the
    # reference's mask makes those rows uniform attention over ALL keys.
    am = np.asarray(attention_mask)
    if not np.all(am == 1):
        rep = N_HEADS // NUM_KV_HEADS
        for b in range(B):
            rows = np.where(am[b] == 0)[0]
            if rows.size:
                V = x[b] @ Wv
                Vfull = np.repeat(
                    V.reshape(T, NUM_KV_HEADS, D_K), rep, axis=1
                ).reshape(T, D_MODEL)
                y[b, rows] = (Vfull.mean(axis=0) @ Wo)[None, :]
    return y


# revision 36
# speedup vs baseline: 1.2750x; 1.0146x over previous
"""Trainium2 Bass kernel for causal multi-head attention with RoPE + GQA.

Model: D_MODEL=1024, N_HEADS=16, NUM_KV_HEADS=4, D_K=64, B=4, T=2048.
Sharding (8 cores): core c -> batch b = c//2, head-group hg = c%2
(8 query heads / 2 kv heads per core). Each core computes a partial
output  y_partial = attn_out_local @ Wo[rows of its heads]  and the host
sums the two partials per batch (the tensor-parallel all-reduce happens
at gather time).

Perf design (cost-model driven; ACT exp ~147us is the floor engine):
  - All GEMMs bf16 (1 PE cycle/row) except S = K^T.T Q^T, which runs in
    fp8e4m3 MatmulPerfMode.DoubleRow (0.5 cycles/row) with the d_k=64
    contraction laid out [32 partitions, 2 k-subtiles]. fp32 PSUM
    accumulation everywhere; end-to-end rel_err ~5e-3 (gate 2e-2).
  - Attention is emitted HEAD-PAIR-MAJOR with the q-chunk projections
    interleaved, so ScalarE starts exp'ing ~23us in and stays saturated
    while PE computes the remaining projections underneath it.
  - Each (qp, hp) unit's normalization tail (denominator DRAM-bounce
    broadcast, reciprocal, scale, head-B partition-shift DMA, and the
    previous q-chunk's output projection) is DEFERRED into the next
    unit's kt loop so it never sits between PV and the next S matmul.
  - DMA instruction COUNT is precious (~625ns serialized descriptor-gen
    each): all host inputs are pre-packed for single contiguous DMAs,
    x^T is split qc-major in 4 so the first projections start ~4us in,
    cos/sin load once in bf16.
  - Engine placement: ACT = exp only; Pool = PSUM->SBUF staging + RoPE
    cos-mult; DVE = RoPE sin-mult/add, masking, reciprocal, normalize.

Formulation (features-on-partitions; x arrives host-transposed bf16):
  Q^T = Wq_s^T x^T  [512,2048]    K^T = Wk_s^T x^T  [128,2048]
  V'  = [x @ Wv_s | ones]
  RoPE q*cos + (R q)*sin, R applied by one 128x128 matmul; result
  written fp8e4 and DMA-folded to the DoubleRow [32, (2h+half)*T + t]
  layout.
  S^T = K^T_h.T Q^T_h (fp8 DoubleRow), E^T = exp(S^T/8) -> bf16 (ACT),
  causal triangle on DVE, O'^T = V'_h.T E^T (M=65; row 64 = softmax
  denominator), O^T = O'^T * recip(den), y = O^T.T Wo_s -> bf16 DMA.
Heads are paired (m, m+4) across the two kv groups; Wq columns / Wo
rows are permuted accordingly on the host.
"""

import numpy as np

D_MODEL = 1024
N_HEADS = 16
NUM_KV_HEADS = 4
D_K = 64
ROPE_BASE = 10000.0
B, T = 4, 2048
N_CORES = 8
KT = 16             # 128-row key tiles per sequence
QC = 4              # 512-col query chunks
DCH = 8             # 128-row feature (d_model) tiles

_PROGRAM = None     # cached compiled Bass program
LAST_RESULTS = None  # BassKernelResults of the most recent run


def _mm(nc, out, lhsT, rhs, **kw):
    nc.tensor.matmul(out, lhsT, rhs, **kw)


def _build_program():
    import concourse.mybir as mybir
    import concourse.tile as tile
    from concourse import bacc

    f32 = mybir.dt.float32
    f32r = mybir.dt.float32r
    bf16 = mybir.dt.bfloat16
    f8 = mybir.dt.float8e4
    nc = bacc.Bacc("TRN2", target_bir_lowering=False, debug=False)

    # every input pre-packed on host; x/w as fp8 hi+lo splits (w scaled
    # x64 on host to clear fp8's subnormal floor; compensated via the exp
    # scale and the V' ones value)
    xth_d = nc.dram_tensor("xtp8h", [128, DCH * T], f8, kind="ExternalInput")
    xtl_d = nc.dram_tensor("xtp8l", [128, DCH * T], f8, kind="ExternalInput")
    wqh_d = nc.dram_tensor("wqp8h", [128, DCH * 512], f8, kind="ExternalInput")
    wql_d = nc.dram_tensor("wqp8l", [128, DCH * 512], f8, kind="ExternalInput")
    wkh_d = nc.dram_tensor("wkp8h", [128, DCH * 128], f8, kind="ExternalInput")
    wkl_d = nc.dram_tensor("wkp8l", [128, DCH * 128], f8, kind="ExternalInput")
    wvh_d = nc.dram_tensor("wvp8h", [128, DCH * 128], f8, kind="ExternalInput")
    wvl_d = nc.dram_tensor("wvp8l", [128, DCH * 128], f8, kind="ExternalInput")
    wo_d = nc.dram_tensor("wop", [128, 4 * 1024], bf16, kind="ExternalInput")
    cb_d = nc.dram_tensor("constb", [128, 392], bf16, kind="ExternalInput")
    cos_d = nc.dram_tensor("costab", [128, T], bf16, kind="ExternalInput")
    sin_d = nc.dram_tensor("sintab", [128, T], bf16, kind="ExternalInput")
    y_d = nc.dram_tensor("y", [T, D_MODEL], bf16, kind="ExternalOutput")

    mult = mybir.AluOpType.mult
    add = mybir.AluOpType.add
    div = mybir.AluOpType.divide
    DR = mybir.MatmulPerfMode.DoubleRow

    with tile.TileContext(nc) as tc:
        with (
            tc.tile_pool(name="big", bufs=6) as big,
            tc.tile_pool(name="w", bufs=4) as wp,
            tc.tile_pool(name="const", bufs=1) as constp,
            tc.tile_pool(name="q8", bufs=5) as q8p,
            tc.tile_pool(name="vt", bufs=16) as vtp,
            tc.tile_pool(name="dst", bufs=5) as dstp,
            tc.tile_pool(name="tmp", bufs=4) as tmpp,
            tc.tile_pool(name="e", bufs=4) as ep,
            tc.tile_pool(name="rr", bufs=3) as rrp,
            tc.tile_pool(name="rb", bufs=1) as rbp,
            tc.tile_pool(name="ysb", bufs=2) as ysbp,
            tc.tile_pool(name="dr", bufs=2, space="DRAM") as drp,
            tc.tile_pool(name="ps_g", bufs=2, space="PSUM") as psg,
            tc.tile_pool(name="ps_s", bufs=2, space="PSUM") as pss,
            tc.tile_pool(name="ps_o", bufs=2, space="PSUM") as pso,
        ):
            # ---- input loads, ordered for earliest compute start ------
            cb = constp.tile([128, 392], bf16, tag="cb", name="cb")
            nc.sync.dma_start(cb[:], cb_d[:])
            xt_sb = [
                big.tile([128, DCH * T], f8, tag=f"xt{i}", bufs=1, name=f"xt8{i}")
                for i in range(2)
            ]
            xt3 = [t[:].rearrange("p (k t) -> p k t", k=DCH) for t in xt_sb]
            xtd3 = [
                d[:].rearrange("p (k t) -> p k t", k=DCH) for d in (xth_d, xtl_d)
            ]
            cs0 = slice(0, 512)
            nc.sync.dma_start(xt3[0][:, :, cs0], xtd3[0][:, :, cs0])
            nc.sync.dma_start(xt3[1][:, :, cs0], xtd3[1][:, :, cs0])
            wk_sb = [
                wp.tile([128, DCH * 128], f8, tag=f"wk{i}", bufs=1, name=f"wk8{i}")
                for i in range(2)
            ]
            nc.sync.dma_start(wk_sb[0][:], wkh_d[:])
            nc.sync.dma_start(wk_sb[1][:], wkl_d[:])
            wq_sb = [
                wp.tile([128, DCH * 512], f8, tag=f"wq{i}", bufs=1, name=f"wq8{i}")
                for i in range(2)
            ]
            nc.sync.dma_start(wq_sb[0][:, 0:1024], wqh_d[:, 0:1024])
            nc.sync.dma_start(wq_sb[1][:, 0:1024], wql_d[:, 0:1024])
            cos_sb = constp.tile([128, T], bf16, tag="cos", name="cos_sb")
            nc.sync.dma_start(cos_sb[:], cos_d[:])
            sin_sb = constp.tile([128, T], bf16, tag="sin", name="sin_sb")
            nc.sync.dma_start(sin_sb[:], sin_d[:])
            wv_sb = [
                wp.tile([128, DCH * 128], f8, tag=f"wv{i}", bufs=1, name=f"wv8{i}")
                for i in range(2)
            ]
            nc.sync.dma_start(wv_sb[0][:], wvh_d[:])
            nc.sync.dma_start(wv_sb[1][:], wvl_d[:])
            for qc in range(1, QC):
                cs_ = slice(512 * qc, 512 * (qc + 1))
                nc.sync.dma_start(xt3[0][:, :, cs_], xtd3[0][:, :, cs_])
                nc.sync.dma_start(xt3[1][:, :, cs_], xtd3[1][:, :, cs_])
            nc.sync.dma_start(wq_sb[0][:, 1024:4096], wqh_d[:, 1024:4096])
            nc.sync.dma_start(wq_sb[1][:, 1024:4096], wql_d[:, 1024:4096])
            wo_all = wp.tile([128, 4 * 1024], bf16, tag="wo", bufs=1, name="wo_all")
            nc.sync.dma_start(wo_all[:], wo_d[:])

            # pair views for DoubleRow: [128, 2 k-subtiles, cols]
            wk3 = [t[:].rearrange("p (k j) -> p k j", k=DCH) for t in wk_sb]
            wv3 = [t[:].rearrange("p (k j) -> p k j", k=DCH) for t in wv_sb]
            wq3 = [t[:].rearrange("p (g j) -> p g j", g=4 * DCH) for t in wq_sb]

            def xpair(b, j, cs_):
                return xt3[b][:, 2 * j : 2 * j + 2, cs_]

            def wkpair(a, j):
                return wk3[a][:, 2 * j : 2 * j + 2, :]

            def wqpair(m, a, j):
                # m-major packing: group g = 8*m + k
                return wq3[a][:, 8 * m + 2 * j : 8 * m + 2 * j + 2, :]

            def wo(c):
                return wo_all[:, 1024 * c : 1024 * (c + 1)]

            PRODS = ((0, 0), (1, 0), (0, 1))  # (w hi/lo, x hi/lo)

            rmat_sb = cb[:, 0:128]
            tri_sb = cb[:, 128:384]
            ones_bf = cb[:, 384:392]

            pending = []      # head-phase rope tails: flushed whole
            pending_work = []  # steady-state closures: flushed 1 per kt

            def flush_pending():
                for f in pending:
                    f()
                pending.clear()

            def flush_work(n=1):
                for _ in range(n):
                    if not pending_work:
                        return
                    pending_work.pop(0)()

            # PE pstate warm-up: the cost model charges LOW/MID clocks to
            # matmuls decoded within 3us of an idle->busy transition, so
            # keep PE trivially busy across head-phase DMA waits.
            def warm(n, ring="psg"):
                if ring == "o":
                    wt = pso.tile([128, 512], f32, tag="o", bufs=2, name="warm")
                else:
                    wt = psg.tile([128, 512], f32, tag="psg", bufs=2, name="warm")
                for _ in range(n):
                    _mm(
                        nc,
                        wt[0:64, 0:64],
                        cb[:, 0:64],
                        cb[:, 0:64],
                        start=True,
                        stop=True,
                        skip_group_check=True,
                    )

            # ---- fused projection + RoPE -> fp8 DoubleRow layout ------
            # dr layout: [32 partitions, (2*head + khalf)*T + t]
            # The rope tail (rot matmul + cos/sin combine) of chunk qc is
            # deferred until after chunk qc+1's projection matmuls: the
            # tile scheduler is run-ahead in-order-with-skip per engine,
            # so an op emitted before its input is ready gets parked
            # until the engine idles (which PE never does).
            def project_rope(wpair, dr_dst, head=False, fold_each=False):
                q8full = q8p.tile([128, T], f8, tag="q8f", bufs=2, name="q8full")

                def make_tail(qc, ps):
                    cs_ = slice(512 * qc, 512 * (qc + 1))
                    dst = dstp.tile([128, 512], bf16, tag="dst", bufs=5, name="dst")
                    if head:
                        nc.scalar.copy(dst[:], ps[:])
                    else:
                        nc.vector.tensor_copy(dst[:], ps[:])
                    # cos-multiply needs only dst: run it right away on Pool
                    c1 = tmpp.tile([128, 512], f32, tag="c1", bufs=4, name="c1")
                    nc.gpsimd.tensor_tensor(c1[:], dst[:], cos_sb[:, cs_], mult)

                    def tail():
                        rot = psg.tile([128, 512], f32, tag="psg", bufs=2, name="ps_rot")
                        _mm(nc, rot[:], rmat_sb, dst[:], start=True, stop=True)
                        t1 = tmpp.tile([128, 512], f32, tag="t1", bufs=4, name="t1")
                        nc.vector.tensor_tensor(t1[:], rot[:], sin_sb[:, cs_], mult)
                        nc.vector.tensor_tensor(q8full[:, cs_], c1[:], t1[:], add)
                        # partition fold into DoubleRow layout. For head
                        # tiles: fold qc0 alone (lets attention start on
                        # partial K/Q) and qc1-3 in one batch (HWDGE issue
                        # slots are ~625ns each and get scarce in the head)
                        if fold_each and qc == 0:
                            for g in range(4):
                                nc.sync.dma_start(
                                    dr_dst[0:32, T * g : T * g + 512],
                                    q8full[32 * g : 32 * (g + 1), 0:512],
                                )
                        elif qc == QC - 1:
                            lo = 512 if fold_each else 0
                            for g in range(4):
                                nc.sync.dma_start(
                                    dr_dst[0:32, T * g + lo : T * (g + 1)],
                                    q8full[32 * g : 32 * (g + 1), lo:],
                                )

                    return tail

                prev_tail = None
                for qc in range(QC):
                    cs_ = slice(512 * qc, 512 * (qc + 1))
                    if head and qc % 2 == 0:
                        ps = pso.tile([128, 512], f32, tag="o", bufs=2, name="ps_proj")
                    else:
                        ps = psg.tile([128, 512], f32, tag="psg", bufs=2, name="ps_proj")
                    for j in range(DCH // 2):
                        for pi, (a, b) in enumerate(PRODS):
                            _mm(
                                nc,
                                ps[:],
                                wpair(a, j),
                                xpair(b, j, cs_),
                                start=(j == 0 and pi == 0),
                                stop=(j == DCH // 2 - 1 and pi == 2),
                                perf_mode=DR,
                                tile_position=(0, 0),
                            )
                    if qc == 0:
                        # previous projection's last rope tail rides right
                        # behind this chunk's matmuls on the PE queue
                        flush_pending()
                    if head:
                        warm(18)
                    if prev_tail is not None:
                        prev_tail()
                    prev_tail = make_tail(qc, ps)
                pending.append(prev_tail)

            v_all = vtp.tile([128, KT * 130], bf16, tag="v", bufs=1, name="v_all")
            v3 = v_all[:].rearrange("p (t c) -> p t c", t=KT)
            # V rows carry 64x-scaled V; ones row = 64 keeps num/den exact
            nc.vector.memset(v3[:, :, 64:65], 64.0)
            nc.vector.memset(v3[:, :, 129:130], 64.0)

            def v_proj(t, ring=None):
                if ring == "o":
                    ps = pso.tile([128, 512], f32, tag="o", bufs=2, name="ps_v")
                else:
                    ps = psg.tile([128, 512], f32, tag="psg", bufs=2, name="ps_v")
                for j in range(DCH // 2):
                    for pi, (a, b) in enumerate(PRODS):
                        _mm(
                            nc,
                            ps[:, 0:128],
                            xt3[b][:, 2 * j : 2 * j + 2, 128 * t : 128 * (t + 1)],
                            wv3[a][:, 2 * j : 2 * j + 2, :],
                            start=(j == 0 and pi == 0),
                            stop=(j == DCH // 2 - 1 and pi == 2),
                            perf_mode=DR,
                            tile_position=(0, 0),
                        )
                vt = v_all[:, 130 * t : 130 * (t + 1)]
                nc.vector.tensor_copy(vt[:, 0:64], ps[:, 0:64])
                nc.vector.tensor_copy(vt[:, 65:129], ps[:, 64:128])
                v_sb.append(vt)

            v_sb = []
            qT8 = []

            def q_proj(m, head=False):
                qt = q8p.tile([32, 4 * T], f8, tag="dr", bufs=5, name=f"qT8{m}")
                project_rope(
                    lambda a, j: wqpair(m, a, j), qt, head=head, fold_each=head
                )
                qT8.append(qt)

            def q_proj_deferred(m):
                """emit q-chunk m's projection as per-qc closures so the
                attention kt loop interleaves them 1.7us at a time."""
                qt = q8p.tile([32, 4 * T], f8, tag="dr", bufs=5, name=f"qT8{m}")
                qT8.append(qt)
                wsel = lambda a, j: wqpair(m, a, j)
                q8full = q8p.tile([128, T], f8, tag="q8f", bufs=2, name="q8full")
                state = {"tail": None}

                def make_qc(qc):
                    cs_ = slice(512 * qc, 512 * (qc + 1))

                    def go():
                        ps = psg.tile(
                            [128, 512], f32, tag="psg", bufs=2, name="ps_proj"
                        )
                        for j in range(DCH // 2):
                            for pi, (a, b) in enumerate(PRODS):
                                _mm(
                                    nc,
                                    ps[:],
                                    wsel(a, j),
                                    xpair(b, j, cs_),
                                    start=(j == 0 and pi == 0),
                                    stop=(j == DCH // 2 - 1 and pi == 2),
                                    perf_mode=DR,
                                    tile_position=(0, 0),
                                )
                        if state["tail"] is not None:
                            state["tail"]()
                        dst = dstp.tile(
                            [128, 512], bf16, tag="dst", bufs=5, name="dst"
                        )
                        nc.vector.tensor_copy(dst[:], ps[:])
                        c1 = tmpp.tile([128, 512], f32, tag="c1", bufs=4, name="c1")
                        nc.gpsimd.tensor_tensor(c1[:], dst[:], cos_sb[:, cs_], mult)

                        def tail():
                            rot = psg.tile(
                                [128, 512], f32, tag="psg", bufs=2, name="ps_rot"
                            )
                            _mm(nc, rot[:], rmat_sb, dst[:], start=True, stop=True)
                            t1 = tmpp.tile(
                                [128, 512], f32, tag="t1", bufs=4, name="t1"
                            )
                            nc.vector.tensor_tensor(
                                t1[:], rot[:], sin_sb[:, cs_], mult
                            )
                            nc.vector.tensor_tensor(q8full[:, cs_], c1[:], t1[:], add)
                            if qc == QC - 1:
                                for g in range(4):
                                    nc.sync.dma_start(
                                        qt[0:32, T * g : T * (g + 1)],
                                        q8full[32 * g : 32 * (g + 1), :],
                                    )

                        state["tail"] = tail

                    return go

                for qc in range(QC):
                    pending_work.append(make_qc(qc))
                pending_work.append(lambda: (state["tail"](), state.update(tail=None)))

            oT = [
                big.tile([128, T], bf16, tag="oT", bufs=4, name=f"oT{m}")
                for m in range(4)
            ]
            tri3 = tri_sb.rearrange("p (two q) -> p two q", two=2)
            # PSUM carries 64x-scaled Q/K (w*64 on host): S is 4096x
            escale = float(1.0 / np.sqrt(D_K)) / 4096.0

            def y_tile(t):
                """output projection for one 128-row token tile. nh=0 uses
                the 'o' psum ring, nh=1 the 'psg' ring (parallel banks)."""
                ty = ysbp.tile([128, 1024], bf16, tag="y", name="ty")
                for nh in range(2):
                    ps = psg.tile([128, 512], f32, tag="psg", bufs=2, name="ps_y")
                    for c in range(4):
                        _mm(
                            nc,
                            ps[:],
                            oT[c][:, 128 * t : 128 * (t + 1)],
                            wo(c)[:, 512 * nh : 512 * (nh + 1)],
                            start=(c == 0),
                            stop=(c == 3),
                        )
                    nc.vector.tensor_copy(ty[:, 512 * nh : 512 * (nh + 1)], ps[:])
                nc.sync.dma_start(y_d[128 * t : 128 * (t + 1), :], ty[:])

            # ---- attention unit (one q-chunk x one head-pair) ---------
            kv4 = [None]
            escale_f = escale

            def attn_unit(qp, hp):
                qsl = slice(512 * qp, 512 * (qp + 1))
                qv4 = qT8[hp][:].rearrange("p (f t) -> p f t", f=4)
                oA = pso.tile([128, 512], f32, tag="o", bufs=2, name="oA")
                oB = pso.tile([128, 512], f32, tag="o", bufs=2, name="oB")
                nkt = 4 * qp + 4
                for kt in range(nkt):
                    a = max(0, 128 * kt - 512 * qp)
                    s = pss.tile([128, 1024], f32, tag="s", name="s")
                    for h in range(2):
                        out_sl = s[:, a:512] if h == 0 else s[:, 512 + a : 1024]
                        _mm(
                            nc,
                            out_sl,
                            kv4[0][:, 2 * h : 2 * h + 2, 128 * kt : 128 * (kt + 1)],
                            qv4[:, 2 * h : 2 * h + 2, 512 * qp + a : 512 * (qp + 1)],
                            start=True,
                            stop=True,
                            perf_mode=DR,
                            tile_position=(0, 0),
                        )
                    e = ep.tile([128, 1024], bf16, tag="e", bufs=4, name="e")
                    if a:
                        sv = s[:].rearrange("p (two q) -> p two q", two=2)[:, :, a:512]
                        ev = e[:].rearrange("p (two q) -> p two q", two=2)[:, :, a:512]
                        nc.scalar.activation(
                            out=ev,
                            in_=sv,
                            func=mybir.ActivationFunctionType.Exp,
                            scale=escale_f,
                        )
                    else:
                        nc.scalar.activation(
                            out=e[:],
                            in_=s[:],
                            func=mybir.ActivationFunctionType.Exp,
                            scale=escale_f,
                        )
                    if kt >= 4 * qp:  # diagonal: causal triangle mask
                        o = 128 * kt - 512 * qp
                        e3 = e[:].rearrange("p (two q) -> p two q", two=2)[
                            :, :, o : o + 128
                        ]
                        nc.vector.tensor_tensor(e3, e3, tri3, mult)
                    st, sp = (kt == 0), (kt == nkt - 1)
                    _mm(
                        nc,
                        oA[0:65, a:512],
                        v_sb[kt][:, 0:65],
                        e[:, a:512],
                        start=st,
                        stop=sp,
                        skip_group_check=True,
                    )
                    _mm(
                        nc,
                        oB[0:65, a:512],
                        v_sb[kt][:, 65:130],
                        e[:, 512 + a : 1024],
                        start=st,
                        stop=sp,
                        skip_group_check=True,
                    )
                    if kt == 1:
                        flush_pending()
                    if kt >= 1:
                        flush_work(1)
                # evacuate O' right away (frees the oA/oB psum ring for the
                # next unit); the rest of the tail is deferred
                oraw = rrp.tile([128, 1024], f32r, tag="rr", bufs=3, name="oraw")
                nc.vector.tensor_copy(oraw[0:65, 0:512], oA[0:65, :])
                nc.vector.tensor_copy(oraw[0:65, 512:1024], oB[0:65, :])

                is_last = qp == 3 and hp == 3

                def tail():
                    rb = rbp.tile([128, 1024], f32, tag="rb", bufs=3, name="rb")
                    scr = drp.tile([1, 1024], f32r, tag="scr", name="scr")
                    nc.sync.dma_start(scr[:], oraw[64:65, :])
                    nc.sync.dma_start(
                        rb[0:64, :].bitcast(f32r), scr[:].to_broadcast((64, 1024))
                    )
                    nc.vector.reciprocal_approx_fast(rb[0:64, :], rb[0:64, :])
                    if is_last:
                        # terminal tail: normalize per 128-token slice so
                        # each y_tile launches as soon as its slice lands
                        nb = tmpp.tile([128, 512], bf16, tag="nb", bufs=3, name="nb")
                        for j in range(4):
                            js = slice(128 * j, 128 * (j + 1))
                            jq = slice(512 * qp + 128 * j, 512 * qp + 128 * (j + 1))
                            nc.vector.tensor_tensor(
                                oT[hp][0:64, jq],
                                oraw[0:64, js],
                                rb[0:64, js],
                                mult,
                            )
                            nc.vector.tensor_tensor(
                                nb[0:64, js],
                                oraw[0:64, 512 + 128 * j : 512 + 128 * (j + 1)],
                                rb[0:64, 512 + 128 * j : 512 + 128 * (j + 1)],
                                mult,
                            )
                            nc.sync.dma_start(oT[hp][64:128, jq], nb[0:64, js])
                            y_tile(4 * qp + j)
                        return
                    nc.vector.tensor_tensor(
                        oT[hp][0:64, qsl], oraw[0:64, 0:512], rb[0:64, 0:512], mult
                    )
                    nb = tmpp.tile([128, 512], bf16, tag="nb", bufs=3, name="nb")
                    nc.vector.tensor_tensor(
                        nb[0:64, :], oraw[0:64, 512:1024], rb[0:64, 512:1024], mult
                    )
                    # head B -> oT partitions 64-127 (partition-shift DMA)
                    nc.sync.dma_start(oT[hp][64:128, qsl], nb[0:64, :])
                    if hp == 3:
                        for j in range(4):
                            pending_work.append(lambda j=j: y_tile(4 * qp + j))

                pending_work.append(tail)

            # ---- emission order: saturate ACT early, spread q-projs ---
            warm(130)
            kT8 = q8p.tile([32, 4 * T], f8, tag="dr", bufs=5, name="kT8")
            project_rope(wkpair, kT8, head=True, fold_each=True)
            q_proj(0, head=True)
            for t in range(4):
                v_proj(t, ring="o")
            kv4[0] = kT8[:].rearrange("p (f t) -> p f t", f=4)
            flush_pending()
            attn_unit(0, 0)
            for t in range(4, 8):
                v_proj(t)
            attn_unit(1, 0)
            for t in range(8, 12):
                v_proj(t)
            q_proj_deferred(1)
            attn_unit(2, 0)
            for t in range(12, 16):
                v_proj(t)
            attn_unit(3, 0)
            attn_unit(0, 1)
            q_proj_deferred(2)
            attn_unit(1, 1)
            attn_unit(2, 1)
            attn_unit(3, 1)
            attn_unit(0, 2)
            q_proj_deferred(3)
            attn_unit(1, 2)
            attn_unit(0, 3)
            attn_unit(2, 2)
            attn_unit(1, 3)
            attn_unit(3, 2)
            attn_unit(2, 3)
            attn_unit(3, 3)
            warm(60, ring="o")
            flush_pending()
            while pending_work:
                flush_work(1)

    nc.compile()
    return nc


def _get_program():
    global _PROGRAM
    if _PROGRAM is None:
        _PROGRAM = _build_program()
    return _PROGRAM


def _host_tables():
    """cos/sin [128, T] (two stacked 64-row copies), R^T (lhsT), tri mask."""
    d = D_K
    inv_freq = 1.0 / (ROPE_BASE ** (np.arange(0, d, 2, dtype=np.float32) / d))
    ang = np.arange(T, dtype=np.float32)[:, None] * inv_freq[None, :]  # [T, 32]
    cos64 = np.repeat(np.cos(ang).astype(np.float32), 2, axis=1).T.copy()
    sin64 = np.repeat(np.sin(ang).astype(np.float32), 2, axis=1).T.copy()
    cos128 = np.ascontiguousarray(np.concatenate([cos64, cos64], axis=0))
    sin128 = np.ascontiguousarray(np.concatenate([sin64, sin64], axis=0))
    # rot = R @ q with rot[2i] = -q[2i+1], rot[2i+1] = q[2i]; pass lhsT = R^T
    R = np.zeros((128, 128), dtype=np.float32)
    for i in range(64):
        R[2 * i, 2 * i + 1] = -1.0
        R[2 * i + 1, 2 * i] = 1.0
    rmat = np.ascontiguousarray(R.T)
    tri = np.triu(np.ones((128, 128), dtype=np.float32))  # keep kk <= qq
    tri2 = np.ascontiguousarray(np.concatenate([tri, tri], axis=1))
    return cos128, sin128, rmat, tri2


def _head_perm():
    """chunk m holds local heads (m, m+4) -> permute Wq cols / Wo rows."""
    perm = []
    for m in range(4):
        perm.extend(range(64 * m, 64 * m + 64))
        perm.extend(range(64 * (m + 4), 64 * (m + 4) + 64))
    return np.array(perm)


def _pack_rows(a, rows_per_tile=128):
    """[N*128, C] -> [128, N*C]: tile k's rows become column block k."""
    n = a.shape[0] // rows_per_tile
    return np.ascontiguousarray(
        np.concatenate(
            [a[rows_per_tile * k : rows_per_tile * (k + 1)] for k in range(n)], axis=1
        )
    )


def _pack_wq_mmajor(a):
    """[1024, 512] -> [128, (m, k, 128)]: chunk m's k-tiles contiguous."""
    out = np.empty((128, 4 * 8 * 128), dtype=a.dtype)
    for m in range(4):
        for k in range(8):
            out[:, 1024 * m + 128 * k : 1024 * m + 128 * (k + 1)] = a[
                128 * k : 128 * (k + 1), 128 * m : 128 * (m + 1)
            ]
    return np.ascontiguousarray(out)


def _split8(a):
    """fp8e4m3 hi+lo split of an f32 array."""
    import ml_dtypes

    F8 = ml_dtypes.float8_e4m3
    hi = a.astype(F8)
    lo = (a - hi.astype(np.float32)).astype(F8)
    return hi, lo


def make_in_maps(x, Wq, Wk, Wv, Wo):
    import ml_dtypes

    bf = ml_dtypes.bfloat16
    cos128, sin128, rmat, tri2 = _host_tables()
    perm = _head_perm()
    constb = np.concatenate(
        [rmat, tri2, np.ones((128, 8), dtype=np.float32)], axis=1
    ).astype(bf)
    in_maps = []
    for c in range(N_CORES):
        b, hg = c // 2, c % 2
        xth, xtl = _split8(np.ascontiguousarray(x[b].T))
        wqh, wql = _split8(Wq[:, hg * 512 : (hg + 1) * 512][:, perm] * 64.0)
        wkh, wkl = _split8(Wk[:, hg * 128 : (hg + 1) * 128] * 64.0)
        wvh, wvl = _split8(Wv[:, hg * 128 : (hg + 1) * 128] * 64.0)
        in_maps.append(
            {
                "xtp8h": _pack_rows(xth),
                "xtp8l": _pack_rows(xtl),
                "wqp8h": _pack_wq_mmajor(wqh),
                "wqp8l": _pack_wq_mmajor(wql),
                "wkp8h": _pack_rows(wkh),
                "wkp8l": _pack_rows(wkl),
                "wvp8h": _pack_rows(wvh),
                "wvp8l": _pack_rows(wvl),
                "wop": _pack_rows(
                    Wo[hg * 512 : (hg + 1) * 512, :][perm, :].astype(bf)
                ),
                "constb": constb,
                "costab": cos128.astype(bf),
                "sintab": sin128.astype(bf),
            }
        )
    return in_maps


def kernel(x, attention_mask, Wq, Wk, Wv, Wo, _trace=False, _trace_kwargs=None):
    global LAST_RESULTS
    from concourse import bass_utils

    x = np.asarray(x, dtype=np.float32)
    Wq = np.asarray(Wq, dtype=np.float32)
    Wk = np.asarray(Wk, dtype=np.float32)
    Wv = np.asarray(Wv, dtype=np.float32)
    Wo = np.asarray(Wo, dtype=np.float32)

    nc = _get_program()
    in_maps = make_in_maps(x, Wq, Wk, Wv, Wo)
    res = bass_utils.run_bass_kernel_spmd(
        nc,
        in_maps,
        core_ids=list(range(N_CORES)),
        trace=_trace,
        **(_trace_kwargs or {}),
    )
    LAST_RESULTS = res

    y = np.zeros((B, T, D_MODEL), dtype=np.float32)
    for b in range(B):
        y[b] = np.asarray(res.results[2 * b]["y"], dtype=np.float32) + np.asarray(
            res.results[2 * b + 1]["y"], dtype=np.float32
        )

    # faithful handling of padded (attention_mask == 0) query rows: the
    # reference's mask makes those rows uniform attention over ALL keys.
    am = np.asarray(attention_mask)
    if not np.all(am == 1):
        rep = N_HEADS // NUM_KV_HEADS
        for b in range(B):
            rows = np.where(am[b] == 0)[0]
            if rows.size:
                V = x[b] @ Wv
                Vfull = np.repeat(
                    V.reshape(T, NUM_KV_HEADS, D_K), rep, axis=1
                ).reshape(T, D_MODEL)
                y[b, rows] = (Vfull.mean(axis=0) @ Wo)[None, :]
    return y


# revision 37
# speedup vs baseline: 1.2772x; 1.0017x over previous
"""Trainium2 Bass kernel for causal multi-head attention with RoPE + GQA.

Model: D_MODEL=1024, N_HEADS=16, NUM_KV_HEADS=4, D_K=64, B=4, T=2048.
Sharding (8 cores): core c -> batch b = c//2, head-group hg = c%2
(8 query heads / 2 kv heads per core). Each core computes a partial
output  y_partial = attn_out_local @ Wo[rows of its heads]  and the host
sums the two partials per batch (the tensor-parallel all-reduce happens
at gather time).

Perf design (cost-model driven; ACT exp ~147us is the floor engine):
  - All GEMMs bf16 (1 PE cycle/row) except S = K^T.T Q^T, which runs in
    fp8e4m3 MatmulPerfMode.DoubleRow (0.5 cycles/row) with the d_k=64
    contraction laid out [32 partitions, 2 k-subtiles]. fp32 PSUM
    accumulation everywhere; end-to-end rel_err ~5e-3 (gate 2e-2).
  - Attention is emitted HEAD-PAIR-MAJOR with the q-chunk projections
    interleaved, so ScalarE starts exp'ing ~23us in and stays saturated
    while PE computes the remaining projections underneath it.
  - Each (qp, hp) unit's normalization tail (denominator DRAM-bounce
    broadcast, reciprocal, scale, head-B partition-shift DMA, and the
    previous q-chunk's output projection) is DEFERRED into the next
    unit's kt loop so it never sits between PV and the next S matmul.
  - DMA instruction COUNT is precious (~625ns serialized descriptor-gen
    each): all host inputs are pre-packed for single contiguous DMAs,
    x^T is split qc-major in 4 so the first projections start ~4us in,
    cos/sin load once in bf16.
  - Engine placement: ACT = exp only; Pool = PSUM->SBUF staging + RoPE
    cos-mult; DVE = RoPE sin-mult/add, masking, reciprocal, normalize.

Formulation (features-on-partitions; x arrives host-transposed bf16):
  Q^T = Wq_s^T x^T  [512,2048]    K^T = Wk_s^T x^T  [128,2048]
  V'  = [x @ Wv_s | ones]
  RoPE q*cos + (R q)*sin, R applied by one 128x128 matmul; result
  written fp8e4 and DMA-folded to the DoubleRow [32, (2h+half)*T + t]
  layout.
  S^T = K^T_h.T Q^T_h (fp8 DoubleRow), E^T = exp(S^T/8) -> bf16 (ACT),
  causal triangle on DVE, O'^T = V'_h.T E^T (M=65; row 64 = softmax
  denominator), O^T = O'^T * recip(den), y = O^T.T Wo_s -> bf16 DMA.
Heads are paired (m, m+4) across the two kv groups; Wq columns / Wo
rows are permuted accordingly on the host.
"""

import numpy as np

D_MODEL = 1024
N_HEADS = 16
NUM_KV_HEADS = 4
D_K = 64
ROPE_BASE = 10000.0
B, T = 4, 2048
N_CORES = 8
KT = 16             # 128-row key tiles per sequence
QC = 4              # 512-col query chunks
DCH = 8             # 128-row feature (d_model) tiles

_PROGRAM = None     # cached compiled Bass program
LAST_RESULTS = None  # BassKernelResults of the most recent run


def _mm(nc, out, lhsT, rhs, **kw):
    nc.tensor.matmul(out, lhsT, rhs, **kw)


def _build_program():
    import concourse.mybir as mybir
    import concourse.tile as tile
    from concourse import bacc

    f32 = mybir.dt.float32
    f32r = mybir.dt.float32r
    bf16 = mybir.dt.bfloat16
    f8 = mybir.dt.float8e4
    nc = bacc.Bacc("TRN2", target_bir_lowering=False, debug=False)

    # every input pre-packed on host; x/w as fp8 hi+lo splits (w scaled
    # x64 on host to clear fp8's subnormal floor; compensated via the exp
    # scale and the V' ones value)
    xth_d = nc.dram_tensor("xtp8h", [128, DCH * T], f8, kind="ExternalInput")
    xtl_d = nc.dram_tensor("xtp8l", [128, DCH * T], f8, kind="ExternalInput")
    wqh_d = nc.dram_tensor("wqp8h", [128, DCH * 512], f8, kind="ExternalInput")
    wql_d = nc.dram_tensor("wqp8l", [128, DCH * 512], f8, kind="ExternalInput")
    wkh_d = nc.dram_tensor("wkp8h", [128, DCH * 128], f8, kind="ExternalInput")
    wkl_d = nc.dram_tensor("wkp8l", [128, DCH * 128], f8, kind="ExternalInput")
    wvh_d = nc.dram_tensor("wvp8h", [128, DCH * 128], f8, kind="ExternalInput")
    wvl_d = nc.dram_tensor("wvp8l", [128, DCH * 128], f8, kind="ExternalInput")
    wo_d = nc.dram_tensor("wop", [128, 4 * 1024], bf16, kind="ExternalInput")
    cb_d = nc.dram_tensor("constb", [128, 392], bf16, kind="ExternalInput")
    cos_d = nc.dram_tensor("costab", [128, T], bf16, kind="ExternalInput")
    sin_d = nc.dram_tensor("sintab", [128, T], bf16, kind="ExternalInput")
    y_d = nc.dram_tensor("y", [T, D_MODEL], bf16, kind="ExternalOutput")

    mult = mybir.AluOpType.mult
    add = mybir.AluOpType.add
    div = mybir.AluOpType.divide
    DR = mybir.MatmulPerfMode.DoubleRow

    with tile.TileContext(nc) as tc:
        with (
            tc.tile_pool(name="big", bufs=6) as big,
            tc.tile_pool(name="w", bufs=4) as wp,
            tc.tile_pool(name="const", bufs=1) as constp,
            tc.tile_pool(name="q8", bufs=5) as q8p,
            tc.tile_pool(name="vt", bufs=16) as vtp,
            tc.tile_pool(name="dst", bufs=5) as dstp,
            tc.tile_pool(name="tmp", bufs=4) as tmpp,
            tc.tile_pool(name="e", bufs=4) as ep,
            tc.tile_pool(name="rr", bufs=3) as rrp,
            tc.tile_pool(name="rb", bufs=1) as rbp,
            tc.tile_pool(name="ysb", bufs=2) as ysbp,
            tc.tile_pool(name="dr", bufs=2, space="DRAM") as drp,
            tc.tile_pool(name="ps_g", bufs=2, space="PSUM") as psg,
            tc.tile_pool(name="ps_s", bufs=2, space="PSUM") as pss,
            tc.tile_pool(name="ps_o", bufs=2, space="PSUM") as pso,
        ):
            # ---- input loads, ordered for earliest compute start ------
            cb = constp.tile([128, 392], bf16, tag="cb", name="cb")
            nc.sync.dma_start(cb[:], cb_d[:])
            xt_sb = [
                big.tile([128, DCH * T], f8, tag=f"xt{i}", bufs=1, name=f"xt8{i}")
                for i in range(2)
            ]
            xt3 = [t[:].rearrange("p (k t) -> p k t", k=DCH) for t in xt_sb]
            xtd3 = [
                d[:].rearrange("p (k t) -> p k t", k=DCH) for d in (xth_d, xtl_d)
            ]
            cs0 = slice(0, 512)
            nc.sync.dma_start(xt3[0][:, :, cs0], xtd3[0][:, :, cs0])
            nc.sync.dma_start(xt3[1][:, :, cs0], xtd3[1][:, :, cs0])
            wk_sb = [
                wp.tile([128, DCH * 128], f8, tag=f"wk{i}", bufs=1, name=f"wk8{i}")
                for i in range(2)
            ]
            nc.sync.dma_start(wk_sb[0][:], wkh_d[:])
            nc.sync.dma_start(wk_sb[1][:], wkl_d[:])
            wq_sb = [
                wp.tile([128, DCH * 512], f8, tag=f"wq{i}", bufs=1, name=f"wq8{i}")
                for i in range(2)
            ]
            nc.sync.dma_start(wq_sb[0][:, 0:1024], wqh_d[:, 0:1024])
            nc.sync.dma_start(wq_sb[1][:, 0:1024], wql_d[:, 0:1024])
            cos_sb = constp.tile([128, T], bf16, tag="cos", name="cos_sb")
            nc.sync.dma_start(cos_sb[:], cos_d[:])
            sin_sb = constp.tile([128, T], bf16, tag="sin", name="sin_sb")
            nc.sync.dma_start(sin_sb[:], sin_d[:])
            wv_sb = [
                wp.tile([128, DCH * 128], f8, tag=f"wv{i}", bufs=1, name=f"wv8{i}")
                for i in range(2)
            ]
            nc.sync.dma_start(wv_sb[0][:], wvh_d[:])
            nc.sync.dma_start(wv_sb[1][:], wvl_d[:])
            for qc in range(1, QC):
                cs_ = slice(512 * qc, 512 * (qc + 1))
                nc.sync.dma_start(xt3[0][:, :, cs_], xtd3[0][:, :, cs_])
                nc.sync.dma_start(xt3[1][:, :, cs_], xtd3[1][:, :, cs_])
            nc.sync.dma_start(wq_sb[0][:, 1024:4096], wqh_d[:, 1024:4096])
            nc.sync.dma_start(wq_sb[1][:, 1024:4096], wql_d[:, 1024:4096])
            wo_all = wp.tile([128, 4 * 1024], bf16, tag="wo", bufs=1, name="wo_all")
            nc.sync.dma_start(wo_all[:], wo_d[:])

            # pair views for DoubleRow: [128, 2 k-subtiles, cols]
            wk3 = [t[:].rearrange("p (k j) -> p k j", k=DCH) for t in wk_sb]
            wv3 = [t[:].rearrange("p (k j) -> p k j", k=DCH) for t in wv_sb]
            wq3 = [t[:].rearrange("p (g j) -> p g j", g=4 * DCH) for t in wq_sb]

            def xpair(b, j, cs_):
                return xt3[b][:, 2 * j : 2 * j + 2, cs_]

            def wkpair(a, j):
                return wk3[a][:, 2 * j : 2 * j + 2, :]

            def wqpair(m, a, j):
                # m-major packing: group g = 8*m + k
                return wq3[a][:, 8 * m + 2 * j : 8 * m + 2 * j + 2, :]

            def wo(c):
                return wo_all[:, 1024 * c : 1024 * (c + 1)]

            PRODS = ((0, 0), (1, 0), (0, 1))  # (w hi/lo, x hi/lo)

            rmat_sb = cb[:, 0:128]
            tri_sb = cb[:, 128:384]
            ones_bf = cb[:, 384:392]

            pending = []      # head-phase rope tails: flushed whole
            pending_work = []  # steady-state closures: flushed 1 per kt

            def flush_pending():
                for f in pending:
                    f()
                pending.clear()

            def flush_work(n=1):
                for _ in range(n):
                    if not pending_work:
                        return
                    pending_work.pop(0)()

            # PE pstate warm-up: the cost model charges LOW/MID clocks to
            # matmuls decoded within 3us of an idle->busy transition, so
            # keep PE trivially busy across head-phase DMA waits.
            def warm(n, ring="psg"):
                if ring == "o":
                    wt = pso.tile([128, 512], f32, tag="o", bufs=2, name="warm")
                else:
                    wt = psg.tile([128, 512], f32, tag="psg", bufs=2, name="warm")
                for _ in range(n):
                    _mm(
                        nc,
                        wt[0:64, 0:64],
                        cb[:, 0:64],
                        cb[:, 0:64],
                        start=True,
                        stop=True,
                        skip_group_check=True,
                    )

            # ---- fused projection + RoPE -> fp8 DoubleRow layout ------
            # dr layout: [32 partitions, (2*head + khalf)*T + t]
            # The rope tail (rot matmul + cos/sin combine) of chunk qc is
            # deferred until after chunk qc+1's projection matmuls: the
            # tile scheduler is run-ahead in-order-with-skip per engine,
            # so an op emitted before its input is ready gets parked
            # until the engine idles (which PE never does).
            def project_rope(wpair, dr_dst, head=False, fold_each=False):
                q8full = q8p.tile([128, T], f8, tag="q8f", bufs=2, name="q8full")

                def make_tail(qc, ps):
                    cs_ = slice(512 * qc, 512 * (qc + 1))
                    dst = dstp.tile([128, 512], bf16, tag="dst", bufs=5, name="dst")
                    if head:
                        nc.scalar.copy(dst[:], ps[:])
                    else:
                        nc.vector.tensor_copy(dst[:], ps[:])
                    # cos-multiply needs only dst: run it right away on Pool
                    c1 = tmpp.tile([128, 512], f32, tag="c1", bufs=4, name="c1")
                    nc.gpsimd.tensor_tensor(c1[:], dst[:], cos_sb[:, cs_], mult)

                    def tail():
                        rot = psg.tile([128, 512], f32, tag="psg", bufs=2, name="ps_rot")
                        _mm(nc, rot[:], rmat_sb, dst[:], start=True, stop=True)
                        t1 = tmpp.tile([128, 512], f32, tag="t1", bufs=4, name="t1")
                        nc.vector.tensor_tensor(t1[:], rot[:], sin_sb[:, cs_], mult)
                        nc.vector.tensor_tensor(q8full[:, cs_], c1[:], t1[:], add)
                        # partition fold into DoubleRow layout. For head
                        # tiles: fold qc0 alone (lets attention start on
                        # partial K/Q) and qc1-3 in one batch (HWDGE issue
                        # slots are ~625ns each and get scarce in the head)
                        if fold_each and qc == 0:
                            for g in range(4):
                                nc.sync.dma_start(
                                    dr_dst[0:32, T * g : T * g + 512],
                                    q8full[32 * g : 32 * (g + 1), 0:512],
                                )
                        elif qc == QC - 1:
                            lo = 512 if fold_each else 0
                            for g in range(4):
                                nc.sync.dma_start(
                                    dr_dst[0:32, T * g + lo : T * (g + 1)],
                                    q8full[32 * g : 32 * (g + 1), lo:],
                                )

                    return tail

                prev_tail = None
                for qc in range(QC):
                    cs_ = slice(512 * qc, 512 * (qc + 1))
                    if head and qc % 2 == 0:
                        ps = pso.tile([128, 512], f32, tag="o", bufs=2, name="ps_proj")
                    else:
                        ps = psg.tile([128, 512], f32, tag="psg", bufs=2, name="ps_proj")
                    for j in range(DCH // 2):
                        for pi, (a, b) in enumerate(PRODS):
                            _mm(
                                nc,
                                ps[:],
                                wpair(a, j),
                                xpair(b, j, cs_),
                                start=(j == 0 and pi == 0),
                                stop=(j == DCH // 2 - 1 and pi == 2),
                                perf_mode=DR,
                                tile_position=(0, 0),
                            )
                    if qc == 0:
                        # previous projection's last rope tail rides right
                        # behind this chunk's matmuls on the PE queue
                        flush_pending()
                    if head:
                        warm(18)
                    if prev_tail is not None:
                        prev_tail()
                    prev_tail = make_tail(qc, ps)
                pending.append(prev_tail)

            v_all = vtp.tile([128, KT * 130], bf16, tag="v", bufs=1, name="v_all")
            v3 = v_all[:].rearrange("p (t c) -> p t c", t=KT)
            # V rows carry 64x-scaled V; ones row = 64 keeps num/den exact
            nc.vector.memset(v3[:, :, 64:65], 64.0)
            nc.vector.memset(v3[:, :, 129:130], 64.0)

            def v_proj(t, ring=None):
                if ring == "o":
                    ps = pso.tile([128, 512], f32, tag="o", bufs=2, name="ps_v")
                else:
                    ps = psg.tile([128, 512], f32, tag="psg", bufs=2, name="ps_v")
                for j in range(DCH // 2):
                    for pi, (a, b) in enumerate(PRODS):
                        _mm(
                            nc,
                            ps[:, 0:128],
                            xt3[b][:, 2 * j : 2 * j + 2, 128 * t : 128 * (t + 1)],
                            wv3[a][:, 2 * j : 2 * j + 2, :],
                            start=(j == 0 and pi == 0),
                            stop=(j == DCH // 2 - 1 and pi == 2),
                            perf_mode=DR,
                            tile_position=(0, 0),
                        )
                vt = v_all[:, 130 * t : 130 * (t + 1)]
                nc.vector.tensor_copy(vt[:, 0:64], ps[:, 0:64])
                nc.vector.tensor_copy(vt[:, 65:129], ps[:, 64:128])
                v_sb.append(vt)

            v_sb = []
            qT8 = []

            def q_proj(m, head=False):
                qt = q8p.tile([32, 4 * T], f8, tag="dr", bufs=5, name=f"qT8{m}")
                project_rope(
                    lambda a, j: wqpair(m, a, j), qt, head=head, fold_each=head
                )
                qT8.append(qt)

            def q_proj_deferred(m):
                """emit q-chunk m's projection as per-qc closures so the
                attention kt loop interleaves them 1.7us at a time."""
                qt = q8p.tile([32, 4 * T], f8, tag="dr", bufs=5, name=f"qT8{m}")
                qT8.append(qt)
                wsel = lambda a, j: wqpair(m, a, j)
                q8full = q8p.tile([128, T], f8, tag="q8f", bufs=2, name="q8full")
                state = {"tail": None}

                def make_qc(qc):
                    cs_ = slice(512 * qc, 512 * (qc + 1))

                    def go():
                        ps = psg.tile(
                            [128, 512], f32, tag="psg", bufs=2, name="ps_proj"
                        )
                        for j in range(DCH // 2):
                            for pi, (a, b) in enumerate(PRODS):
                                _mm(
                                    nc,
                                    ps[:],
                                    wsel(a, j),
                                    xpair(b, j, cs_),
                                    start=(j == 0 and pi == 0),
                                    stop=(j == DCH // 2 - 1 and pi == 2),
                                    perf_mode=DR,
                                    tile_position=(0, 0),
                                )
                        if state["tail"] is not None:
                            state["tail"]()
                        dst = dstp.tile(
                            [128, 512], bf16, tag="dst", bufs=5, name="dst"
                        )
                        nc.vector.tensor_copy(dst[:], ps[:])
                        c1 = tmpp.tile([128, 512], f32, tag="c1", bufs=4, name="c1")
                        nc.gpsimd.tensor_tensor(c1[:], dst[:], cos_sb[:, cs_], mult)

                        def tail():
                            rot = psg.tile(
                                [128, 512], f32, tag="psg", bufs=2, name="ps_rot"
                            )
                            _mm(nc, rot[:], rmat_sb, dst[:], start=True, stop=True)
                            t1 = tmpp.tile(
                                [128, 512], f32, tag="t1", bufs=4, name="t1"
                            )
                            nc.vector.tensor_tensor(
                                t1[:], rot[:], sin_sb[:, cs_], mult
                            )
                            nc.vector.tensor_tensor(q8full[:, cs_], c1[:], t1[:], add)
                            if qc == QC - 1:
                                for g in range(4):
                                    nc.sync.dma_start(
                                        qt[0:32, T * g : T * (g + 1)],
                                        q8full[32 * g : 32 * (g + 1), :],
                                    )

                        state["tail"] = tail

                    return go

                for qc in range(QC):
                    pending_work.append(make_qc(qc))
                pending_work.append(lambda: (state["tail"](), state.update(tail=None)))

            oT = [
                big.tile([128, T], bf16, tag="oT", bufs=4, name=f"oT{m}")
                for m in range(4)
            ]
            tri3 = tri_sb.rearrange("p (two q) -> p two q", two=2)
            # PSUM carries 64x-scaled Q/K (w*64 on host): S is 4096x
            escale = float(1.0 / np.sqrt(D_K)) / 4096.0

            def y_tile(t):
                """output projection for one 128-row token tile. nh=0 uses
                the 'o' psum ring, nh=1 the 'psg' ring (parallel banks)."""
                ty = ysbp.tile([128, 1024], bf16, tag="y", name="ty")
                for nh in range(2):
                    ps = psg.tile([128, 512], f32, tag="psg", bufs=2, name="ps_y")
                    for c in range(4):
                        _mm(
                            nc,
                            ps[:],
                            oT[c][:, 128 * t : 128 * (t + 1)],
                            wo(c)[:, 512 * nh : 512 * (nh + 1)],
                            start=(c == 0),
                            stop=(c == 3),
                        )
                    nc.vector.tensor_copy(ty[:, 512 * nh : 512 * (nh + 1)], ps[:])
                nc.sync.dma_start(y_d[128 * t : 128 * (t + 1), :], ty[:])

            # ---- attention unit (one q-chunk x one head-pair) ---------
            kv4 = [None]
            escale_f = escale

            def attn_unit(qp, hp):
                qsl = slice(512 * qp, 512 * (qp + 1))
                qv4 = qT8[hp][:].rearrange("p (f t) -> p f t", f=4)
                oA = pso.tile([128, 512], f32, tag="o", bufs=2, name="oA")
                oB = pso.tile([128, 512], f32, tag="o", bufs=2, name="oB")
                nkt = 4 * qp + 4
                for kt in range(nkt):
                    a = max(0, 128 * kt - 512 * qp)
                    s = pss.tile([128, 1024], f32, tag="s", name="s")
                    for h in range(2):
                        out_sl = s[:, a:512] if h == 0 else s[:, 512 + a : 1024]
                        _mm(
                            nc,
                            out_sl,
                            kv4[0][:, 2 * h : 2 * h + 2, 128 * kt : 128 * (kt + 1)],
                            qv4[:, 2 * h : 2 * h + 2, 512 * qp + a : 512 * (qp + 1)],
                            start=True,
                            stop=True,
                            perf_mode=DR,
                            tile_position=(0, 0),
                        )
                    e = ep.tile([128, 1024], bf16, tag="e", bufs=4, name="e")
                    if a:
                        sv = s[:].rearrange("p (two q) -> p two q", two=2)[:, :, a:512]
                        ev = e[:].rearrange("p (two q) -> p two q", two=2)[:, :, a:512]
                        nc.scalar.activation(
                            out=ev,
                            in_=sv,
                            func=mybir.ActivationFunctionType.Exp,
                            scale=escale_f,
                        )
                    else:
                        nc.scalar.activation(
                            out=e[:],
                            in_=s[:],
                            func=mybir.ActivationFunctionType.Exp,
                            scale=escale_f,
                        )
                    if kt >= 4 * qp:  # diagonal: causal triangle mask
                        o = 128 * kt - 512 * qp
                        e3 = e[:].rearrange("p (two q) -> p two q", two=2)[
                            :, :, o : o + 128
                        ]
                        nc.vector.tensor_tensor(e3, e3, tri3, mult)
                    st, sp = (kt == 0), (kt == nkt - 1)
                    _mm(
                        nc,
                        oA[0:65, a:512],
                        v_sb[kt][:, 0:65],
                        e[:, a:512],
                        start=st,
                        stop=sp,
                        skip_group_check=True,
                    )
                    _mm(
                        nc,
                        oB[0:65, a:512],
                        v_sb[kt][:, 65:130],
                        e[:, 512 + a : 1024],
                        start=st,
                        stop=sp,
                        skip_group_check=True,
                    )
                    if kt == 1:
                        flush_pending()
                    if kt >= 1:
                        flush_work(1)
                # evacuate O' right away (frees the oA/oB psum ring for the
                # next unit); the rest of the tail is deferred. The terminal
                # unit evacuates on ACT: it is idle after the last exp while
                # DVE still drains y-tile copies.
                oraw = rrp.tile([128, 1024], f32r, tag="rr", bufs=3, name="oraw")
                if qp == 3 and hp == 3:
                    nc.scalar.copy(oraw[0:65, 0:512], oA[0:65, :])
                    nc.scalar.copy(oraw[0:65, 512:1024], oB[0:65, :])
                else:
                    nc.vector.tensor_copy(oraw[0:65, 0:512], oA[0:65, :])
                    nc.vector.tensor_copy(oraw[0:65, 512:1024], oB[0:65, :])

                is_last = qp == 3 and hp == 3

                def tail():
                    rb = rbp.tile([128, 1024], f32, tag="rb", bufs=3, name="rb")
                    scr = drp.tile([1, 1024], f32r, tag="scr", name="scr")
                    nc.sync.dma_start(scr[:], oraw[64:65, :])
                    nc.sync.dma_start(
                        rb[0:64, :].bitcast(f32r), scr[:].to_broadcast((64, 1024))
                    )
                    nc.vector.reciprocal_approx_fast(rb[0:64, :], rb[0:64, :])
                    if is_last:
                        # terminal tail: normalize per 128-token slice so
                        # each y_tile launches as soon as its slice lands
                        nb = tmpp.tile([128, 512], bf16, tag="nb", bufs=3, name="nb")
                        for j in range(4):
                            js = slice(128 * j, 128 * (j + 1))
                            jq = slice(512 * qp + 128 * j, 512 * qp + 128 * (j + 1))
                            nc.vector.tensor_tensor(
                                oT[hp][0:64, jq],
                                oraw[0:64, js],
                                rb[0:64, js],
                                mult,
                            )
                            nc.vector.tensor_tensor(
                                nb[0:64, js],
                                oraw[0:64, 512 + 128 * j : 512 + 128 * (j + 1)],
                                rb[0:64, 512 + 128 * j : 512 + 128 * (j + 1)],
                                mult,
                            )
                            nc.sync.dma_start(oT[hp][64:128, jq], nb[0:64, js])
                            y_tile(4 * qp + j)
                        return
                    nc.vector.tensor_tensor(
                        oT[hp][0:64, qsl], oraw[0:64, 0:512], rb[0:64, 0:512], mult
                    )
                    nb = tmpp.tile([128, 512], bf16, tag="nb", bufs=3, name="nb")
                    nc.vector.tensor_tensor(
                        nb[0:64, :], oraw[0:64, 512:1024], rb[0:64, 512:1024], mult
                    )
                    # head B -> oT partitions 64-127 (partition-shift DMA)
                    nc.sync.dma_start(oT[hp][64:128, qsl], nb[0:64, :])
                    if hp == 3:
                        for j in range(4):
                            pending_work.append(lambda j=j: y_tile(4 * qp + j))

                pending_work.append(tail)

            # ---- emission order: saturate ACT early, spread q-projs ---
            warm(130)
            kT8 = q8p.tile([32, 4 * T], f8, tag="dr", bufs=5, name="kT8")
            project_rope(wkpair, kT8, head=True, fold_each=True)
            q_proj(0, head=True)
            for t in range(4):
                v_proj(t, ring="o")
            kv4[0] = kT8[:].rearrange("p (f t) -> p f t", f=4)
            flush_pending()
            attn_unit(0, 0)
            for t in range(4, 8):
                v_proj(t)
            attn_unit(1, 0)
            for t in range(8, 12):
                v_proj(t)
            q_proj_deferred(1)
            attn_unit(2, 0)
            for t in range(12, 16):
                v_proj(t)
            attn_unit(3, 0)
            attn_unit(0, 1)
            q_proj_deferred(2)
            attn_unit(1, 1)
            attn_unit(2, 1)
            attn_unit(3, 1)
            attn_unit(0, 2)
            q_proj_deferred(3)
            attn_unit(1, 2)
            attn_unit(0, 3)
            attn_unit(2, 2)
            attn_unit(1, 3)
            attn_unit(3, 2)
            attn_unit(2, 3)
            attn_unit(3, 3)
            warm(60, ring="o")
            flush_pending()
            while pending_work:
                flush_work(1)

    nc.compile()
    return nc


def _get_program():
    global _PROGRAM
    if _PROGRAM is None:
        _PROGRAM = _build_program()
    return _PROGRAM


def _host_tables():
    """cos/sin [128, T] (two stacked 64-row copies), R^T (lhsT), tri mask."""
    d = D_K
    inv_freq = 1.0 / (ROPE_BASE ** (np.arange(0, d, 2, dtype=np.float32) / d))
    ang = np.arange(T, dtype=np.float32)[:, None] * inv_freq[None, :]  # [T, 32]
    cos64 = np.repeat(np.cos(ang).astype(np.float32), 2, axis=1).T.copy()
    sin64 = np.repeat(np.sin(ang).astype(np.float32), 2, axis=1).T.copy()
    cos128 = np.ascontiguousarray(np.concatenate([cos64, cos64], axis=0))
    sin128 = np.ascontiguousarray(np.concatenate([sin64, sin64], axis=0))
    # rot = R @ q with rot[2i] = -q[2i+1], rot[2i+1] = q[2i]; pass lhsT = R^T
    R = np.zeros((128, 128), dtype=np.float32)
    for i in range(64):
        R[2 * i, 2 * i + 1] = -1.0
        R[2 * i + 1, 2 * i] = 1.0
    rmat = np.ascontiguousarray(R.T)
    tri = np.triu(np.ones((128, 128), dtype=np.float32))  # keep kk <= qq
    tri2 = np.ascontiguousarray(np.concatenate([tri, tri], axis=1))
    return cos128, sin128, rmat, tri2


def _head_perm():
    """chunk m holds local heads (m, m+4) -> permute Wq cols / Wo rows."""
    perm = []
    for m in range(4):
        perm.extend(range(64 * m, 64 * m + 64))
        perm.extend(range(64 * (m + 4), 64 * (m + 4) + 64))
    return np.array(perm)


def _pack_rows(a, rows_per_tile=128):
    """[N*128, C] -> [128, N*C]: tile k's rows become column block k."""
    n = a.shape[0] // rows_per_tile
    return np.ascontiguousarray(
        np.concatenate(
            [a[rows_per_tile * k : rows_per_tile * (k + 1)] for k in range(n)], axis=1
        )
    )


def _pack_wq_mmajor(a):
    """[1024, 512] -> [128, (m, k, 128)]: chunk m's k-tiles contiguous."""
    out = np.empty((128, 4 * 8 * 128), dtype=a.dtype)
    for m in range(4):
        for k in range(8):
            out[:, 1024 * m + 128 * k : 1024 * m + 128 * (k + 1)] = a[
                128 * k : 128 * (k + 1), 128 * m : 128 * (m + 1)
            ]
    return np.ascontiguousarray(out)


def _split8(a):
    """fp8e4m3 hi+lo split of an f32 array."""
    import ml_dtypes

    F8 = ml_dtypes.float8_e4m3
    hi = a.astype(F8)
    lo = (a - hi.astype(np.float32)).astype(F8)
    return hi, lo


def make_in_maps(x, Wq, Wk, Wv, Wo):
    import ml_dtypes

    bf = ml_dtypes.bfloat16
    cos128, sin128, rmat, tri2 = _host_tables()
    perm = _head_perm()
    constb = np.concatenate(
        [rmat, tri2, np.ones((128, 8), dtype=np.float32)], axis=1
    ).astype(bf)
    in_maps = []
    for c in range(N_CORES):
        b, hg = c // 2, c % 2
        xth, xtl = _split8(np.ascontiguousarray(x[b].T))
        wqh, wql = _split8(Wq[:, hg * 512 : (hg + 1) * 512][:, perm] * 64.0)
        wkh, wkl = _split8(Wk[:, hg * 128 : (hg + 1) * 128] * 64.0)
        wvh, wvl = _split8(Wv[:, hg * 128 : (hg + 1) * 128] * 64.0)
        in_maps.append(
            {
                "xtp8h": _pack_rows(xth),
                "xtp8l": _pack_rows(xtl),
                "wqp8h": _pack_wq_mmajor(wqh),
                "wqp8l": _pack_wq_mmajor(wql),
                "wkp8h": _pack_rows(wkh),
                "wkp8l": _pack_rows(wkl),
                "wvp8h": _pack_rows(wvh),
                "wvp8l": _pack_rows(wvl),
                "wop": _pack_rows(
                    Wo[hg * 512 : (hg + 1) * 512, :][perm, :].astype(bf)
                ),
                "constb": constb,
                "costab": cos128.astype(bf),
                "sintab": sin128.astype(bf),
            }
        )
    return in_maps


def kernel(x, attention_mask, Wq, Wk, Wv, Wo, _trace=False, _trace_kwargs=None):
    global LAST_RESULTS
    from concourse import bass_utils

    x = np.asarray(x, dtype=np.float32)
    Wq = np.asarray(Wq, dtype=np.float32)
    Wk = np.asarray(Wk, dtype=np.float32)
    Wv = np.asarray(Wv, dtype=np.float32)
    Wo = np.asarray(Wo, dtype=np.float32)

    nc = _get_program()
    in_maps = make_in_maps(x, Wq, Wk, Wv, Wo)
    res = bass_utils.run_bass_kernel_spmd(
        nc,
        in_maps,
        core_ids=list(range(N_CORES)),
        trace=_trace,
        **(_trace_kwargs or {}),
    )
    LAST_RESULTS = res

    y = np.zeros((B, T, D_MODEL), dtype=np.float32)
    for b in range(B):
        y[b] = np.asarray(res.results[2 * b]["y"], dtype=np.float32) + np.asarray(
            res.results[2 * b + 1]["y"], dtype=np.float32
        )

    # faithful handling of padded (attention_mask == 0) query rows: the
    # reference's mask makes those rows uniform attention over ALL keys.
    am = np.asarray(attention_mask)
    if not np.all(am == 1):
        rep = N_HEADS // NUM_KV_HEADS
        for b in range(B):
            rows = np.where(am[b] == 0)[0]
            if rows.size:
                V = x[b] @ Wv
                Vfull = np.repeat(
                    V.reshape(T, NUM_KV_HEADS, D_K), rep, axis=1
                ).reshape(T, D_MODEL)
                y[b, rows] = (Vfull.mean(axis=0) @ Wo)[None, :]
    return y


# revision 38
# speedup vs baseline: 1.2841x; 1.0054x over previous
"""Trainium2 Bass kernel for causal multi-head attention with RoPE + GQA.

Model: D_MODEL=1024, N_HEADS=16, NUM_KV_HEADS=4, D_K=64, B=4, T=2048.
Sharding (8 cores): core c -> batch b = c//2, head-group hg = c%2
(8 query heads / 2 kv heads per core). Each core computes a partial
output  y_partial = attn_out_local @ Wo[rows of its heads]  and the host
sums the two partials per batch (the tensor-parallel all-reduce happens
at gather time).

Perf design (cost-model driven; ACT exp ~147us is the floor engine):
  - All GEMMs bf16 (1 PE cycle/row) except S = K^T.T Q^T, which runs in
    fp8e4m3 MatmulPerfMode.DoubleRow (0.5 cycles/row) with the d_k=64
    contraction laid out [32 partitions, 2 k-subtiles]. fp32 PSUM
    accumulation everywhere; end-to-end rel_err ~5e-3 (gate 2e-2).
  - Attention is emitted HEAD-PAIR-MAJOR with the q-chunk projections
    interleaved, so ScalarE starts exp'ing ~23us in and stays saturated
    while PE computes the remaining projections underneath it.
  - Each (qp, hp) unit's normalization tail (denominator DRAM-bounce
    broadcast, reciprocal, scale, head-B partition-shift DMA, and the
    previous q-chunk's output projection) is DEFERRED into the next
    unit's kt loop so it never sits between PV and the next S matmul.
  - DMA instruction COUNT is precious (~625ns serialized descriptor-gen
    each): all host inputs are pre-packed for single contiguous DMAs,
    x^T is split qc-major in 4 so the first projections start ~4us in,
    cos/sin load once in bf16.
  - Engine placement: ACT = exp only; Pool = PSUM->SBUF staging + RoPE
    cos-mult; DVE = RoPE sin-mult/add, masking, reciprocal, normalize.

Formulation (features-on-partitions; x arrives host-transposed bf16):
  Q^T = Wq_s^T x^T  [512,2048]    K^T = Wk_s^T x^T  [128,2048]
  V'  = [x @ Wv_s | ones]
  RoPE q*cos + (R q)*sin, R applied by one 128x128 matmul; result
  written fp8e4 and DMA-folded to the DoubleRow [32, (2h+half)*T + t]
  layout.
  S^T = K^T_h.T Q^T_h (fp8 DoubleRow), E^T = exp(S^T/8) -> bf16 (ACT),
  causal triangle on DVE, O'^T = V'_h.T E^T (M=65; row 64 = softmax
  denominator), O^T = O'^T * recip(den), y = O^T.T Wo_s -> bf16 DMA.
Heads are paired (m, m+4) across the two kv groups; Wq columns / Wo
rows are permuted accordingly on the host.
"""

import numpy as np

D_MODEL = 1024
N_HEADS = 16
NUM_KV_HEADS = 4
D_K = 64
ROPE_BASE = 10000.0
B, T = 4, 2048
N_CORES = 8
KT = 16             # 128-row key tiles per sequence
QC = 4              # 512-col query chunks
DCH = 8             # 128-row feature (d_model) tiles

_PROGRAM = None     # cached compiled Bass program
LAST_RESULTS = None  # BassKernelResults of the most recent run


def _mm(nc, out, lhsT, rhs, **kw):
    nc.tensor.matmul(out, lhsT, rhs, **kw)


def _build_program():
    import concourse.mybir as mybir
    import concourse.tile as tile
    from concourse import bacc

    f32 = mybir.dt.float32
    f32r = mybir.dt.float32r
    bf16 = mybir.dt.bfloat16
    f8 = mybir.dt.float8e4
    nc = bacc.Bacc("TRN2", target_bir_lowering=False, debug=False)

    # every input pre-packed on host; x/w as fp8 hi+lo splits (w scaled
    # x64 on host to clear fp8's subnormal floor; compensated via the exp
    # scale and the V' ones value)
    xth_d = nc.dram_tensor("xtp8h", [128, DCH * T], f8, kind="ExternalInput")
    xtl_d = nc.dram_tensor("xtp8l", [128, DCH * T], f8, kind="ExternalInput")
    wqh_d = nc.dram_tensor("wqp8h", [128, DCH * 512], f8, kind="ExternalInput")
    wql_d = nc.dram_tensor("wqp8l", [128, DCH * 512], f8, kind="ExternalInput")
    wkh_d = nc.dram_tensor("wkp8h", [128, DCH * 128], f8, kind="ExternalInput")
    wkl_d = nc.dram_tensor("wkp8l", [128, DCH * 128], f8, kind="ExternalInput")
    wvh_d = nc.dram_tensor("wvp8h", [128, DCH * 128], f8, kind="ExternalInput")
    wvl_d = nc.dram_tensor("wvp8l", [128, DCH * 128], f8, kind="ExternalInput")
    wo_d = nc.dram_tensor("wop", [128, 4 * 1024], bf16, kind="ExternalInput")
    cb_d = nc.dram_tensor("constb", [128, 392], bf16, kind="ExternalInput")
    cos_d = nc.dram_tensor("costab", [128, T], bf16, kind="ExternalInput")
    sin_d = nc.dram_tensor("sintab", [128, T], bf16, kind="ExternalInput")
    y_d = nc.dram_tensor("y", [T, D_MODEL], bf16, kind="ExternalOutput")

    mult = mybir.AluOpType.mult
    add = mybir.AluOpType.add
    div = mybir.AluOpType.divide
    DR = mybir.MatmulPerfMode.DoubleRow

    with tile.TileContext(nc) as tc:
        with (
            tc.tile_pool(name="big", bufs=6) as big,
            tc.tile_pool(name="w", bufs=4) as wp,
            tc.tile_pool(name="const", bufs=1) as constp,
            tc.tile_pool(name="q8", bufs=5) as q8p,
            tc.tile_pool(name="vt", bufs=16) as vtp,
            tc.tile_pool(name="dst", bufs=5) as dstp,
            tc.tile_pool(name="tmp", bufs=4) as tmpp,
            tc.tile_pool(name="e", bufs=4) as ep,
            tc.tile_pool(name="rr", bufs=3) as rrp,
            tc.tile_pool(name="rb", bufs=1) as rbp,
            tc.tile_pool(name="ysb", bufs=2) as ysbp,
            tc.tile_pool(name="dr", bufs=2, space="DRAM") as drp,
            tc.tile_pool(name="ps_g", bufs=2, space="PSUM") as psg,
            tc.tile_pool(name="ps_s", bufs=2, space="PSUM") as pss,
            tc.tile_pool(name="ps_o", bufs=2, space="PSUM") as pso,
        ):
            # ---- input loads, ordered for earliest compute start ------
            cb = constp.tile([128, 392], bf16, tag="cb", name="cb")
            nc.sync.dma_start(cb[:], cb_d[:])
            xt_sb = [
                big.tile([128, DCH * T], f8, tag=f"xt{i}", bufs=1, name=f"xt8{i}")
                for i in range(2)
            ]
            xt3 = [t[:].rearrange("p (k t) -> p k t", k=DCH) for t in xt_sb]
            xtd3 = [
                d[:].rearrange("p (k t) -> p k t", k=DCH) for d in (xth_d, xtl_d)
            ]
            cs0 = slice(0, 512)
            nc.sync.dma_start(xt3[0][:, :, cs0], xtd3[0][:, :, cs0])
            nc.sync.dma_start(xt3[1][:, :, cs0], xtd3[1][:, :, cs0])
            wk_sb = [
                wp.tile([128, DCH * 128], f8, tag=f"wk{i}", bufs=1, name=f"wk8{i}")
                for i in range(2)
            ]
            nc.sync.dma_start(wk_sb[0][:], wkh_d[:])
            nc.sync.dma_start(wk_sb[1][:], wkl_d[:])
            wq_sb = [
                wp.tile([128, DCH * 512], f8, tag=f"wq{i}", bufs=1, name=f"wq8{i}")
                for i in range(2)
            ]
            nc.sync.dma_start(wq_sb[0][:, 0:1024], wqh_d[:, 0:1024])
            nc.sync.dma_start(wq_sb[1][:, 0:1024], wql_d[:, 0:1024])
            # cos/sin: first 512 cols early (qc0 rope is on the critical
            # path to the first exp), remainder after xt qc1
            cos_sb = constp.tile([128, T], bf16, tag="cos", name="cos_sb")
            nc.sync.dma_start(cos_sb[:, 0:512], cos_d[:, 0:512])
            sin_sb = constp.tile([128, T], bf16, tag="sin", name="sin_sb")
            nc.sync.dma_start(sin_sb[:, 0:512], sin_d[:, 0:512])
            wv_sb = [
                wp.tile([128, DCH * 128], f8, tag=f"wv{i}", bufs=1, name=f"wv8{i}")
                for i in range(2)
            ]
            nc.sync.dma_start(wv_sb[0][:], wvh_d[:])
            nc.sync.dma_start(wv_sb[1][:], wvl_d[:])
            for qc in range(1, QC):
                cs_ = slice(512 * qc, 512 * (qc + 1))
                nc.sync.dma_start(xt3[0][:, :, cs_], xtd3[0][:, :, cs_])
                nc.sync.dma_start(xt3[1][:, :, cs_], xtd3[1][:, :, cs_])
                if qc == 1:
                    nc.sync.dma_start(cos_sb[:, 512:], cos_d[:, 512:])
                    nc.sync.dma_start(sin_sb[:, 512:], sin_d[:, 512:])
            nc.sync.dma_start(wq_sb[0][:, 1024:4096], wqh_d[:, 1024:4096])
            nc.sync.dma_start(wq_sb[1][:, 1024:4096], wql_d[:, 1024:4096])
            wo_all = wp.tile([128, 4 * 1024], bf16, tag="wo", bufs=1, name="wo_all")
            nc.sync.dma_start(wo_all[:], wo_d[:])

            # pair views for DoubleRow: [128, 2 k-subtiles, cols]
            wk3 = [t[:].rearrange("p (k j) -> p k j", k=DCH) for t in wk_sb]
            wv3 = [t[:].rearrange("p (k j) -> p k j", k=DCH) for t in wv_sb]
            wq3 = [t[:].rearrange("p (g j) -> p g j", g=4 * DCH) for t in wq_sb]

            def xpair(b, j, cs_):
                return xt3[b][:, 2 * j : 2 * j + 2, cs_]

            def wkpair(a, j):
                return wk3[a][:, 2 * j : 2 * j + 2, :]

            def wqpair(m, a, j):
                # m-major packing: group g = 8*m + k
                return wq3[a][:, 8 * m + 2 * j : 8 * m + 2 * j + 2, :]

            def wo(c):
                return wo_all[:, 1024 * c : 1024 * (c + 1)]

            PRODS = ((0, 0), (1, 0), (0, 1))  # (w hi/lo, x hi/lo)

            rmat_sb = cb[:, 0:128]
            tri_sb = cb[:, 128:384]
            ones_bf = cb[:, 384:392]

            pending = []      # head-phase rope tails: flushed whole
            pending_work = []  # steady-state closures: flushed 1 per kt

            def flush_pending():
                for f in pending:
                    f()
                pending.clear()

            def flush_work(n=1):
                for _ in range(n):
                    if not pending_work:
                        return
                    pending_work.pop(0)()

            # PE pstate warm-up: the cost model charges LOW/MID clocks to
            # matmuls decoded within 3us of an idle->busy transition, so
            # keep PE trivially busy across head-phase DMA waits.
            def warm(n, ring="psg"):
                if ring == "o":
                    wt = pso.tile([128, 512], f32, tag="o", bufs=2, name="warm")
                else:
                    wt = psg.tile([128, 512], f32, tag="psg", bufs=2, name="warm")
                for _ in range(n):
                    _mm(
                        nc,
                        wt[0:64, 0:64],
                        cb[:, 0:64],
                        cb[:, 0:64],
                        start=True,
                        stop=True,
                        skip_group_check=True,
                    )

            # ---- fused projection + RoPE -> fp8 DoubleRow layout ------
            # dr layout: [32 partitions, (2*head + khalf)*T + t]
            # The rope tail (rot matmul + cos/sin combine) of chunk qc is
            # deferred until after chunk qc+1's projection matmuls: the
            # tile scheduler is run-ahead in-order-with-skip per engine,
            # so an op emitted before its input is ready gets parked
            # until the engine idles (which PE never does).
            def project_rope(wpair, dr_dst, head=False, fold_each=False):
                q8full = q8p.tile([128, T], f8, tag="q8f", bufs=2, name="q8full")

                def make_tail(qc, ps):
                    cs_ = slice(512 * qc, 512 * (qc + 1))
                    dst = dstp.tile([128, 512], bf16, tag="dst", bufs=5, name="dst")
                    if head:
                        nc.scalar.copy(dst[:], ps[:])
                    else:
                        nc.vector.tensor_copy(dst[:], ps[:])
                    # cos-multiply needs only dst: run it right away on Pool
                    c1 = tmpp.tile([128, 512], f32, tag="c1", bufs=4, name="c1")
                    nc.gpsimd.tensor_tensor(c1[:], dst[:], cos_sb[:, cs_], mult)

                    def tail():
                        rot = psg.tile([128, 512], f32, tag="psg", bufs=2, name="ps_rot")
                        _mm(nc, rot[:], rmat_sb, dst[:], start=True, stop=True)
                        t1 = tmpp.tile([128, 512], f32, tag="t1", bufs=4, name="t1")
                        nc.vector.tensor_tensor(t1[:], rot[:], sin_sb[:, cs_], mult)
                        nc.vector.tensor_tensor(q8full[:, cs_], c1[:], t1[:], add)
                        # partition fold into DoubleRow layout. For head
                        # tiles: fold qc0 alone (lets attention start on
                        # partial K/Q) and qc1-3 in one batch (HWDGE issue
                        # slots are ~625ns each and get scarce in the head)
                        if fold_each and qc == 0:
                            for g in range(4):
                                nc.sync.dma_start(
                                    dr_dst[0:32, T * g : T * g + 512],
                                    q8full[32 * g : 32 * (g + 1), 0:512],
                                )
                        elif qc == QC - 1:
                            lo = 512 if fold_each else 0
                            for g in range(4):
                                nc.sync.dma_start(
                                    dr_dst[0:32, T * g + lo : T * (g + 1)],
                                    q8full[32 * g : 32 * (g + 1), lo:],
                                )

                    return tail

                prev_tail = None
                for qc in range(QC):
                    cs_ = slice(512 * qc, 512 * (qc + 1))
                    if head and qc % 2 == 0:
                        ps = pso.tile([128, 512], f32, tag="o", bufs=2, name="ps_proj")
                    else:
                        ps = psg.tile([128, 512], f32, tag="psg", bufs=2, name="ps_proj")
                    for j in range(DCH // 2):
                        for pi, (a, b) in enumerate(PRODS):
                            _mm(
                                nc,
                                ps[:],
                                wpair(a, j),
                                xpair(b, j, cs_),
                                start=(j == 0 and pi == 0),
                                stop=(j == DCH // 2 - 1 and pi == 2),
                                perf_mode=DR,
                                tile_position=(0, 0),
                            )
                    if qc == 0:
                        # previous projection's last rope tail rides right
                        # behind this chunk's matmuls on the PE queue
                        flush_pending()
                    if head:
                        warm(18)
                    if prev_tail is not None:
                        prev_tail()
                    prev_tail = make_tail(qc, ps)
                pending.append(prev_tail)

            v_all = vtp.tile([128, KT * 130], bf16, tag="v", bufs=1, name="v_all")
            v3 = v_all[:].rearrange("p (t c) -> p t c", t=KT)
            # V rows carry 64x-scaled V; ones row = 64 keeps num/den exact
            nc.vector.memset(v3[:, :, 64:65], 64.0)
            nc.vector.memset(v3[:, :, 129:130], 64.0)

            def v_proj(t, ring=None):
                if ring == "o":
                    ps = pso.tile([128, 512], f32, tag="o", bufs=2, name="ps_v")
                else:
                    ps = psg.tile([128, 512], f32, tag="psg", bufs=2, name="ps_v")
                for j in range(DCH // 2):
                    for pi, (a, b) in enumerate(PRODS):
                        _mm(
                            nc,
                            ps[:, 0:128],
                            xt3[b][:, 2 * j : 2 * j + 2, 128 * t : 128 * (t + 1)],
                            wv3[a][:, 2 * j : 2 * j + 2, :],
                            start=(j == 0 and pi == 0),
                            stop=(j == DCH // 2 - 1 and pi == 2),
                            perf_mode=DR,
                            tile_position=(0, 0),
                        )
                vt = v_all[:, 130 * t : 130 * (t + 1)]
                nc.vector.tensor_copy(vt[:, 0:64], ps[:, 0:64])
                nc.vector.tensor_copy(vt[:, 65:129], ps[:, 64:128])
                v_sb.append(vt)

            v_sb = []
            qT8 = []

            def q_proj(m, head=False):
                qt = q8p.tile([32, 4 * T], f8, tag="dr", bufs=5, name=f"qT8{m}")
                project_rope(
                    lambda a, j: wqpair(m, a, j), qt, head=head, fold_each=head
                )
                qT8.append(qt)

            def q_proj_deferred(m):
                """emit q-chunk m's projection as per-qc closures so the
                attention kt loop interleaves them 1.7us at a time."""
                qt = q8p.tile([32, 4 * T], f8, tag="dr", bufs=5, name=f"qT8{m}")
                qT8.append(qt)
                wsel = lambda a, j: wqpair(m, a, j)
                q8full = q8p.tile([128, T], f8, tag="q8f", bufs=2, name="q8full")
                state = {"tail": None}

                def make_qc(qc):
                    cs_ = slice(512 * qc, 512 * (qc + 1))

                    def go():
                        ps = psg.tile(
                            [128, 512], f32, tag="psg", bufs=2, name="ps_proj"
                        )
                        for j in range(DCH // 2):
                            for pi, (a, b) in enumerate(PRODS):
                                _mm(
                                    nc,
                                    ps[:],
                                    wsel(a, j),
                                    xpair(b, j, cs_),
                                    start=(j == 0 and pi == 0),
                                    stop=(j == DCH // 2 - 1 and pi == 2),
                                    perf_mode=DR,
                                    tile_position=(0, 0),
                                )
                        if state["tail"] is not None:
                            state["tail"]()
                        dst = dstp.tile(
                            [128, 512], bf16, tag="dst", bufs=5, name="dst"
                        )
                        nc.vector.tensor_copy(dst[:], ps[:])
                        c1 = tmpp.tile([128, 512], f32, tag="c1", bufs=4, name="c1")
                        nc.gpsimd.tensor_tensor(c1[:], dst[:], cos_sb[:, cs_], mult)

                        def tail():
                            rot = psg.tile(
                                [128, 512], f32, tag="psg", bufs=2, name="ps_rot"
                            )
                            _mm(nc, rot[:], rmat_sb, dst[:], start=True, stop=True)
                            t1 = tmpp.tile(
                                [128, 512], f32, tag="t1", bufs=4, name="t1"
                            )
                            nc.vector.tensor_tensor(
                                t1[:], rot[:], sin_sb[:, cs_], mult
                            )
                            nc.vector.tensor_tensor(q8full[:, cs_], c1[:], t1[:], add)
                            if qc == QC - 1:
                                for g in range(4):
                                    nc.sync.dma_start(
                                        qt[0:32, T * g : T * (g + 1)],
                                        q8full[32 * g : 32 * (g + 1), :],
                                    )

                        state["tail"] = tail

                    return go

                for qc in range(QC):
                    pending_work.append(make_qc(qc))
                pending_work.append(lambda: (state["tail"](), state.update(tail=None)))

            oT = [
                big.tile([128, T], bf16, tag="oT", bufs=4, name=f"oT{m}")
                for m in range(4)
            ]
            tri3 = tri_sb.rearrange("p (two q) -> p two q", two=2)
            # PSUM carries 64x-scaled Q/K (w*64 on host): S is 4096x
            escale = float(1.0 / np.sqrt(D_K)) / 4096.0

            def y_tile(t):
                """output projection for one 128-row token tile. nh=0 uses
                the 'o' psum ring, nh=1 the 'psg' ring (parallel banks)."""
                ty = ysbp.tile([128, 1024], bf16, tag="y", name="ty")
                for nh in range(2):
                    ps = psg.tile([128, 512], f32, tag="psg", bufs=2, name="ps_y")
                    for c in range(4):
                        _mm(
                            nc,
                            ps[:],
                            oT[c][:, 128 * t : 128 * (t + 1)],
                            wo(c)[:, 512 * nh : 512 * (nh + 1)],
                            start=(c == 0),
                            stop=(c == 3),
                        )
                    nc.vector.tensor_copy(ty[:, 512 * nh : 512 * (nh + 1)], ps[:])
                nc.sync.dma_start(y_d[128 * t : 128 * (t + 1), :], ty[:])

            # ---- attention unit (one q-chunk x one head-pair) ---------
            kv4 = [None]
            escale_f = escale

            def attn_unit(qp, hp):
                qsl = slice(512 * qp, 512 * (qp + 1))
                qv4 = qT8[hp][:].rearrange("p (f t) -> p f t", f=4)
                oA = pso.tile([128, 512], f32, tag="o", bufs=2, name="oA")
                oB = pso.tile([128, 512], f32, tag="o", bufs=2, name="oB")
                nkt = 4 * qp + 4
                for kt in range(nkt):
                    a = max(0, 128 * kt - 512 * qp)
                    s = pss.tile([128, 1024], f32, tag="s", name="s")
                    for h in range(2):
                        out_sl = s[:, a:512] if h == 0 else s[:, 512 + a : 1024]
                        _mm(
                            nc,
                            out_sl,
                            kv4[0][:, 2 * h : 2 * h + 2, 128 * kt : 128 * (kt + 1)],
                            qv4[:, 2 * h : 2 * h + 2, 512 * qp + a : 512 * (qp + 1)],
                            start=True,
                            stop=True,
                            perf_mode=DR,
                            tile_position=(0, 0),
                        )
                    e = ep.tile([128, 1024], bf16, tag="e", bufs=4, name="e")
                    if a:
                        sv = s[:].rearrange("p (two q) -> p two q", two=2)[:, :, a:512]
                        ev = e[:].rearrange("p (two q) -> p two q", two=2)[:, :, a:512]
                        nc.scalar.activation(
                            out=ev,
                            in_=sv,
                            func=mybir.ActivationFunctionType.Exp,
                            scale=escale_f,
                        )
                    else:
                        nc.scalar.activation(
                            out=e[:],
                            in_=s[:],
                            func=mybir.ActivationFunctionType.Exp,
                            scale=escale_f,
                        )
                    if kt >= 4 * qp:  # diagonal: causal triangle mask
                        o = 128 * kt - 512 * qp
                        e3 = e[:].rearrange("p (two q) -> p two q", two=2)[
                            :, :, o : o + 128
                        ]
                        nc.vector.tensor_tensor(e3, e3, tri3, mult)
                    st, sp = (kt == 0), (kt == nkt - 1)
                    _mm(
                        nc,
                        oA[0:65, a:512],
                        v_sb[kt][:, 0:65],
                        e[:, a:512],
                        start=st,
                        stop=sp,
                        skip_group_check=True,
                    )
                    _mm(
                        nc,
                        oB[0:65, a:512],
                        v_sb[kt][:, 65:130],
                        e[:, 512 + a : 1024],
                        start=st,
                        stop=sp,
                        skip_group_check=True,
                    )
                    if kt == 1:
                        flush_pending()
                    if kt >= 1:
                        flush_work(1)
                # evacuate O' right away (frees the oA/oB psum ring for the
                # next unit); the rest of the tail is deferred. The terminal
                # unit evacuates on ACT: it is idle after the last exp while
                # DVE still drains y-tile copies.
                oraw = rrp.tile([128, 1024], f32r, tag="rr", bufs=3, name="oraw")
                if qp == 3 and hp == 3:
                    nc.scalar.copy(oraw[0:65, 0:512], oA[0:65, :])
                    nc.scalar.copy(oraw[0:65, 512:1024], oB[0:65, :])
                else:
                    nc.vector.tensor_copy(oraw[0:65, 0:512], oA[0:65, :])
                    nc.vector.tensor_copy(oraw[0:65, 512:1024], oB[0:65, :])

                is_last = qp == 3 and hp == 3

                def tail():
                    rb = rbp.tile([128, 1024], f32, tag="rb", bufs=3, name="rb")
                    scr = drp.tile([1, 1024], f32r, tag="scr", name="scr")
                    nc.sync.dma_start(scr[:], oraw[64:65, :])
                    nc.sync.dma_start(
                        rb[0:64, :].bitcast(f32r), scr[:].to_broadcast((64, 1024))
                    )
                    nc.vector.reciprocal_approx_fast(rb[0:64, :], rb[0:64, :])
                    if is_last:
                        # terminal tail: normalize per 128-token slice so
                        # each y_tile launches as soon as its slice lands
                        nb = tmpp.tile([128, 512], bf16, tag="nb", bufs=3, name="nb")
                        for j in range(4):
                            js = slice(128 * j, 128 * (j + 1))
                            jq = slice(512 * qp + 128 * j, 512 * qp + 128 * (j + 1))
                            nc.vector.tensor_tensor(
                                oT[hp][0:64, jq],
                                oraw[0:64, js],
                                rb[0:64, js],
                                mult,
                            )
                            nc.vector.tensor_tensor(
                                nb[0:64, js],
                                oraw[0:64, 512 + 128 * j : 512 + 128 * (j + 1)],
                                rb[0:64, 512 + 128 * j : 512 + 128 * (j + 1)],
                                mult,
                            )
                            nc.sync.dma_start(oT[hp][64:128, jq], nb[0:64, js])
                            y_tile(4 * qp + j)
                        return
                    nc.vector.tensor_tensor(
                        oT[hp][0:64, qsl], oraw[0:64, 0:512], rb[0:64, 0:512], mult
                    )
                    nb = tmpp.tile([128, 512], bf16, tag="nb", bufs=3, name="nb")
                    nc.vector.tensor_tensor(
                        nb[0:64, :], oraw[0:64, 512:1024], rb[0:64, 512:1024], mult
                    )
                    # head B -> oT partitions 64-127 (partition-shift DMA)
                    nc.sync.dma_start(oT[hp][64:128, qsl], nb[0:64, :])
                    if hp == 3:
                        for j in range(4):
                            pending_work.append(lambda j=j: y_tile(4 * qp + j))

                pending_work.append(tail)

            # ---- emission order: saturate ACT early, spread q-projs ---
            warm(130)
            kT8 = q8p.tile([32, 4 * T], f8, tag="dr", bufs=5, name="kT8")
            project_rope(wkpair, kT8, head=True, fold_each=True)
            q_proj(0, head=True)
            for t in range(4):
                v_proj(t, ring="o")
            kv4[0] = kT8[:].rearrange("p (f t) -> p f t", f=4)
            flush_pending()
            attn_unit(0, 0)
            for t in range(4, 8):
                v_proj(t)
            attn_unit(1, 0)
            for t in range(8, 12):
                v_proj(t)
            q_proj_deferred(1)
            attn_unit(2, 0)
            for t in range(12, 16):
                v_proj(t)
            attn_unit(3, 0)
            attn_unit(0, 1)
            q_proj_deferred(2)
            attn_unit(1, 1)
            attn_unit(2, 1)
            attn_unit(3, 1)
            attn_unit(0, 2)
            q_proj_deferred(3)
            attn_unit(1, 2)
            attn_unit(0, 3)
            attn_unit(2, 2)
            attn_unit(1, 3)
            attn_unit(3, 2)
            attn_unit(2, 3)
            attn_unit(3, 3)
            warm(60, ring="o")
            flush_pending()
            while pending_work:
                flush_work(1)

    nc.compile()
    return nc


def _get_program():
    global _PROGRAM
    if _PROGRAM is None:
        _PROGRAM = _build_program()
    return _PROGRAM


def _host_tables():
    """cos/sin [128, T] (two stacked 64-row copies), R^T (lhsT), tri mask."""
    d = D_K
    inv_freq = 1.0 / (ROPE_BASE ** (np.arange(0, d, 2, dtype=np.float32) / d))
    ang = np.arange(T, dtype=np.float32)[:, None] * inv_freq[None, :]  # [T, 32]
    cos64 = np.repeat(np.cos(ang).astype(np.float32), 2, axis=1).T.copy()
    sin64 = np.repeat(np.sin(ang).astype(np.float32), 2, axis=1).T.copy()
    cos128 = np.ascontiguousarray(np.concatenate([cos64, cos64], axis=0))
    sin128 = np.ascontiguousarray(np.concatenate([sin64, sin64], axis=0))
    # rot = R @ q with rot[2i] = -q[2i+1], rot[2i+1] = q[2i]; pass lhsT = R^T
    R = np.zeros((128, 128), dtype=np.float32)
    for i in range(64):
        R[2 * i, 2 * i + 1] = -1.0
        R[2 * i + 1, 2 * i] = 1.0
    rmat = np.ascontiguousarray(R.T)
    tri = np.triu(np.ones((128, 128), dtype=np.float32))  # keep kk <= qq
    tri2 = np.ascontiguousarray(np.concatenate([tri, tri], axis=1))
    return cos128, sin128, rmat, tri2


def _head_perm():
    """chunk m holds local heads (m, m+4) -> permute Wq cols / Wo rows."""
    perm = []
    for m in range(4):
        perm.extend(range(64 * m, 64 * m + 64))
        perm.extend(range(64 * (m + 4), 64 * (m + 4) + 64))
    return np.array(perm)


def _pack_rows(a, rows_per_tile=128):
    """[N*128, C] -> [128, N*C]: tile k's rows become column block k."""
    n = a.shape[0] // rows_per_tile
    return np.ascontiguousarray(
        np.concatenate(
            [a[rows_per_tile * k : rows_per_tile * (k + 1)] for k in range(n)], axis=1
        )
    )


def _pack_wq_mmajor(a):
    """[1024, 512] -> [128, (m, k, 128)]: chunk m's k-tiles contiguous."""
    out = np.empty((128, 4 * 8 * 128), dtype=a.dtype)
    for m in range(4):
        for k in range(8):
            out[:, 1024 * m + 128 * k : 1024 * m + 128 * (k + 1)] = a[
                128 * k : 128 * (k + 1), 128 * m : 128 * (m + 1)
            ]
    return np.ascontiguousarray(out)


def _split8(a):
    """fp8e4m3 hi+lo split of an f32 array."""
    import ml_dtypes

    F8 = ml_dtypes.float8_e4m3
    hi = a.astype(F8)
    lo = (a - hi.astype(np.float32)).astype(F8)
    return hi, lo


def make_in_maps(x, Wq, Wk, Wv, Wo):
    import ml_dtypes

    bf = ml_dtypes.bfloat16
    cos128, sin128, rmat, tri2 = _host_tables()
    perm = _head_perm()
    constb = np.concatenate(
        [rmat, tri2, np.ones((128, 8), dtype=np.float32)], axis=1
    ).astype(bf)
    in_maps = []
    for c in range(N_CORES):
        b, hg = c // 2, c % 2
        xth, xtl = _split8(np.ascontiguousarray(x[b].T))
        wqh, wql = _split8(Wq[:, hg * 512 : (hg + 1) * 512][:, perm] * 64.0)
        wkh, wkl = _split8(Wk[:, hg * 128 : (hg + 1) * 128] * 64.0)
        wvh, wvl = _split8(Wv[:, hg * 128 : (hg + 1) * 128] * 64.0)
        in_maps.append(
            {
                "xtp8h": _pack_rows(xth),
                "xtp8l": _pack_rows(xtl),
                "wqp8h": _pack_wq_mmajor(wqh),
                "wqp8l": _pack_wq_mmajor(wql),
                "wkp8h": _pack_rows(wkh),
                "wkp8l": _pack_rows(wkl),
                "wvp8h": _pack_rows(wvh),
                "wvp8l": _pack_rows(wvl),
                "wop": _pack_rows(
                    Wo[hg * 512 : (hg + 1) * 512, :][perm, :].astype(bf)
                ),
                "constb": constb,
                "costab": cos128.astype(bf),
                "sintab": sin128.astype(bf),
            }
        )
    return in_maps


def kernel(x, attention_mask, Wq, Wk, Wv, Wo, _trace=False, _trace_kwargs=None):
    global LAST_RESULTS
    from concourse import bass_utils

    x = np.asarray(x, dtype=np.float32)
    Wq = np.asarray(Wq, dtype=np.float32)
    Wk = np.asarray(Wk, dtype=np.float32)
    Wv = np.asarray(Wv, dtype=np.float32)
    Wo = np.asarray(Wo, dtype=np.float32)

    nc = _get_program()
    in_maps = make_in_maps(x, Wq, Wk, Wv, Wo)
    res = bass_utils.run_bass_kernel_spmd(
        nc,
        in_maps,
        core_ids=list(range(N_CORES)),
        trace=_trace,
        **(_trace_kwargs or {}),
    )
    LAST_RESULTS = res

    y = np.zeros((B, T, D_MODEL), dtype=np.float32)
    for b in range(B):
        y[b] = np.asarray(res.results[2 * b]["y"], dtype=np.float32) + np.asarray(
            res.results[2 * b + 1]["y"], dtype=np.float32
        )

    # faithful handling of padded (attention_mask == 0) query rows: the
    # reference's mask makes those rows uniform attention over ALL keys.
    am = np.asarray(attention_mask)
    if not np.all(am == 1):
        rep = N_HEADS // NUM_KV_HEADS
        for b in range(B):
            rows = np.where(am[b] == 0)[0]
            if rows.size:
                V = x[b] @ Wv
                Vfull = np.repeat(
                    V.reshape(T, NUM_KV_HEADS, D_K), rep, axis=1
                ).reshape(T, D_MODEL)
                y[b, rows] = (Vfull.mean(axis=0) @ Wo)[None, :]
    return y


# revision 42
# speedup vs baseline: 1.2870x; 1.0023x over previous
"""Trainium2 Bass kernel for causal multi-head attention with RoPE + GQA.

Model: D_MODEL=1024, N_HEADS=16, NUM_KV_HEADS=4, D_K=64, B=4, T=2048.
Sharding (8 cores): core c -> batch b = c//2, head-group hg = c%2
(8 query heads / 2 kv heads per core). Each core computes a partial
output  y_partial = attn_out_local @ Wo[rows of its heads]  and the host
sums the two partials per batch (the tensor-parallel all-reduce happens
at gather time).

Perf design (cost-model driven; ACT exp ~147us is the floor engine):
  - All GEMMs bf16 (1 PE cycle/row) except S = K^T.T Q^T, which runs in
    fp8e4m3 MatmulPerfMode.DoubleRow (0.5 cycles/row) with the d_k=64
    contraction laid out [32 partitions, 2 k-subtiles]. fp32 PSUM
    accumulation everywhere; end-to-end rel_err ~5e-3 (gate 2e-2).
  - Attention is emitted HEAD-PAIR-MAJOR with the q-chunk projections
    interleaved, so ScalarE starts exp'ing ~23us in and stays saturated
    while PE computes the remaining projections underneath it.
  - Each (qp, hp) unit's normalization tail (denominator DRAM-bounce
    broadcast, reciprocal, scale, head-B partition-shift DMA, and the
    previous q-chunk's output projection) is DEFERRED into the next
    unit's kt loop so it never sits between PV and the next S matmul.
  - DMA instruction COUNT is precious (~625ns serialized descriptor-gen
    each): all host inputs are pre-packed for single contiguous DMAs,
    x^T is split qc-major in 4 so the first projections start ~4us in,
    cos/sin load once in bf16.
  - Engine placement: ACT = exp only; Pool = PSUM->SBUF staging + RoPE
    cos-mult; DVE = RoPE sin-mult/add, masking, reciprocal, normalize.

Formulation (features-on-partitions; x arrives host-transposed bf16):
  Q^T = Wq_s^T x^T  [512,2048]    K^T = Wk_s^T x^T  [128,2048]
  V'  = [x @ Wv_s | ones]
  RoPE q*cos + (R q)*sin, R applied by one 128x128 matmul; result
  written fp8e4 and DMA-folded to the DoubleRow [32, (2h+half)*T + t]
  layout.
  S^T = K^T_h.T Q^T_h (fp8 DoubleRow), E^T = exp(S^T/8) -> bf16 (ACT),
  causal triangle on DVE, O'^T = V'_h.T E^T (M=65; row 64 = softmax
  denominator), O^T = O'^T * recip(den), y = O^T.T Wo_s -> bf16 DMA.
Heads are paired (m, m+4) across the two kv groups; Wq columns / Wo
rows are permuted accordingly on the host.
"""

import numpy as np

D_MODEL = 1024
N_HEADS = 16
NUM_KV_HEADS = 4
D_K = 64
ROPE_BASE = 10000.0
B, T = 4, 2048
N_CORES = 8
KT = 16             # 128-row key tiles per sequence
QC = 4              # 512-col query chunks
DCH = 8             # 128-row feature (d_model) tiles

_PROGRAM = None     # cached compiled Bass program
LAST_RESULTS = None  # BassKernelResults of the most recent run


def _mm(nc, out, lhsT, rhs, **kw):
    nc.tensor.matmul(out, lhsT, rhs, **kw)


def _build_program():
    import concourse.mybir as mybir
    import concourse.tile as tile
    from concourse import bacc

    f32 = mybir.dt.float32
    f32r = mybir.dt.float32r
    bf16 = mybir.dt.bfloat16
    f8 = mybir.dt.float8e4
    nc = bacc.Bacc("TRN2", target_bir_lowering=False, debug=False)

    # every input pre-packed on host; x/w as fp8 hi+lo splits (w scaled
    # x64 on host to clear fp8's subnormal floor; compensated via the exp
    # scale and the V' ones value)
    xth_d = nc.dram_tensor("xtp8h", [128, DCH * T], f8, kind="ExternalInput")
    xtl_d = nc.dram_tensor("xtp8l", [128, DCH * T], f8, kind="ExternalInput")
    wqh_d = nc.dram_tensor("wqp8h", [128, DCH * 512], f8, kind="ExternalInput")
    wql_d = nc.dram_tensor("wqp8l", [128, DCH * 512], f8, kind="ExternalInput")
    wkh_d = nc.dram_tensor("wkp8h", [128, DCH * 128], f8, kind="ExternalInput")
    wkl_d = nc.dram_tensor("wkp8l", [128, DCH * 128], f8, kind="ExternalInput")
    wvh_d = nc.dram_tensor("wvp8h", [128, DCH * 128], f8, kind="ExternalInput")
    wvl_d = nc.dram_tensor("wvp8l", [128, DCH * 128], f8, kind="ExternalInput")
    wo_d = nc.dram_tensor("wop", [128, 4 * 1024], bf16, kind="ExternalInput")
    cb_d = nc.dram_tensor("constb", [128, 392], bf16, kind="ExternalInput")
    cos_d = nc.dram_tensor("costab", [128, T], bf16, kind="ExternalInput")
    sin_d = nc.dram_tensor("sintab", [128, T], bf16, kind="ExternalInput")
    y_d = nc.dram_tensor("y", [T, D_MODEL], bf16, kind="ExternalOutput")

    mult = mybir.AluOpType.mult
    add = mybir.AluOpType.add
    div = mybir.AluOpType.divide
    DR = mybir.MatmulPerfMode.DoubleRow

    with tile.TileContext(nc) as tc:
        with (
            tc.tile_pool(name="big", bufs=6) as big,
            tc.tile_pool(name="w", bufs=4) as wp,
            tc.tile_pool(name="const", bufs=1) as constp,
            tc.tile_pool(name="q8", bufs=5) as q8p,
            tc.tile_pool(name="vt", bufs=16) as vtp,
            tc.tile_pool(name="dst", bufs=5) as dstp,
            tc.tile_pool(name="tmp", bufs=4) as tmpp,
            tc.tile_pool(name="e", bufs=4) as ep,
            tc.tile_pool(name="rr", bufs=3) as rrp,
            tc.tile_pool(name="rb", bufs=1) as rbp,
            tc.tile_pool(name="ysb", bufs=2) as ysbp,
            tc.tile_pool(name="dr", bufs=2, space="DRAM") as drp,
            tc.tile_pool(name="ps_g", bufs=2, space="PSUM") as psg,
            tc.tile_pool(name="ps_s", bufs=2, space="PSUM") as pss,
            tc.tile_pool(name="ps_o", bufs=2, space="PSUM") as pso,
        ):
            # ---- input loads, ordered for earliest compute start ------
            cb = constp.tile([128, 392], bf16, tag="cb", name="cb")
            nc.sync.dma_start(cb[:], cb_d[:])
            xt_sb = [
                big.tile([128, DCH * T], f8, tag=f"xt{i}", bufs=1, name=f"xt8{i}")
                for i in range(2)
            ]
            xt3 = [t[:].rearrange("p (k t) -> p k t", k=DCH) for t in xt_sb]
            xtd3 = [
                d[:].rearrange("p (k t) -> p k t", k=DCH) for d in (xth_d, xtl_d)
            ]
            cs0 = slice(0, 512)
            nc.sync.dma_start(xt3[0][:, :, cs0], xtd3[0][:, :, cs0])
            nc.sync.dma_start(xt3[1][:, :, cs0], xtd3[1][:, :, cs0])
            wk_sb = [
                wp.tile([128, DCH * 128], f8, tag=f"wk{i}", bufs=1, name=f"wk8{i}")
                for i in range(2)
            ]
            nc.sync.dma_start(wk_sb[0][:], wkh_d[:])
            nc.sync.dma_start(wk_sb[1][:], wkl_d[:])
            wq_sb = [
                wp.tile([128, DCH * 512], f8, tag=f"wq{i}", bufs=1, name=f"wq8{i}")
                for i in range(2)
            ]
            nc.sync.dma_start(wq_sb[0][:, 0:1024], wqh_d[:, 0:1024])
            nc.sync.dma_start(wq_sb[1][:, 0:1024], wql_d[:, 0:1024])
            # cos/sin: first 512 cols early (qc0 rope is on the critical
            # path to the first exp), remainder after xt qc1
            cos_sb = constp.tile([128, T], bf16, tag="cos", name="cos_sb")
            nc.sync.dma_start(cos_sb[:, 0:512], cos_d[:, 0:512])
            sin_sb = constp.tile([128, T], bf16, tag="sin", name="sin_sb")
            nc.sync.dma_start(sin_sb[:, 0:512], sin_d[:, 0:512])
            wv_sb = [
                wp.tile([128, DCH * 128], f8, tag=f"wv{i}", bufs=1, name=f"wv8{i}")
                for i in range(2)
            ]
            nc.sync.dma_start(wv_sb[0][:], wvh_d[:])
            nc.sync.dma_start(wv_sb[1][:], wvl_d[:])
            for qc in range(1, QC):
                cs_ = slice(512 * qc, 512 * (qc + 1))
                nc.sync.dma_start(xt3[0][:, :, cs_], xtd3[0][:, :, cs_])
                nc.sync.dma_start(xt3[1][:, :, cs_], xtd3[1][:, :, cs_])
                if qc == 1:
                    nc.sync.dma_start(cos_sb[:, 512:], cos_d[:, 512:])
                    nc.sync.dma_start(sin_sb[:, 512:], sin_d[:, 512:])
            nc.sync.dma_start(wq_sb[0][:, 1024:4096], wqh_d[:, 1024:4096])
            nc.sync.dma_start(wq_sb[1][:, 1024:4096], wql_d[:, 1024:4096])
            wo_all = wp.tile([128, 4 * 1024], bf16, tag="wo", bufs=1, name="wo_all")
            nc.sync.dma_start(wo_all[:], wo_d[:])

            # pair views for DoubleRow: [128, 2 k-subtiles, cols]
            wk3 = [t[:].rearrange("p (k j) -> p k j", k=DCH) for t in wk_sb]
            wv3 = [t[:].rearrange("p (k j) -> p k j", k=DCH) for t in wv_sb]
            wq3 = [t[:].rearrange("p (g j) -> p g j", g=4 * DCH) for t in wq_sb]

            def xpair(b, j, cs_):
                return xt3[b][:, 2 * j : 2 * j + 2, cs_]

            def wkpair(a, j):
                return wk3[a][:, 2 * j : 2 * j + 2, :]

            def wqpair(m, a, j):
                # m-major packing: group g = 8*m + k
                return wq3[a][:, 8 * m + 2 * j : 8 * m + 2 * j + 2, :]

            def wo(c):
                return wo_all[:, 1024 * c : 1024 * (c + 1)]

            PRODS = ((0, 0), (1, 0), (0, 1))  # (w hi/lo, x hi/lo)

            rmat_sb = cb[:, 0:128]
            tri_sb = cb[:, 128:384]
            ones_bf = cb[:, 384:392]

            pending = []      # head-phase rope tails: flushed whole
            pending_work = []  # steady-state closures: flushed 1 per kt

            def flush_pending():
                for f in pending:
                    f()
                pending.clear()

            def flush_work(n=1):
                for _ in range(n):
                    if not pending_work:
                        return
                    pending_work.pop(0)()

            # PE pstate warm-up: the cost model charges LOW/MID clocks to
            # matmuls decoded within 3us of an idle->busy transition, so
            # keep PE trivially busy across head-phase DMA waits.
            def warm(n, ring="psg"):
                if ring == "o":
                    wt = pso.tile([128, 512], f32, tag="o", bufs=2, name="warm")
                else:
                    wt = psg.tile([128, 512], f32, tag="psg", bufs=2, name="warm")
                for _ in range(n):
                    _mm(
                        nc,
                        wt[0:64, 0:64],
                        cb[:, 0:64],
                        cb[:, 0:64],
                        start=True,
                        stop=True,
                        skip_group_check=True,
                    )

            # ---- fused projection + RoPE -> fp8 DoubleRow layout ------
            # dr layout: [32 partitions, (2*head + khalf)*T + t]
            # The rope tail (rot matmul + cos/sin combine) of chunk qc is
            # deferred until after chunk qc+1's projection matmuls: the
            # tile scheduler is run-ahead in-order-with-skip per engine,
            # so an op emitted before its input is ready gets parked
            # until the engine idles (which PE never does).
            def project_rope(wpair, dr_dst, head=False, fold_each=False):
                q8full = q8p.tile([128, T], f8, tag="q8f", bufs=2, name="q8full")

                def make_tail(qc, ps):
                    cs_ = slice(512 * qc, 512 * (qc + 1))
                    dst = dstp.tile([128, 512], bf16, tag="dst", bufs=5, name="dst")
                    if head:
                        nc.scalar.copy(dst[:], ps[:])
                    else:
                        nc.vector.tensor_copy(dst[:], ps[:])
                    # cos-multiply needs only dst: run it right away on Pool
                    c1 = tmpp.tile([128, 512], f32, tag="c1", bufs=4, name="c1")
                    nc.gpsimd.tensor_tensor(c1[:], dst[:], cos_sb[:, cs_], mult)

                    def tail():
                        rot = psg.tile([128, 512], f32, tag="psg", bufs=2, name="ps_rot")
                        _mm(nc, rot[:], rmat_sb, dst[:], start=True, stop=True)
                        t1 = tmpp.tile([128, 512], f32, tag="t1", bufs=4, name="t1")
                        nc.vector.tensor_tensor(t1[:], rot[:], sin_sb[:, cs_], mult)
                        nc.vector.tensor_tensor(q8full[:, cs_], c1[:], t1[:], add)
                        # partition fold into DoubleRow layout. For head
                        # tiles: fold qc0 alone (lets attention start on
                        # partial K/Q) and qc1-3 in one batch (HWDGE issue
                        # slots are ~625ns each and get scarce in the head)
                        if fold_each and qc == 0:
                            for g in range(4):
                                nc.sync.dma_start(
                                    dr_dst[0:32, T * g : T * g + 512],
                                    q8full[32 * g : 32 * (g + 1), 0:512],
                                )
                        elif qc == QC - 1:
                            lo = 512 if fold_each else 0
                            for g in range(4):
                                nc.sync.dma_start(
                                    dr_dst[0:32, T * g + lo : T * (g + 1)],
                                    q8full[32 * g : 32 * (g + 1), lo:],
                                )

                    return tail

                prev_tail = None
                for qc in range(QC):
                    cs_ = slice(512 * qc, 512 * (qc + 1))
                    if head and qc % 2 == 0:
                        ps = pso.tile([128, 512], f32, tag="o", bufs=2, name="ps_proj")
                    else:
                        ps = psg.tile([128, 512], f32, tag="psg", bufs=2, name="ps_proj")
                    for j in range(DCH // 2):
                        for pi, (a, b) in enumerate(PRODS):
                            _mm(
                                nc,
                                ps[:],
                                wpair(a, j),
                                xpair(b, j, cs_),
                                start=(j == 0 and pi == 0),
                                stop=(j == DCH // 2 - 1 and pi == 2),
                                perf_mode=DR,
                                tile_position=(0, 0),
                            )
                    if qc == 0:
                        # previous projection's last rope tail rides right
                        # behind this chunk's matmuls on the PE queue
                        flush_pending()
                    if head:
                        warm(18)
                    if prev_tail is not None:
                        prev_tail()
                    prev_tail = make_tail(qc, ps)
                pending.append(prev_tail)

            v_all = vtp.tile([128, KT * 130], bf16, tag="v", bufs=1, name="v_all")
            v3 = v_all[:].rearrange("p (t c) -> p t c", t=KT)
            # V rows carry 64x-scaled V; ones row = 64 keeps num/den exact
            nc.vector.memset(v3[:, :, 64:65], 64.0)
            nc.vector.memset(v3[:, :, 129:130], 64.0)

            def v_proj(t, ring=None):
                if ring == "o":
                    ps = pso.tile([128, 512], f32, tag="o", bufs=2, name="ps_v")
                else:
                    ps = psg.tile([128, 512], f32, tag="psg", bufs=2, name="ps_v")
                for j in range(DCH // 2):
                    for pi, (a, b) in enumerate(PRODS):
                        _mm(
                            nc,
                            ps[:, 0:128],
                            xt3[b][:, 2 * j : 2 * j + 2, 128 * t : 128 * (t + 1)],
                            wv3[a][:, 2 * j : 2 * j + 2, :],
                            start=(j == 0 and pi == 0),
                            stop=(j == DCH // 2 - 1 and pi == 2),
                            perf_mode=DR,
                            tile_position=(0, 0),
                        )
                vt = v_all[:, 130 * t : 130 * (t + 1)]
                nc.vector.tensor_copy(vt[:, 0:64], ps[:, 0:64])
                nc.vector.tensor_copy(vt[:, 65:129], ps[:, 64:128])
                v_sb.append(vt)

            v_sb = []
            qT8 = []

            def q_proj(m, head=False):
                qt = q8p.tile([32, 4 * T], f8, tag="dr", bufs=5, name=f"qT8{m}")
                project_rope(
                    lambda a, j: wqpair(m, a, j), qt, head=head, fold_each=head
                )
                qT8.append(qt)

            def q_proj_deferred(m):
                """emit q-chunk m's projection as per-qc closures so the
                attention kt loop interleaves them 1.7us at a time."""
                qt = q8p.tile([32, 4 * T], f8, tag="dr", bufs=5, name=f"qT8{m}")
                qT8.append(qt)
                wsel = lambda a, j: wqpair(m, a, j)
                q8full = q8p.tile([128, T], f8, tag="q8f", bufs=2, name="q8full")
                state = {"tail": None}

                def make_qc(qc):
                    cs_ = slice(512 * qc, 512 * (qc + 1))

                    def go():
                        ps = psg.tile(
                            [128, 512], f32, tag="psg", bufs=2, name="ps_proj"
                        )
                        for j in range(DCH // 2):
                            for pi, (a, b) in enumerate(PRODS):
                                _mm(
                                    nc,
                                    ps[:],
                                    wsel(a, j),
                                    xpair(b, j, cs_),
                                    start=(j == 0 and pi == 0),
                                    stop=(j == DCH // 2 - 1 and pi == 2),
                                    perf_mode=DR,
                                    tile_position=(0, 0),
                                )
                        if state["tail"] is not None:
                            state["tail"]()
                        dst = dstp.tile(
                            [128, 512], bf16, tag="dst", bufs=5, name="dst"
                        )
                        nc.vector.tensor_copy(dst[:], ps[:])
                        c1 = tmpp.tile([128, 512], f32, tag="c1", bufs=4, name="c1")
                        nc.gpsimd.tensor_tensor(c1[:], dst[:], cos_sb[:, cs_], mult)

                        def tail():
                            rot = psg.tile(
                                [128, 512], f32, tag="psg", bufs=2, name="ps_rot"
                            )
                            _mm(nc, rot[:], rmat_sb, dst[:], start=True, stop=True)
                            t1 = tmpp.tile(
                                [128, 512], f32, tag="t1", bufs=4, name="t1"
                            )
                            nc.vector.tensor_tensor(
                                t1[:], rot[:], sin_sb[:, cs_], mult
                            )
                            nc.vector.tensor_tensor(q8full[:, cs_], c1[:], t1[:], add)
                            if qc == QC - 1:
                                for g in range(4):
                                    nc.sync.dma_start(
                                        qt[0:32, T * g : T * (g + 1)],
                                        q8full[32 * g : 32 * (g + 1), :],
                                    )

                        state["tail"] = tail

                    return go

                for qc in range(QC):
                    pending_work.append(make_qc(qc))
                pending_work.append(lambda: (state["tail"](), state.update(tail=None)))

            oT = [
                big.tile([128, T], bf16, tag="oT", bufs=4, name=f"oT{m}")
                for m in range(4)
            ]
            tri3 = tri_sb.rearrange("p (two q) -> p two q", two=2)
            # PSUM carries 64x-scaled Q/K (w*64 on host): S is 4096x
            escale = float(1.0 / np.sqrt(D_K)) / 4096.0

            def y_tile(t):
                """output projection for one 128-row token tile. nh=0 uses
                the 'o' psum ring, nh=1 the 'psg' ring (parallel banks)."""
                ty = ysbp.tile([128, 1024], bf16, tag="y", name="ty")
                for nh in range(2):
                    ps = psg.tile([128, 512], f32, tag="psg", bufs=2, name="ps_y")
                    for c in range(4):
                        _mm(
                            nc,
                            ps[:],
                            oT[c][:, 128 * t : 128 * (t + 1)],
                            wo(c)[:, 512 * nh : 512 * (nh + 1)],
                            start=(c == 0),
                            stop=(c == 3),
                        )
                    nc.vector.tensor_copy(ty[:, 512 * nh : 512 * (nh + 1)], ps[:])
                nc.sync.dma_start(y_d[128 * t : 128 * (t + 1), :], ty[:])

            # ---- attention unit (one q-chunk x one head-pair) ---------
            kv4 = [None]
            escale_f = escale

            def attn_unit(qp, hp):
                qsl = slice(512 * qp, 512 * (qp + 1))
                qv4 = qT8[hp][:].rearrange("p (f t) -> p f t", f=4)
                oA = pso.tile([128, 512], f32, tag="o", bufs=2, name="oA")
                oB = pso.tile([128, 512], f32, tag="o", bufs=2, name="oB")
                nkt = 4 * qp + 4
                for kt in range(nkt):
                    a = max(0, 128 * kt - 512 * qp)
                    s = pss.tile([128, 1024], f32, tag="s", name="s")
                    for h in range(2):
                        out_sl = s[:, a:512] if h == 0 else s[:, 512 + a : 1024]
                        _mm(
                            nc,
                            out_sl,
                            kv4[0][:, 2 * h : 2 * h + 2, 128 * kt : 128 * (kt + 1)],
                            qv4[:, 2 * h : 2 * h + 2, 512 * qp + a : 512 * (qp + 1)],
                            start=True,
                            stop=True,
                            perf_mode=DR,
                            tile_position=(0, 0),
                        )
                    e = ep.tile([128, 1024], bf16, tag="e", bufs=4, name="e")
                    if a:
                        sv = s[:].rearrange("p (two q) -> p two q", two=2)[:, :, a:512]
                        ev = e[:].rearrange("p (two q) -> p two q", two=2)[:, :, a:512]
                        nc.scalar.activation(
                            out=ev,
                            in_=sv,
                            func=mybir.ActivationFunctionType.Exp,
                            scale=escale_f,
                        )
                    else:
                        nc.scalar.activation(
                            out=e[:],
                            in_=s[:],
                            func=mybir.ActivationFunctionType.Exp,
                            scale=escale_f,
                        )
                    if kt >= 4 * qp:  # diagonal: causal triangle mask
                        o = 128 * kt - 512 * qp
                        e3 = e[:].rearrange("p (two q) -> p two q", two=2)[
                            :, :, o : o + 128
                        ]
                        nc.vector.tensor_tensor(e3, e3, tri3, mult)
                    st, sp = (kt == 0), (kt == nkt - 1)
                    _mm(
                        nc,
                        oA[0:65, a:512],
                        v_sb[kt][:, 0:65],
                        e[:, a:512],
                        start=st,
                        stop=sp,
                        skip_group_check=True,
                    )
                    _mm(
                        nc,
                        oB[0:65, a:512],
                        v_sb[kt][:, 65:130],
                        e[:, 512 + a : 1024],
                        start=st,
                        stop=sp,
                        skip_group_check=True,
                    )
                    if kt == 1:
                        flush_pending()
                    if kt >= 1 or nkt >= 12:
                        flush_work(1)
                # evacuate O' right away (frees the oA/oB psum ring for the
                # next unit); the rest of the tail is deferred. The terminal
                # unit evacuates on ACT: it is idle after the last exp while
                # DVE still drains y-tile copies.
                oraw = rrp.tile([128, 1024], f32r, tag="rr", bufs=3, name="oraw")
                if qp == 3 and hp == 3:
                    nc.scalar.copy(oraw[0:65, 0:512], oA[0:65, :])
                    nc.scalar.copy(oraw[0:65, 512:1024], oB[0:65, :])
                else:
                    nc.vector.tensor_copy(oraw[0:65, 0:512], oA[0:65, :])
                    nc.vector.tensor_copy(oraw[0:65, 512:1024], oB[0:65, :])

                is_last = qp == 3 and hp == 3

                def tail():
                    rb = rbp.tile([128, 1024], f32, tag="rb", bufs=3, name="rb")
                    scr = drp.tile([1, 1024], f32r, tag="scr", name="scr")
                    nc.sync.dma_start(scr[:], oraw[64:65, :])
                    nc.sync.dma_start(
                        rb[0:64, :].bitcast(f32r), scr[:].to_broadcast((64, 1024))
                    )
                    nc.vector.reciprocal_approx_fast(rb[0:64, :], rb[0:64, :])
                    if is_last:
                        # terminal tail: normalize per 128-token slice so
                        # each y_tile launches as soon as its slice lands
                        nb = tmpp.tile([128, 512], bf16, tag="nb", bufs=3, name="nb")
                        for j in range(4):
                            js = slice(128 * j, 128 * (j + 1))
                            jq = slice(512 * qp + 128 * j, 512 * qp + 128 * (j + 1))
                            nc.vector.tensor_tensor(
                                oT[hp][0:64, jq],
                                oraw[0:64, js],
                                rb[0:64, js],
                                mult,
                            )
                            nc.vector.tensor_tensor(
                                nb[0:64, js],
                                oraw[0:64, 512 + 128 * j : 512 + 128 * (j + 1)],
                                rb[0:64, 512 + 128 * j : 512 + 128 * (j + 1)],
                                mult,
                            )
                            nc.sync.dma_start(oT[hp][64:128, jq], nb[0:64, js])
                            y_tile(4 * qp + j)
                        return
                    nc.vector.tensor_tensor(
                        oT[hp][0:64, qsl], oraw[0:64, 0:512], rb[0:64, 0:512], mult
                    )
                    nb = tmpp.tile([128, 512], bf16, tag="nb", bufs=3, name="nb")
                    nc.vector.tensor_tensor(
                        nb[0:64, :], oraw[0:64, 512:1024], rb[0:64, 512:1024], mult
                    )
                    # head B -> oT partitions 64-127 (partition-shift DMA)
                    nc.sync.dma_start(oT[hp][64:128, qsl], nb[0:64, :])
                    if hp == 3:
                        for j in range(4):
                            pending_work.append(lambda j=j: y_tile(4 * qp + j))

                pending_work.append(tail)

            # ---- emission order: saturate ACT early, spread q-projs ---
            warm(130)
            kT8 = q8p.tile([32, 4 * T], f8, tag="dr", bufs=5, name="kT8")
            project_rope(wkpair, kT8, head=True, fold_each=True)
            q_proj(0, head=True)
            for t in range(4):
                v_proj(t, ring="o")
            kv4[0] = kT8[:].rearrange("p (f t) -> p f t", f=4)
            flush_pending()
            attn_unit(0, 0)
            for t in range(4, 8):
                v_proj(t)
            attn_unit(1, 0)
            for t in range(8, 12):
                v_proj(t)
            q_proj_deferred(1)
            attn_unit(2, 0)
            for t in range(12, 16):
                v_proj(t)
            attn_unit(3, 0)
            attn_unit(0, 1)
            q_proj_deferred(2)
            attn_unit(1, 1)
            attn_unit(2, 1)
            attn_unit(3, 1)
            attn_unit(0, 2)
            q_proj_deferred(3)
            attn_unit(1, 2)
            attn_unit(0, 3)
            attn_unit(2, 2)
            attn_unit(1, 3)
            attn_unit(3, 2)
            attn_unit(2, 3)
            attn_unit(3, 3)
            warm(60, ring="o")
            flush_pending()
            while pending_work:
                flush_work(1)

    nc.compile()
    return nc


def _get_program():
    global _PROGRAM
    if _PROGRAM is None:
        _PROGRAM = _build_program()
    return _PROGRAM


def _host_tables():
    """cos/sin [128, T] (two stacked 64-row copies), R^T (lhsT), tri mask."""
    d = D_K
    inv_freq = 1.0 / (ROPE_BASE ** (np.arange(0, d, 2, dtype=np.float32) / d))
    ang = np.arange(T, dtype=np.float32)[:, None] * inv_freq[None, :]  # [T, 32]
    cos64 = np.repeat(np.cos(ang).astype(np.float32), 2, axis=1).T.copy()
    sin64 = np.repeat(np.sin(ang).astype(np.float32), 2, axis=1).T.copy()
    cos128 = np.ascontiguousarray(np.concatenate([cos64, cos64], axis=0))
    sin128 = np.ascontiguousarray(np.concatenate([sin64, sin64], axis=0))
    # rot = R @ q with rot[2i] = -q[2i+1], rot[2i+1] = q[2i]; pass lhsT = R^T
    R = np.zeros((128, 128), dtype=np.float32)
    for i in range(64):
        R[2 * i, 2 * i + 1] = -1.0
        R[2 * i + 1, 2 * i] = 1.0
    rmat = np.ascontiguousarray(R.T)
    tri = np.triu(np.ones((128, 128), dtype=np.float32))  # keep kk <= qq
    tri2 = np.ascontiguousarray(np.concatenate([tri, tri], axis=1))
    return cos128, sin128, rmat, tri2


def _head_perm():
    """chunk m holds local heads (m, m+4) -> permute Wq cols / Wo rows."""
    perm = []
    for m in range(4):
        perm.extend(range(64 * m, 64 * m + 64))
        perm.extend(range(64 * (m + 4), 64 * (m + 4) + 64))
    return np.array(perm)


def _pack_rows(a, rows_per_tile=128):
    """[N*128, C] -> [128, N*C]: tile k's rows become column block k."""
    n = a.shape[0] // rows_per_tile
    return np.ascontiguousarray(
        np.concatenate(
            [a[rows_per_tile * k : rows_per_tile * (k + 1)] for k in range(n)], axis=1
        )
    )


def _pack_wq_mmajor(a):
    """[1024, 512] -> [128, (m, k, 128)]: chunk m's k-tiles contiguous."""
    out = np.empty((128, 4 * 8 * 128), dtype=a.dtype)
    for m in range(4):
        for k in range(8):
            out[:, 1024 * m + 128 * k : 1024 * m + 128 * (k + 1)] = a[
                128 * k : 128 * (k + 1), 128 * m : 128 * (m + 1)
            ]
    return np.ascontiguousarray(out)


def _split8(a):
    """fp8e4m3 hi+lo split of an f32 array."""
    import ml_dtypes

    F8 = ml_dtypes.float8_e4m3
    hi = a.astype(F8)
    lo = (a - hi.astype(np.float32)).astype(F8)
    return hi, lo


def make_in_maps(x, Wq, Wk, Wv, Wo):
    import ml_dtypes

    bf = ml_dtypes.bfloat16
    cos128, sin128, rmat, tri2 = _host_tables()
    perm = _head_perm()
    constb = np.concatenate(
        [rmat, tri2, np.ones((128, 8), dtype=np.float32)], axis=1
    ).astype(bf)
    in_maps = []
    for c in range(N_CORES):
        b, hg = c // 2, c % 2
        xth, xtl = _split8(np.ascontiguousarray(x[b].T))
        wqh, wql = _split8(Wq[:, hg * 512 : (hg + 1) * 512][:, perm] * 64.0)
        wkh, wkl = _split8(Wk[:, hg * 128 : (hg + 1) * 128] * 64.0)
        wvh, wvl = _split8(Wv[:, hg * 128 : (hg + 1) * 128] * 64.0)
        in_maps.append(
            {
                "xtp8h": _pack_rows(xth),
                "xtp8l": _pack_rows(xtl),
                "wqp8h": _pack_wq_mmajor(wqh),
                "wqp8l": _pack_wq_mmajor(wql),
                "wkp8h": _pack_rows(wkh),
                "wkp8l": _pack_rows(wkl),
                "wvp8h": _pack_rows(wvh),
                "wvp8l": _pack_rows(wvl),
                "wop": _pack_rows(
                    Wo[hg * 512 : (hg + 1) * 512, :][perm, :].astype(bf)
                ),
                "constb": constb,
                "costab": cos128.astype(bf),
                "sintab": sin128.astype(bf),
            }
        )
    return in_maps


def kernel(x, attention_mask, Wq, Wk, Wv, Wo, _trace=False, _trace_kwargs=None):
    global LAST_RESULTS
    from concourse import bass_utils

    x = np.asarray(x, dtype=np.float32)
    Wq = np.asarray(Wq, dtype=np.float32)
    Wk = np.asarray(Wk, dtype=np.float32)
    Wv = np.asarray(Wv, dtype=np.float32)
    Wo = np.asarray(Wo, dtype=np.float32)

    nc = _get_program()
    in_maps = make_in_maps(x, Wq, Wk, Wv, Wo)
    res = bass_utils.run_bass_kernel_spmd(
        nc,
        in_maps,
        core_ids=list(range(N_CORES)),
        trace=_trace,
        **(_trace_kwargs or {}),
    )
    LAST_RESULTS = res

    y = np.zeros((B, T, D_MODEL), dtype=np.float32)
    for b in range(B):
        y[b] = np.asarray(res.results[2 * b]["y"], dtype=np.float32) + np.asarray(
            res.results[2 * b + 1]["y"], dtype=np.float32
        )

    # faithful handling of padded (attention_mask == 0) query rows: the
    # reference's mask makes those rows uniform attention over ALL keys.
    am = np.asarray(attention_mask)
    if not np.all(am == 1):
        rep = N_HEADS // NUM_KV_HEADS
        for b in range(B):
            rows = np.where(am[b] == 0)[0]
            if rows.size:
                V = x[b] @ Wv
                Vfull = np.repeat(
                    V.reshape(T, NUM_KV_HEADS, D_K), rep, axis=1
                ).reshape(T, D_MODEL)
                y[b, rows] = (Vfull.mean(axis=0) @ Wo)[None, :]
    return y
